# revision 75
# baseline (speedup 1.0000x reference)
"""Causal self-attention (QK-RMSNorm + RoPE) Trainium2 kernel.

Sharding: 8 cores = 4 batches x 2 head-groups (Megatron-style over heads).
Core c handles batch b=c//2, heads [g*8, g*8+8) with g=c%2.
Each core computes y[b, :, g*512:(g+1)*512] (output-column sharding of the
projection after a pairwise AllGather of attention outputs), so the host
only concatenates slices - no host-side arithmetic.

Perf notes (cost model charges out-free-size rows per matmul, independent of
contraction depth and output-partition count):
- AV is computed in the [q-tokens(part), head-dims(free)] orientation with a
  ones column appended to V per head: the 65-wide moving tensor makes AV cost
  65 rows/tile instead of 128-512, and the softmax denominator accumulates
  for free in column 64.  The division is then a per-partition scalar
  multiply (DVE), and the output is transposed back to [dims, tok] with
  cheap PE transposes (128 rows each) for the AllGather + projection.
- The per-head sum-of-squares for QK-RMSNorm uses one block-diagonal-ones
  matmul covering both packed heads; squares are computed on DVE in bf16
  from a Pool-engine drain of the qkv psum (keeps the scalar engine free
  for the attention exp()s, which are its binding load).
- The in-order PE is kept saturated (and in max p-state) by interleaving
  filler matmuls into the attention loop, driven by a PE-vs-ACT issued-work
  ledger: qkv of the next pair, deferred v-proj tiles, and the partial
  projection of already-gathered f-tiles (staged per AllGather arrival,
  accumulated into a bf16 partial on the Pool engine).  Only the last
  pair's two f-tiles + a DVE add remain after the final AllGather.
- Transpose scratch lives in the s2 (scores) PSUM ring, so the AV
  accumulator ring is released by the division and never serializes
  consecutive q-chunks.
"""


import numpy as np
import ml_dtypes
from collections import deque
from contextlib import ExitStack

import concourse.bass as bass
import concourse.bacc as bacc

# Force all activations into the one table set that covers Exp+Ln+Square+
# Copy+Identity, so the whole kernel needs exactly one ACT_TABLE_LOAD.
import concourse.hw_specs as _hw_specs
_orig_gat = _hw_specs.get_activation_tables

def _gat_one_set(arch):
    t = _orig_gat(arch)
    return {k: (v if k == "natural_log_exp_and_others" else set())
            for k, v in t.items()}

bacc.get_activation_tables = _gat_one_set
import concourse.mybir as mybir
import concourse.tile as tile
from concourse.bass_utils import run_bass_kernel_spmd

BF16 = mybir.dt.bfloat16
F32 = mybir.dt.float32

N_HEAD = 16
HEAD_DIM = 64
EPS = 1e-5
ROPE_BASE = 10000.0

B, T, C = 4, 2048, 1024
H_LOCAL = N_HEAD // 2          # heads per core
PAIRS = H_LOCAL // 2           # head-pairs per core (processed 2-at-a-time)
CT = C // 128                  # contraction tiles over C
FL = H_LOCAL * HEAD_DIM        # local feature width (512)
QCH = 512                      # q-chunk width
NQC = T // QCH                 # q-chunks
NKT = T // 128                 # k tiles
NTT = T // 128                 # token tiles
VW = 2 * (HEAD_DIM + 1)        # per-pair v2 width: [A dims|onesA|B dims|onesB]

_cached = {}


def _reap(ap, dims):
    """Rebuild an AP keeping tensor/offset/partition dim, with free dims
    `dims` given as (stride, size) pairs."""
    return bass.AP(tensor=ap.tensor, offset=ap.offset,
                   ap=[ap.ap[0]] + [list(d) for d in dims])


def _fbcast2(ap):
    """[128, N] AP -> [128, 2, N] with the middle (free) dim broadcast."""
    return bass.AP(
        tensor=ap.tensor, offset=ap.offset, ap=[ap.ap[0], [0, 2], ap.ap[1]]
    )


def _rope_tables():
    inv_freq = 1.0 / (ROPE_BASE ** (np.arange(0, HEAD_DIM, 2, dtype=np.float64) / HEAD_DIM))
    t = np.arange(T, dtype=np.float64)
    freqs = np.outer(t, inv_freq)                       # [T, 32]
    emb = np.concatenate([freqs, freqs], -1)            # [T, 64]
    cos = np.cos(emb).astype(np.float32).T              # [64, T]
    sin = np.sin(emb).astype(np.float32).T              # [64, T]
    cos2 = np.concatenate([cos, cos], 0)                # [128, T] two heads
    sin_s = sin.copy()
    sin_s[0:32] = -sin_s[0:32]                          # rotate-half sign
    sin2 = np.concatenate([sin_s, sin_s], 0)            # [128, T]
    return cos2.astype(ml_dtypes.bfloat16), sin2.astype(ml_dtypes.bfloat16)


def _diag_masks():
    # corner mask: keep where k_partition <= q_col (lower-triangular 128x128)
    p = np.arange(128)[:, None]
    qf = np.arange(128)[None, :]
    m = (p <= qf).astype(np.float32)
    return m.astype(ml_dtypes.bfloat16)                 # [128, 128]


def build_program(no_cc=False):
    nc = bacc.Bacc("TRN2", target_bir_lowering=False, debug=False,
                   num_devices=1 if no_cc else 8)

    xT_d = nc.dram_tensor("xT", [C, T], BF16, kind="ExternalInput")
    wq_d = nc.dram_tensor("Wq", [C, FL], BF16, kind="ExternalInput")
    wk_d = nc.dram_tensor("Wk", [C, FL], BF16, kind="ExternalInput")
    wv_d = nc.dram_tensor("Wv", [C, FL], BF16, kind="ExternalInput")
    wp_d = nc.dram_tensor("Wp", [C, FL], BF16, kind="ExternalInput")
    y_d = nc.dram_tensor("y", [T, FL], F32, kind="ExternalOutput")

    cos2_np, sin2_np = _rope_tables()
    cos_d = nc.inline_tensor(np.ascontiguousarray(cos2_np), "cos2")
    sin_d = nc.inline_tensor(np.ascontiguousarray(sin2_np), "sin2")
    # causal mask as a score bias: out[p,g,f] += mneg[f,p] = -30000*(p>f),
    # added to the diagonal 128x128 block by one PE matmul (keeps the
    # exp->AV chain off the vector engine)
    mneg_np = -30000.0 * (np.arange(128)[None, :] > np.arange(128)[:, None])
    mneg_d = nc.inline_tensor(
        np.ascontiguousarray(mneg_np.astype(ml_dtypes.bfloat16)), "mneg")
    id2_np = np.tile(np.eye(128, dtype=ml_dtypes.bfloat16), (1, 2))
    ident2_d = nc.inline_tensor(np.ascontiguousarray(id2_np), "ident2")
    bd_np = np.zeros((128, 128), dtype=ml_dtypes.bfloat16)
    bd_np[0:64, 0:64] = 1.0
    bd_np[64:128, 64:128] = 1.0
    onesbd_d = nc.inline_tensor(np.ascontiguousarray(bd_np), "onesbd")
    ident_d = nc.inline_tensor(
        np.ascontiguousarray(np.eye(128, dtype=ml_dtypes.bfloat16)), "ident")

    # per-pair exchange buffers
    cc_ins = [nc.dram_tensor(f"cc_in{p}", [128, T], BF16) for p in range(PAIRS)]
    cc_outs = [nc.dram_tensor(f"cc_out{p}", [2, 128, T], BF16) for p in range(PAIRS)]

    # --- PE-vs-ACT issued-work ledger (units: bf16 matmul rows = 0.4167ns) ---
    led = {"pe": 0.0, "act": 0.0}

    def mm_cost(rows):
        led["pe"] += rows

    def act_cost(free):
        led["act"] += 2.0 * free + 444.0

    with tile.TileContext(nc) as tc:
        with (
            tc.tile_pool(name="const", bufs=1) as const,
            tc.tile_pool(name="work", bufs=2) as work,
            tc.tile_pool(name="evw", bufs=6) as evw,
            tc.tile_pool(name="rope", bufs=2) as ropep,
            tc.tile_pool(name="pt", bufs=5) as ptp,
            tc.tile_pool(name="qraw", bufs=3) as qrawp,
            tc.tile_pool(name="ao", bufs=6) as aop,
            tc.tile_pool(name="ps_s2", bufs=2, space="PSUM") as ps_s2,
            tc.tile_pool(name="ps_y", bufs=2, space="PSUM") as ps_y,
            tc.tile_pool(name="ps_mm", bufs=2, space="PSUM") as ps_mm,
        ):
            early_ctx = ExitStack()
            early = early_ctx.enter_context(tc.tile_pool(name="early", bufs=1))

            # ---- inputs: wq/wk/xT first (qkv(0) consumes them first) ----
            wq_sb = early.tile([128, CT, FL], BF16)
            wk_sb = early.tile([128, CT, FL], BF16)
            xT_sb = early.tile([128, CT, T], BF16)
            # few, large DMAs: the HWDGE serializes ~625ns per DMA instruction.
            # xT halved with first halves first: qkv chunk 0/1 start earlier.
            nc.sync.dma_start(wq_sb[:], wq_d[:].rearrange("(k p) f -> p k f", p=128))
            for k in range(CT):
                nc.sync.dma_start(xT_sb[:, k, 0:T // 2],
                                  xT_d[k * 128:(k + 1) * 128, 0:T // 2])
            # wk after xT half-0: the primed q-mtile only needs wq + xT
            nc.sync.dma_start(wk_sb[:], wk_d[:].rearrange("(k p) f -> p k f", p=128))
            for k in range(CT):
                nc.sync.dma_start(xT_sb[:, k, T // 2:T],
                                  xT_d[k * 128:(k + 1) * 128, T // 2:T])
            wv_sb = early.tile([128, CT, FL], BF16)
            nc.sync.dma_start(wv_sb[:], wv_d[:].rearrange("(k p) f -> p k f", p=128))
            cos_sb = early.tile([128, T], BF16)
            nc.sync.dma_start(cos_sb[:], cos_d[:])
            sin_sb = early.tile([128, T], BF16)
            nc.sync.dma_start(sin_sb[:], sin_d[:])
            mneg_sb = const.tile([128, 128], BF16)
            nc.sync.dma_start(mneg_sb[:], mneg_d[:])
            ident2_sb = const.tile([128, 256], BF16)
            nc.sync.dma_start(ident2_sb[:], ident2_d[:])
            ident_sb = const.tile([128, 128], BF16)
            nc.sync.dma_start(ident_sb[:], ident_d[:])
            wp_sb = const.tile([128, CT, FL], BF16)
            nc.sync.dma_start(wp_sb[:], wp_d[:].rearrange("(k p) f -> p k f", p=128))

            # block-diagonal ones: one matmul sums squares of both packed heads
            ones_bd = const.tile([128, 128], BF16)
            nc.sync.dma_start(ones_bd[:], onesbd_d[:])

            qk_sb = const.tile([128, 2 * PAIRS, T], BF16)
            # v2: per tt, per pair: [A dims(64) | onesA | B dims(64) | onesB]
            v2_sb = const.tile([128, NTT, PAIRS * VW], BF16)
            aoT_sb = const.tile([128, PAIRS, T], BF16)
            # bf16 partial of the projection over already-gathered f-tiles
            ppart = const.tile([128, NTT, FL], BF16)

            # ones columns of v2 (once, before any v-proj writes)
            pstr = v2_sb.ap[0][0]
            ones_ap = bass.AP(
                tensor=v2_sb.tensor, offset=v2_sb.offset + HEAD_DIM,
                ap=[[pstr, 128], [PAIRS * VW, NTT], [VW, PAIRS], [HEAD_DIM + 1, 2]])
            nc.vector.memset(ones_ap, 1.0)

            # ---------------- unit generators (PE fillers) ----------------
            def v_unit(tt):
                pss = ps_mm.tile([128, FL], F32, tag="mm", name=f"vp{tt}")
                for k0 in range(0, CT, 2):
                    for k in (k0, k0 + 1):
                        nc.tensor.matmul(
                            pss,
                            lhsT=xT_sb[:, k, tt * 128:(tt + 1) * 128],
                            rhs=wv_sb[:, k, :],
                            start=(k == 0), stop=(k == CT - 1),
                        )
                        mm_cost(FL)
                    yield
                # drain into v2 layout: [128,4,2,64] both sides (DVE --
                # GPSIMD cannot touch PSUM on real hardware)
                src = _reap(pss, [(128, PAIRS), (64, 2), (1, 64)])
                dst = bass.AP(tensor=v2_sb.tensor,
                              offset=v2_sb.offset + tt * PAIRS * VW,
                              ap=[[pstr, 128], [VW, PAIRS], [HEAD_DIM + 1, 2], [1, 64]])
                nc.vector.tensor_copy(dst, src)
                yield

            def qkv_tail(m, pss, sq, n):
                ss = ps_y.tile([128, QCH], F32, tag="y", name=f"ss{m}_{n}")
                nc.tensor.matmul(ss, lhsT=ones_bd[:], rhs=sq[:],
                                 start=True, stop=True, skip_group_check=True)
                mm_cost(QCH)
                # rstd = (ss/64)^(-1/2) = exp(-0.5*ln(ss/64)); eps negligible.
                rr = work.tile([128, QCH], BF16, tag="rr")
                nc.scalar.activation(rr[:], ss,
                                     mybir.ActivationFunctionType.Ln,
                                     scale=1.0 / HEAD_DIM)
                act_cost(QCH)
                rstd = work.tile([128, QCH], BF16, tag="rs")
                nc.scalar.activation(rstd[:], rr[:],
                                     mybir.ActivationFunctionType.Exp,
                                     scale=-0.5)
                act_cost(QCH)
                nc.vector.tensor_mul(qk_sb[:, m, n * QCH:(n + 1) * QCH],
                                     pss, rstd[:])

            # lazy rope: one closure per DVE op, drained one per qkv yield so
            # the qkv norm-muls (psum release) never queue behind a burst
            ropeq = deque()

            def rope_mtile(m):
                src = qk_sb[:, m, :]
                sw = ropep.tile([128, T], BF16, tag="rp")
                for off in (0, 64):
                    ropeq.append(lambda o=off: nc.vector.tensor_copy(
                        sw[o:o + 32, :], src[o + 32:o + 64, :]))
                    ropeq.append(lambda o=off: nc.vector.tensor_copy(
                        sw[o + 32:o + 64, :], src[o:o + 32, :]))
                ropeq.append(lambda: nc.vector.tensor_mul(sw[:], sw[:], sin_sb[:]))
                ropeq.append(lambda: nc.vector.tensor_mul(src, src, cos_sb[:]))
                ropeq.append(lambda: nc.vector.tensor_add(src, src, sw[:]))

            def qkv_unit(m):
                # pss borrows the (pre-phase-idle) s2 slots, ss the y slots:
                # the qkv pipeline gets baseline's 4-bank depth while the
                # attention pools keep their static reservation.
                # private 2-deep pss ring per generator (q-mtiles borrow the
                # pre-phase-idle s2 slots, k-mtiles the mm slots): a slot is
                # reused only 2 chunks later, far past its tail chain
                w_sb = wq_sb if m < PAIRS else wk_sb
                mloc = (m % PAIRS) * 128
                pend = None
                ps2 = None
                for n in range(NQC):
                    if m < PAIRS:
                        # 4-deep ring: two chunks per 2-bank s2 slot
                        if n % 2 == 0:
                            ps2 = ps_s2.tile([128, 2, QCH], F32, tag="s2",
                                             name=f"qk{m}_{n}")
                        pss = ps2[:, n % 2, :]
                    else:
                        pss = ps_mm.tile([128, QCH], F32, tag="mm",
                                         name=f"qk{m}_{n}")
                    for k0 in range(0, CT, 2):
                        for k in (k0, k0 + 1):
                            nc.tensor.matmul(
                                pss,
                                lhsT=w_sb[:, k, mloc:mloc + 128],
                                rhs=xT_sb[:, k, n * QCH:(n + 1) * QCH],
                                start=(k == 0), stop=(k == CT - 1),
                                skip_group_check=True,
                            )
                            mm_cost(QCH)
                        yield
                    # square must be ACT: DVE cannot read two PSUM operands
                    sq = work.tile([128, QCH], BF16, tag="sq")
                    nc.scalar.activation(sq[:], pss,
                                         mybir.ActivationFunctionType.Square)
                    act_cost(QCH)
                    if pend is not None:
                        qkv_tail(m, *pend)
                        yield
                    pend = (pss, sq, n)
                qkv_tail(m, *pend)
                yield
                rope_mtile(m)

            def qkv_pair_gen(pn):
                # alternate q- and k-mtile steps (q primed 2 chunks ahead so
                # the two tails never bunch at the same boundary)
                a, b = qkv_unit(pn), qkv_unit(PAIRS + pn)
                for _ in range(8):
                    try:
                        next(a)
                    except StopIteration:
                        break
                while True:
                    na = nb = False
                    try:
                        next(a)
                    except StopIteration:
                        na = True
                    if ropeq:
                        ropeq.popleft()()
                    try:
                        next(b)
                    except StopIteration:
                        nb = True
                    if ropeq:
                        ropeq.popleft()()
                    if na and nb:
                        return
                    yield

            def proj_stage_gen(kfs, first_stage):
                for tt in range(NTT):
                    pss = ps_mm.tile([128, FL], F32, tag="mm",
                                     name=f"pj{kfs[0]}_{tt}")
                    for i, kf in enumerate(kfs):
                        nc.tensor.matmul(
                            pss,
                            lhsT=qk_sb[:, kf, tt * 128:(tt + 1) * 128],
                            rhs=wp_sb[:, kf, :],
                            start=(i == 0), stop=(i == len(kfs) - 1),
                        )
                        mm_cost(FL)
                        yield
                    pp = ppart[:, tt, :]
                    if first_stage:
                        nc.vector.tensor_copy(pp, pss)
                    else:
                        nc.vector.tensor_add(pp, pss, pp)
                    yield

            # ---------------- filler machinery ----------------
            # list of [label, generator, done]; fill_step picks the first
            # not-done generator whose gate is open (order = priority).
            fillers = []
            gate_from = {}   # label -> cc whose writeback must be issued
            cc_issued = set()
            cur_pair = [0]

            def fill_step():
                for ent in fillers:
                    if ent[2] or gate_from.get(ent[0], -1) not in cc_issued | {-1}:
                        continue
                    try:
                        next(ent[1])
                        return True
                    except StopIteration:
                        ent[2] = True
                        continue
                return False

            FILL_BIAS = 2000.0   # keep PE a bit ahead of ACT (rows-equiv)

            def fill():
                # pair 0: steady front-loaded drip of deferred v-proj tiles
                # (AV(j=tt) consumes v2 tile tt -- the drip beats deadlines)
                if cur_pair[0] == 0 and not fillers[0][2]:
                    for _ in range(2):
                        try:
                            next(fillers[0][1])
                        except StopIteration:
                            fillers[0][2] = True
                            break
                while led["pe"] < led["act"] + FILL_BIAS:
                    if not fill_step():
                        break

            def flush(label):
                for ent in fillers:
                    if ent[0] == label and not ent[2]:
                        for _ in ent[1]:
                            pass
                        ent[2] = True

            # ---------------- attention (flipped AV) ----------------
            # cross-pair queue of deferred work: ("tr", p, ao, qt) transposes
            # and ("cc", p) exchange launches drain one per j-iteration, so
            # cqi/pair epilogues never idle the scalar engine
            postq = deque()

            def do_transpose(p, ao, qt):
                # scratch from the s2 ring: released fast (Pool drain)
                tr_t = ps_s2.tile([128, 2, QCH], F32, tag="s2",
                                  name=f"tr{p}_{qt}")
                area = tr_t[:, 0, 0:64].bitcast(BF16)
                nc.tensor.transpose(area, ao[:].opt(), ident_sb[:])
                mm_cost(128)
                nc.vector.tensor_copy(
                    aoT_sb[:, p, qt * 128:(qt + 1) * 128], area)

            def do_cc(p):
                nc.sync.dma_start(cc_ins[p][:], aoT_sb[:, p, :])
                if not no_cc:
                    nc.gpsimd.collective_compute(
                        "AllGather",
                        mybir.AluOpType.bypass,
                        replica_groups=[[0, 1], [2, 3], [4, 5], [6, 7]],
                        ins=[cc_ins[p][:].opt()],
                        outs=[cc_outs[p][:].opt()],
                    )
                # qk slots p and 4+p are dead after attention p: receive the
                # gathered pair there (slot index == global f-tile index)
                if no_cc:
                    nc.sync.dma_start(qk_sb[:, p, :], cc_ins[p][:])
                    nc.sync.dma_start(qk_sb[:, PAIRS + p, :], cc_ins[p][:])
                else:
                    nc.sync.dma_start(qk_sb[:, p, :], cc_outs[p][0])
                    nc.sync.dma_start(qk_sb[:, PAIRS + p, :], cc_outs[p][1])
                cc_issued.add(p)

            def pop_post(curp=None):
                # keep >=3 of the CURRENT pair's transposes pending (their
                # divs need DVE time); older pairs' items drain immediately
                if not postq:
                    return False
                if len(postq) <= 3 and postq[0][1] == curp:
                    return False
                it = postq.popleft()
                if it[0] == "tr":
                    do_transpose(*it[1:])
                else:
                    do_cc(it[1])
                return True

            # AV issuance lags QK/exp by 2 k-tiles and carries across
            # q-chunk and pair boundaries: the scalar engine's exp stream
            # never waits for an epilogue
            pend_av = deque()

            def attention_pair(p):
                qT = qk_sb[:, p, :]
                kT = qk_sb[:, PAIRS + p, :]
                vbase = p * VW

                def do_qk(cqi, j):
                    qg0 = cqi * 4
                    jr = j - qg0
                    q0 = max(jr, 0) * 128
                    sq_sl = slice(cqi * QCH + q0, (cqi + 1) * QCH)
                    s2 = ps_s2.tile([128, 2, QCH], F32, tag="s2",
                                    name=f"s2_{p}_{cqi}_{j}")
                    nc.tensor.matmul(s2[:, 0, q0:QCH],
                                     lhsT=kT[0:64, j * 128:(j + 1) * 128],
                                     rhs=qT[0:64, sq_sl], start=True,
                                     stop=(jr < 0),
                                     skip_group_check=True)
                    mm_cost(QCH - q0)
                    nc.tensor.matmul(s2[:, 1, q0:QCH],
                                     lhsT=kT[64:128, j * 128:(j + 1) * 128],
                                     rhs=qT[64:128, sq_sl], start=True,
                                     stop=(jr < 0),
                                     skip_group_check=True)
                    mm_cost(QCH - q0)
                    if jr >= 0:
                        # causal bias on the diagonal block (per head --
                        # a matmul output must stay inside one psum bank)
                        for h in range(2):
                            nc.tensor.matmul(
                                s2[:, h, q0:q0 + 128], lhsT=mneg_sb[:],
                                rhs=ident2_sb[:, 0:128],
                                start=False, stop=(h == 1),
                                skip_group_check=True)
                            mm_cost(128)
                    pt = ptp.tile([128, 2, QCH], BF16, tag="pt")
                    nc.scalar.activation(pt[:, :, q0:QCH], s2[:, :, q0:QCH],
                                         mybir.ActivationFunctionType.Exp,
                                         scale=0.125)
                    act_cost(2 * (QCH - q0))
                    return pt

                for cqi in range(NQC):
                    qg0 = cqi * 4          # first global qtile of this chunk
                    kmax = qg0 + 4
                    # peel the first QK/exps (they touch only s2/pt): the
                    # previous chunk's AV flush issues under their exps and
                    # the scalar engine never starves at the boundary
                    peeled = [(do_qk(cqi, j), j) for j in range(min(2, kmax))]
                    while pend_av:
                        fn, pt_, j_ = pend_av.popleft()
                        fn(pt_, j_)
                    y_t = [ps_y.tile([128, 2 * VW], F32, tag="y",
                                     name=f"yp{p}_{cqi}_{h}")
                           for h in range(2)]

                    def finalize(qt, yt, base):
                        # dens at cols base+64 and base+129
                        recip2 = work.tile([128, 2], F32, tag="rc",
                                           name=f"rc{p}_{qt}")
                        den_ap = _reap(yt[:, base + 64], [(HEAD_DIM + 1, 2)])
                        nc.vector.reciprocal_approx_fast(recip2[:], den_ap)
                        ao = aop.tile([128, 2, 64], BF16, tag="ao",
                                      name=f"ao{p}_{qt}")
                        for h in range(2):
                            nc.vector.tensor_scalar_mul(
                                ao[:, h, :],
                                _reap(yt[:, base + h * (HEAD_DIM + 1)], [(1, 64)]),
                                recip2[:, h:h + 1])
                        postq.append(("tr", p, ao, qt))

                    def issue_av(pt, j, qg0=qg0, y_t=y_t):
                        # default args bind THIS q-chunk's state: entries are
                        # popped after the loop variables have moved on
                        jr = j - qg0
                        for qtl in range(max(jr, 0), 4):
                            qt = qg0 + qtl
                            yt = y_t[qtl // 2]
                            base = (qtl % 2) * VW
                            for h in range(2):
                                o = h * (HEAD_DIM + 1)
                                # psum 'start' poisons the whole zero-region
                                # (bank): only the FIRST matmul touching each
                                # y tile may set it; the lazy zero-on-write
                                # initializes the other head/qt sub-groups
                                nc.tensor.matmul(
                                    yt[:, base + o: base + o + HEAD_DIM + 1],
                                    lhsT=pt[:, h, qtl * 128:(qtl + 1) * 128],
                                    rhs=v2_sb[:, j, vbase + o: vbase + o + HEAD_DIM + 1],
                                    start=(j == 0 and h == 0 and qtl % 2 == 0),
                                    stop=(j == qt),
                                    skip_group_check=True,
                                )
                                mm_cost(HEAD_DIM + 1)
                            if j == qt:
                                finalize(qt, yt, base)

                    for pt_p, j_p in peeled:
                        pend_av.append((issue_av, pt_p, j_p))
                    for j in range(len(peeled), kmax):
                        pt = do_qk(cqi, j)
                        pend_av.append((issue_av, pt, j))
                        # one deferred transpose/cc per iteration
                        pop_post(p)
                        # fillers go BEFORE the lagged AV so the in-order PE
                        # chews them while exp(j-2) finishes
                        fill()
                        if len(pend_av) > 2:
                            fn, pt_, j_ = pend_av.popleft()
                            fn(pt_, j_)
                # pair epilogue: flush so the cc enters postq after this
                # pair's transposes
                while pend_av:
                    fn, pt_, j_ = pend_av.popleft()
                    fn(pt_, j_)

            # ================= program =================
            # pre-phase: ALL qkv+norm+rope (their ACT/DVE tails hide under
            # the PE-dense mains), then v-proj tiles 0-3.  v tiles 4-15 and
            # the staged projection are the attention fillers: pure PE work
            # with no cross-engine chains to tangle with the attention loop.
            for pn in range(PAIRS):
                for _ in qkv_pair_gen(pn):
                    pass
            while ropeq:    # last pair's rope ops
                ropeq.popleft()()
            for tt in range(6):
                for _ in v_unit(tt):
                    pass

            def vdef_gen():
                for tt in range(6, NTT):
                    yield from v_unit(tt)

            fillers.append(["vdef", vdef_gen(), False])
            fillers.append(["projA", proj_stage_gen([0, PAIRS], True), False])
            fillers.append(["projB", proj_stage_gen([1, PAIRS + 1], False), False])
            fillers.append(["projC", proj_stage_gen([2, PAIRS + 2], False), False])
            gate_from.update({"projA": 0, "projB": 1, "projC": 2})

            for p in range(PAIRS):
                cur_pair[0] = p
                led["pe"] = led["act"] = 0.0
                attention_pair(p)
                # pair's transposes are already queued to postq (its last
                # q-chunk flushed pend_av), so FIFO keeps cc after them
                postq.append(("cc", p))
                if p == PAIRS - 2:
                    flush("vdef")  # safety: wv/xT die with the early pool
                    early_ctx.close()

            cur_pair[0] = PAIRS
            while postq:
                pop_post(None)
            for ent in fillers:
                flush(ent[0])

            # ---- projection tail: last pair's f-tiles + combine ----
            # psum cycles all three pools (6 slots): the adds/DMAs pipeline
            for tt in range(NTT):
                sel = tt % 3
                if sel == 0:
                    pss = ps_mm.tile([128, FL], F32, tag="mm", name=f"pf{tt}")
                elif sel == 1:
                    pss = ps_s2.tile([128, 2, QCH], F32, tag="s2",
                                     name=f"pf{tt}")[:, 0, :]
                else:
                    pss = ps_y.tile([128, FL], F32, tag="y", name=f"pf{tt}")
                for i, kf in enumerate([PAIRS - 1, 2 * PAIRS - 1]):
                    nc.tensor.matmul(
                        pss,
                        lhsT=qk_sb[:, kf, tt * 128:(tt + 1) * 128],
                        rhs=wp_sb[:, kf, :],
                        start=(i == 0), stop=(i == 1),
                    )
                    mm_cost(FL)
                ysb = evw.tile([128, FL], F32, tag="ev")
                nc.vector.tensor_add(ysb[:], pss, ppart[:, tt, :])
                nc.sync.dma_start(y_d[tt * 128:(tt + 1) * 128, :], ysb[:])

    nc.compile()
    return nc


def _prep_core_inputs(x, Wqkv, Wproj, q_norm_w, k_norm_w, core):
    b, g = core // 2, core % 2
    bf = ml_dtypes.bfloat16
    xT = np.ascontiguousarray(x[b].T).astype(bf)
    cols = slice(g * FL, (g + 1) * FL)
    wq = Wqkv[:, 0:C][:, cols] * np.tile(q_norm_w, H_LOCAL)[None, :]
    wk = Wqkv[:, C:2 * C][:, cols] * np.tile(k_norm_w, H_LOCAL)[None, :]
    wv = Wqkv[:, 2 * C:3 * C][:, cols]
    wp = Wproj[:, cols]
    return {
        "xT": xT,
        "Wq": np.ascontiguousarray(wq).astype(bf),
        "Wk": np.ascontiguousarray(wk).astype(bf),
        "Wv": np.ascontiguousarray(wv).astype(bf),
        "Wp": np.ascontiguousarray(wp).astype(bf),
    }


def kernel(x, Wqkv, Wproj, q_norm_w, k_norm_w):
    if "nc" not in _cached:
        _cached["nc"] = build_program()
    nc = _cached["nc"]

    x = np.asarray(x, dtype=np.float32)
    Wqkv = np.asarray(Wqkv, dtype=np.float32)
    Wproj = np.asarray(Wproj, dtype=np.float32)
    q_norm_w = np.asarray(q_norm_w, dtype=np.float32)
    k_norm_w = np.asarray(k_norm_w, dtype=np.float32)

    in_maps = [
        _prep_core_inputs(x, Wqkv, Wproj, q_norm_w, k_norm_w, c) for c in range(8)
    ]
    res = run_bass_kernel_spmd(nc, in_maps, list(range(8)))
    outs = res.results

    y = np.empty((B, T, C), dtype=np.float32)
    for b in range(B):
        y[b, :, 0:FL] = outs[2 * b]["y"]
        y[b, :, FL:C] = outs[2 * b + 1]["y"]
    return y


# revision 76
# speedup vs baseline: 1.0015x; 1.0015x over previous
"""Causal self-attention (QK-RMSNorm + RoPE) Trainium2 kernel.

Sharding: 8 cores = 4 batches x 2 head-groups (Megatron-style over heads).
Core c handles batch b=c//2, heads [g*8, g*8+8) with g=c%2.
Each core computes y[b, :, g*512:(g+1)*512] (output-column sharding of the
projection after a pairwise AllGather of attention outputs), so the host
only concatenates slices - no host-side arithmetic.

Perf notes (cost model charges out-free-size rows per matmul, independent of
contraction depth and output-partition count):
- AV is computed in the [q-tokens(part), head-dims(free)] orientation with a
  ones column appended to V per head: the 65-wide moving tensor makes AV cost
  65 rows/tile instead of 128-512, and the softmax denominator accumulates
  for free in column 64.  The division is then a per-partition scalar
  multiply (DVE), and the output is transposed back to [dims, tok] with
  cheap PE transposes (128 rows each) for the AllGather + projection.
- The per-head sum-of-squares for QK-RMSNorm uses one block-diagonal-ones
  matmul covering both packed heads; squares are computed on DVE in bf16
  from a Pool-engine drain of the qkv psum (keeps the scalar engine free
  for the attention exp()s, which are its binding load).
- The in-order PE is kept saturated (and in max p-state) by interleaving
  filler matmuls into the attention loop, driven by a PE-vs-ACT issued-work
  ledger: qkv of the next pair, deferred v-proj tiles, and the partial
  projection of already-gathered f-tiles (staged per AllGather arrival,
  accumulated into a bf16 partial on the Pool engine).  Only the last
  pair's two f-tiles + a DVE add remain after the final AllGather.
- Transpose scratch lives in the s2 (scores) PSUM ring, so the AV
  accumulator ring is released by the division and never serializes
  consecutive q-chunks.
"""


import numpy as np
import ml_dtypes
from collections import deque
from contextlib import ExitStack

import concourse.bass as bass
import concourse.bacc as bacc

# Force all activations into the one table set that covers Exp+Ln+Square+
# Copy+Identity, so the whole kernel needs exactly one ACT_TABLE_LOAD.
import concourse.hw_specs as _hw_specs
_orig_gat = _hw_specs.get_activation_tables

def _gat_one_set(arch):
    t = _orig_gat(arch)
    return {k: (v if k == "natural_log_exp_and_others" else set())
            for k, v in t.items()}

bacc.get_activation_tables = _gat_one_set
import concourse.mybir as mybir
import concourse.tile as tile
from concourse.bass_utils import run_bass_kernel_spmd

BF16 = mybir.dt.bfloat16
F32 = mybir.dt.float32

N_HEAD = 16
HEAD_DIM = 64
EPS = 1e-5
ROPE_BASE = 10000.0

B, T, C = 4, 2048, 1024
H_LOCAL = N_HEAD // 2          # heads per core
PAIRS = H_LOCAL // 2           # head-pairs per core (processed 2-at-a-time)
CT = C // 128                  # contraction tiles over C
FL = H_LOCAL * HEAD_DIM        # local feature width (512)
QCH = 512                      # q-chunk width
NQC = T // QCH                 # q-chunks
NKT = T // 128                 # k tiles
NTT = T // 128                 # token tiles
VW = 2 * (HEAD_DIM + 1)        # per-pair v2 width: [A dims|onesA|B dims|onesB]

_cached = {}


def _reap(ap, dims):
    """Rebuild an AP keeping tensor/offset/partition dim, with free dims
    `dims` given as (stride, size) pairs."""
    return bass.AP(tensor=ap.tensor, offset=ap.offset,
                   ap=[ap.ap[0]] + [list(d) for d in dims])


def _fbcast2(ap):
    """[128, N] AP -> [128, 2, N] with the middle (free) dim broadcast."""
    return bass.AP(
        tensor=ap.tensor, offset=ap.offset, ap=[ap.ap[0], [0, 2], ap.ap[1]]
    )


def _rope_tables():
    inv_freq = 1.0 / (ROPE_BASE ** (np.arange(0, HEAD_DIM, 2, dtype=np.float64) / HEAD_DIM))
    t = np.arange(T, dtype=np.float64)
    freqs = np.outer(t, inv_freq)                       # [T, 32]
    emb = np.concatenate([freqs, freqs], -1)            # [T, 64]
    cos = np.cos(emb).astype(np.float32).T              # [64, T]
    sin = np.sin(emb).astype(np.float32).T              # [64, T]
    cos2 = np.concatenate([cos, cos], 0)                # [128, T] two heads
    sin_s = sin.copy()
    sin_s[0:32] = -sin_s[0:32]                          # rotate-half sign
    sin2 = np.concatenate([sin_s, sin_s], 0)            # [128, T]
    return cos2.astype(ml_dtypes.bfloat16), sin2.astype(ml_dtypes.bfloat16)


def _diag_masks():
    # corner mask: keep where k_partition <= q_col (lower-triangular 128x128)
    p = np.arange(128)[:, None]
    qf = np.arange(128)[None, :]
    m = (p <= qf).astype(np.float32)
    return m.astype(ml_dtypes.bfloat16)                 # [128, 128]


def build_program(no_cc=False):
    nc = bacc.Bacc("TRN2", target_bir_lowering=False, debug=False,
                   num_devices=1 if no_cc else 8)

    xT_d = nc.dram_tensor("xT", [C, T], BF16, kind="ExternalInput")
    wq_d = nc.dram_tensor("Wq", [C, FL], BF16, kind="ExternalInput")
    wk_d = nc.dram_tensor("Wk", [C, FL], BF16, kind="ExternalInput")
    wv_d = nc.dram_tensor("Wv", [C, FL], BF16, kind="ExternalInput")
    wp_d = nc.dram_tensor("Wp", [C, FL], BF16, kind="ExternalInput")
    y_d = nc.dram_tensor("y", [T, FL], F32, kind="ExternalOutput")

    cos2_np, sin2_np = _rope_tables()
    cos_d = nc.inline_tensor(np.ascontiguousarray(cos2_np), "cos2")
    sin_d = nc.inline_tensor(np.ascontiguousarray(sin2_np), "sin2")
    # causal mask as a score bias: out[p,g,f] += mneg[f,p] = -30000*(p>f),
    # added to the diagonal 128x128 block by one PE matmul (keeps the
    # exp->AV chain off the vector engine)
    mneg_np = -30000.0 * (np.arange(128)[None, :] > np.arange(128)[:, None])
    mneg_d = nc.inline_tensor(
        np.ascontiguousarray(mneg_np.astype(ml_dtypes.bfloat16)), "mneg")
    id2_np = np.tile(np.eye(128, dtype=ml_dtypes.bfloat16), (1, 2))
    ident2_d = nc.inline_tensor(np.ascontiguousarray(id2_np), "ident2")
    bd_np = np.zeros((128, 128), dtype=ml_dtypes.bfloat16)
    bd_np[0:64, 0:64] = 1.0
    bd_np[64:128, 64:128] = 1.0
    onesbd_d = nc.inline_tensor(np.ascontiguousarray(bd_np), "onesbd")
    ident_d = nc.inline_tensor(
        np.ascontiguousarray(np.eye(128, dtype=ml_dtypes.bfloat16)), "ident")

    # per-pair exchange buffers
    cc_ins = [nc.dram_tensor(f"cc_in{p}", [128, T], BF16) for p in range(PAIRS)]
    cc_outs = [nc.dram_tensor(f"cc_out{p}", [2, 128, T], BF16) for p in range(PAIRS)]

    # --- PE-vs-ACT issued-work ledger (units: bf16 matmul rows = 0.4167ns) ---
    led = {"pe": 0.0, "act": 0.0}

    def mm_cost(rows):
        led["pe"] += rows

    def act_cost(free):
        led["act"] += 2.0 * free + 444.0

    with tile.TileContext(nc) as tc:
        with (
            tc.tile_pool(name="const", bufs=1) as const,
            tc.tile_pool(name="work", bufs=2) as work,
            tc.tile_pool(name="evw", bufs=6) as evw,
            tc.tile_pool(name="rope", bufs=2) as ropep,
            tc.tile_pool(name="pt", bufs=5) as ptp,
            tc.tile_pool(name="qraw", bufs=3) as qrawp,
            tc.tile_pool(name="ao", bufs=6) as aop,
            tc.tile_pool(name="ps_s2", bufs=2, space="PSUM") as ps_s2,
            tc.tile_pool(name="ps_y", bufs=2, space="PSUM") as ps_y,
            tc.tile_pool(name="ps_mm", bufs=2, space="PSUM") as ps_mm,
        ):
            early_ctx = ExitStack()
            early = early_ctx.enter_context(tc.tile_pool(name="early", bufs=1))

            # ---- inputs: wq/wk/xT first (qkv(0) consumes them first) ----
            wq_sb = early.tile([128, CT, FL], BF16)
            wk_sb = early.tile([128, CT, FL], BF16)
            xT_sb = early.tile([128, CT, T], BF16)
            # few, large DMAs: the HWDGE serializes ~625ns per DMA instruction.
            # xT halved with first halves first: qkv chunk 0/1 start earlier.
            nc.sync.dma_start(wq_sb[:], wq_d[:].rearrange("(k p) f -> p k f", p=128))
            for k in range(CT):
                nc.sync.dma_start(xT_sb[:, k, 0:T // 2],
                                  xT_d[k * 128:(k + 1) * 128, 0:T // 2])
            # wk after xT half-0: the primed q-mtile only needs wq + xT
            nc.sync.dma_start(wk_sb[:], wk_d[:].rearrange("(k p) f -> p k f", p=128))
            for k in range(CT):
                nc.sync.dma_start(xT_sb[:, k, T // 2:T],
                                  xT_d[k * 128:(k + 1) * 128, T // 2:T])
            wv_sb = early.tile([128, CT, FL], BF16)
            nc.sync.dma_start(wv_sb[:], wv_d[:].rearrange("(k p) f -> p k f", p=128))
            cos_sb = early.tile([128, T], BF16)
            nc.sync.dma_start(cos_sb[:], cos_d[:])
            sin_sb = early.tile([128, T], BF16)
            nc.sync.dma_start(sin_sb[:], sin_d[:])
            mneg_sb = const.tile([128, 128], BF16)
            nc.sync.dma_start(mneg_sb[:], mneg_d[:])
            ident2_sb = const.tile([128, 256], BF16)
            nc.sync.dma_start(ident2_sb[:], ident2_d[:])
            ident_sb = const.tile([128, 128], BF16)
            nc.sync.dma_start(ident_sb[:], ident_d[:])
            wp_sb = const.tile([128, CT, FL], BF16)
            nc.sync.dma_start(wp_sb[:], wp_d[:].rearrange("(k p) f -> p k f", p=128))

            # block-diagonal ones: one matmul sums squares of both packed heads
            ones_bd = const.tile([128, 128], BF16)
            nc.sync.dma_start(ones_bd[:], onesbd_d[:])

            qk_sb = const.tile([128, 2 * PAIRS, T], BF16)
            # v2: per tt, per pair: [A dims(64) | onesA | B dims(64) | onesB]
            v2_sb = const.tile([128, NTT, PAIRS * VW], BF16)
            aoT_sb = const.tile([128, PAIRS, T], BF16)
            # bf16 partial of the projection over already-gathered f-tiles
            ppart = const.tile([128, NTT, FL], BF16)

            # ones columns of v2 (once, before any v-proj writes)
            pstr = v2_sb.ap[0][0]
            ones_ap = bass.AP(
                tensor=v2_sb.tensor, offset=v2_sb.offset + HEAD_DIM,
                ap=[[pstr, 128], [PAIRS * VW, NTT], [VW, PAIRS], [HEAD_DIM + 1, 2]])
            nc.vector.memset(ones_ap, 1.0)

            # ---------------- unit generators (PE fillers) ----------------
            def v_unit(tt):
                pss = ps_mm.tile([128, FL], F32, tag="mm", name=f"vp{tt}")
                for k0 in range(0, CT, 2):
                    for k in (k0, k0 + 1):
                        nc.tensor.matmul(
                            pss,
                            lhsT=xT_sb[:, k, tt * 128:(tt + 1) * 128],
                            rhs=wv_sb[:, k, :],
                            start=(k == 0), stop=(k == CT - 1),
                        )
                        mm_cost(FL)
                    yield
                # drain into v2 layout: [128,4,2,64] both sides (DVE --
                # GPSIMD cannot touch PSUM on real hardware)
                src = _reap(pss, [(128, PAIRS), (64, 2), (1, 64)])
                dst = bass.AP(tensor=v2_sb.tensor,
                              offset=v2_sb.offset + tt * PAIRS * VW,
                              ap=[[pstr, 128], [VW, PAIRS], [HEAD_DIM + 1, 2], [1, 64]])
                nc.vector.tensor_copy(dst, src)
                yield

            def qkv_tail(m, pss, sq, n):
                ss = ps_y.tile([128, QCH], F32, tag="y", name=f"ss{m}_{n}")
                nc.tensor.matmul(ss, lhsT=ones_bd[:], rhs=sq[:],
                                 start=True, stop=True, skip_group_check=True)
                mm_cost(QCH)
                # rstd = (ss/64)^(-1/2) = exp(-0.5*ln(ss/64)); eps negligible.
                rr = work.tile([128, QCH], BF16, tag="rr")
                nc.scalar.activation(rr[:], ss,
                                     mybir.ActivationFunctionType.Ln,
                                     scale=1.0 / HEAD_DIM)
                act_cost(QCH)
                rstd = work.tile([128, QCH], BF16, tag="rs")
                nc.scalar.activation(rstd[:], rr[:],
                                     mybir.ActivationFunctionType.Exp,
                                     scale=-0.5)
                act_cost(QCH)
                nc.vector.tensor_mul(qk_sb[:, m, n * QCH:(n + 1) * QCH],
                                     pss, rstd[:])

            # lazy rope: one closure per DVE op, drained one per qkv yield so
            # the qkv norm-muls (psum release) never queue behind a burst
            ropeq = deque()

            def rope_mtile(m):
                src = qk_sb[:, m, :]
                sw = ropep.tile([128, T], BF16, tag="rp")
                for off in (0, 64):
                    ropeq.append(lambda o=off: nc.vector.tensor_copy(
                        sw[o:o + 32, :], src[o + 32:o + 64, :]))
                    ropeq.append(lambda o=off: nc.vector.tensor_copy(
                        sw[o + 32:o + 64, :], src[o:o + 32, :]))
                ropeq.append(lambda: nc.vector.tensor_mul(sw[:], sw[:], sin_sb[:]))
                ropeq.append(lambda: nc.vector.tensor_mul(src, src, cos_sb[:]))
                ropeq.append(lambda: nc.vector.tensor_add(src, src, sw[:]))

            def qkv_unit(m):
                # pss borrows the (pre-phase-idle) s2 slots, ss the y slots:
                # the qkv pipeline gets baseline's 4-bank depth while the
                # attention pools keep their static reservation.
                # private 2-deep pss ring per generator (q-mtiles borrow the
                # pre-phase-idle s2 slots, k-mtiles the mm slots): a slot is
                # reused only 2 chunks later, far past its tail chain
                w_sb = wq_sb if m < PAIRS else wk_sb
                mloc = (m % PAIRS) * 128
                pend = None
                ps2 = None
                for n in range(NQC):
                    if m < PAIRS:
                        # 4-deep ring: two chunks per 2-bank s2 slot
                        if n % 2 == 0:
                            ps2 = ps_s2.tile([128, 2, QCH], F32, tag="s2",
                                             name=f"qk{m}_{n}")
                        pss = ps2[:, n % 2, :]
                    else:
                        pss = ps_mm.tile([128, QCH], F32, tag="mm",
                                         name=f"qk{m}_{n}")
                    for k0 in range(0, CT, 2):
                        for k in (k0, k0 + 1):
                            nc.tensor.matmul(
                                pss,
                                lhsT=w_sb[:, k, mloc:mloc + 128],
                                rhs=xT_sb[:, k, n * QCH:(n + 1) * QCH],
                                start=(k == 0), stop=(k == CT - 1),
                                skip_group_check=True,
                            )
                            mm_cost(QCH)
                        yield
                    # square must be ACT: DVE cannot read two PSUM operands
                    sq = work.tile([128, QCH], BF16, tag="sq")
                    nc.scalar.activation(sq[:], pss,
                                         mybir.ActivationFunctionType.Square)
                    act_cost(QCH)
                    if pend is not None:
                        qkv_tail(m, *pend)
                        yield
                    pend = (pss, sq, n)
                qkv_tail(m, *pend)
                yield
                rope_mtile(m)

            def qkv_pair_gen(pn):
                # alternate q- and k-mtile steps (q primed 2 chunks ahead so
                # the two tails never bunch at the same boundary)
                a, b = qkv_unit(pn), qkv_unit(PAIRS + pn)
                for _ in range(8):
                    try:
                        next(a)
                    except StopIteration:
                        break
                while True:
                    na = nb = False
                    try:
                        next(a)
                    except StopIteration:
                        na = True
                    if ropeq:
                        ropeq.popleft()()
                    try:
                        next(b)
                    except StopIteration:
                        nb = True
                    if ropeq:
                        ropeq.popleft()()
                    if na and nb:
                        return
                    yield

            def proj_stage_gen(kfs, first_stage):
                for tt in range(NTT):
                    pss = ps_mm.tile([128, FL], F32, tag="mm",
                                     name=f"pj{kfs[0]}_{tt}")
                    for i, kf in enumerate(kfs):
                        nc.tensor.matmul(
                            pss,
                            lhsT=qk_sb[:, kf, tt * 128:(tt + 1) * 128],
                            rhs=wp_sb[:, kf, :],
                            start=(i == 0), stop=(i == len(kfs) - 1),
                        )
                        mm_cost(FL)
                        yield
                    pp = ppart[:, tt, :]
                    if first_stage:
                        nc.vector.tensor_copy(pp, pss)
                    else:
                        nc.vector.tensor_add(pp, pss, pp)
                    yield

            # ---------------- filler machinery ----------------
            # list of [label, generator, done]; fill_step picks the first
            # not-done generator whose gate is open (order = priority).
            fillers = []
            gate_from = {}   # label -> cc whose writeback must be issued
            cc_issued = set()
            cur_pair = [0]

            def fill_step():
                for ent in fillers:
                    if ent[2] or gate_from.get(ent[0], -1) not in cc_issued | {-1}:
                        continue
                    try:
                        next(ent[1])
                        return True
                    except StopIteration:
                        ent[2] = True
                        continue
                return False

            FILL_BIAS = 2000.0   # keep PE a bit ahead of ACT (rows-equiv)

            def fill():
                # pair 0: steady front-loaded drip of deferred v-proj tiles
                # (AV(j=tt) consumes v2 tile tt -- the drip beats deadlines)
                if cur_pair[0] == 0 and not fillers[0][2]:
                    for _ in range(2):
                        try:
                            next(fillers[0][1])
                        except StopIteration:
                            fillers[0][2] = True
                            break
                while led["pe"] < led["act"] + FILL_BIAS:
                    if not fill_step():
                        break

            def flush(label):
                for ent in fillers:
                    if ent[0] == label and not ent[2]:
                        for _ in ent[1]:
                            pass
                        ent[2] = True

            # ---------------- attention (flipped AV) ----------------
            # cross-pair queue of deferred work: ("tr", p, ao, qt) transposes
            # and ("cc", p) exchange launches drain one per j-iteration, so
            # cqi/pair epilogues never idle the scalar engine
            postq = deque()

            def do_transpose(p, ao, qt):
                # scratch from the s2 ring: released fast (Pool drain)
                tr_t = ps_s2.tile([128, 2, QCH], F32, tag="s2",
                                  name=f"tr{p}_{qt}")
                area = tr_t[:, 0, 0:64].bitcast(BF16)
                nc.tensor.transpose(area, ao[:].opt(), ident_sb[:])
                mm_cost(128)
                nc.vector.tensor_copy(
                    aoT_sb[:, p, qt * 128:(qt + 1) * 128], area)

            def do_cc(p):
                nc.sync.dma_start(cc_ins[p][:], aoT_sb[:, p, :])
                if not no_cc:
                    nc.gpsimd.collective_compute(
                        "AllGather",
                        mybir.AluOpType.bypass,
                        replica_groups=[[0, 1], [2, 3], [4, 5], [6, 7]],
                        ins=[cc_ins[p][:].opt()],
                        outs=[cc_outs[p][:].opt()],
                    )
                # qk slots p and 4+p are dead after attention p: receive the
                # gathered pair there (slot index == global f-tile index)
                if no_cc:
                    nc.sync.dma_start(qk_sb[:, p, :], cc_ins[p][:])
                    nc.sync.dma_start(qk_sb[:, PAIRS + p, :], cc_ins[p][:])
                else:
                    nc.sync.dma_start(qk_sb[:, p, :], cc_outs[p][0])
                    nc.sync.dma_start(qk_sb[:, PAIRS + p, :], cc_outs[p][1])
                cc_issued.add(p)

            def pop_post(curp=None):
                # keep >=3 of the CURRENT pair's transposes pending (their
                # divs need DVE time); older pairs' items drain immediately
                if not postq:
                    return False
                if len(postq) <= 3 and postq[0][1] == curp:
                    return False
                it = postq.popleft()
                if it[0] == "tr":
                    do_transpose(*it[1:])
                else:
                    do_cc(it[1])
                return True

            # AV issuance lags QK/exp by 2 k-tiles and carries across
            # q-chunk and pair boundaries: the scalar engine's exp stream
            # never waits for an epilogue
            pend_av = deque()

            def attention_pair(p):
                qT = qk_sb[:, p, :]
                kT = qk_sb[:, PAIRS + p, :]
                vbase = p * VW

                def do_qk(cqi, j):
                    qg0 = cqi * 4
                    jr = j - qg0
                    q0 = max(jr, 0) * 128
                    sq_sl = slice(cqi * QCH + q0, (cqi + 1) * QCH)
                    s2 = ps_s2.tile([128, 2, QCH], F32, tag="s2",
                                    name=f"s2_{p}_{cqi}_{j}")
                    nc.tensor.matmul(s2[:, 0, q0:QCH],
                                     lhsT=kT[0:64, j * 128:(j + 1) * 128],
                                     rhs=qT[0:64, sq_sl], start=True,
                                     stop=(jr < 0),
                                     skip_group_check=True)
                    mm_cost(QCH - q0)
                    nc.tensor.matmul(s2[:, 1, q0:QCH],
                                     lhsT=kT[64:128, j * 128:(j + 1) * 128],
                                     rhs=qT[64:128, sq_sl], start=True,
                                     stop=(jr < 0),
                                     skip_group_check=True)
                    mm_cost(QCH - q0)
                    if jr >= 0:
                        # causal bias on the diagonal block (per head --
                        # a matmul output must stay inside one psum bank)
                        for h in range(2):
                            nc.tensor.matmul(
                                s2[:, h, q0:q0 + 128], lhsT=mneg_sb[:],
                                rhs=ident2_sb[:, 0:128],
                                start=False, stop=(h == 1),
                                skip_group_check=True)
                            mm_cost(128)
                    pt = ptp.tile([128, 2, QCH], BF16, tag="pt")
                    nc.scalar.activation(pt[:, :, q0:QCH], s2[:, :, q0:QCH],
                                         mybir.ActivationFunctionType.Exp,
                                         scale=0.125)
                    act_cost(2 * (QCH - q0))
                    return pt

                for cqi in range(NQC):
                    qg0 = cqi * 4          # first global qtile of this chunk
                    kmax = qg0 + 4
                    y_t = [ps_y.tile([128, 2 * VW], F32, tag="y",
                                     name=f"yp{p}_{cqi}_{h}")
                           for h in range(2)]

                    def finalize(qt, yt, base):
                        # dens at cols base+64 and base+129
                        recip2 = work.tile([128, 2], F32, tag="rc",
                                           name=f"rc{p}_{qt}")
                        den_ap = _reap(yt[:, base + 64], [(HEAD_DIM + 1, 2)])
                        nc.vector.reciprocal_approx_fast(recip2[:], den_ap)
                        ao = aop.tile([128, 2, 64], BF16, tag="ao",
                                      name=f"ao{p}_{qt}")
                        for h in range(2):
                            nc.vector.tensor_scalar_mul(
                                ao[:, h, :],
                                _reap(yt[:, base + h * (HEAD_DIM + 1)], [(1, 64)]),
                                recip2[:, h:h + 1])
                        postq.append(("tr", p, ao, qt))

                    def issue_av(pt, j, qg0=qg0, y_t=y_t):
                        # default args bind THIS q-chunk's state: entries are
                        # popped after the loop variables have moved on
                        jr = j - qg0
                        for qtl in range(max(jr, 0), 4):
                            qt = qg0 + qtl
                            yt = y_t[qtl // 2]
                            base = (qtl % 2) * VW
                            for h in range(2):
                                o = h * (HEAD_DIM + 1)
                                # psum 'start' poisons the whole zero-region
                                # (bank): only the FIRST matmul touching each
                                # y tile may set it; the lazy zero-on-write
                                # initializes the other head/qt sub-groups
                                nc.tensor.matmul(
                                    yt[:, base + o: base + o + HEAD_DIM + 1],
                                    lhsT=pt[:, h, qtl * 128:(qtl + 1) * 128],
                                    rhs=v2_sb[:, j, vbase + o: vbase + o + HEAD_DIM + 1],
                                    start=(j == 0 and h == 0 and qtl % 2 == 0),
                                    stop=(j == qt),
                                    skip_group_check=True,
                                )
                                mm_cost(HEAD_DIM + 1)
                            if j == qt:
                                finalize(qt, yt, base)

                    for j in range(kmax):
                        pt = do_qk(cqi, j)
                        pend_av.append((issue_av, pt, j))
                        # one deferred transpose/cc per iteration
                        pop_post(p)
                        # fillers go BEFORE the lagged AV so the in-order PE
                        # chews them while exp(j-2) finishes
                        fill()
                        if len(pend_av) > 2:
                            fn, pt_, j_ = pend_av.popleft()
                            fn(pt_, j_)
                    # q-chunk epilogue: the accumulator ring recycles at the
                    # next y_t alloc, so all its AVs must be issued NOW (the
                    # transposes/cc stay deferred -- they use fresh scratch)
                    while pend_av:
                        fn, pt_, j_ = pend_av.popleft()
                        fn(pt_, j_)

            # ================= program =================
            # pre-phase: ALL qkv+norm+rope (their ACT/DVE tails hide under
            # the PE-dense mains), then v-proj tiles 0-3.  v tiles 4-15 and
            # the staged projection are the attention fillers: pure PE work
            # with no cross-engine chains to tangle with the attention loop.
            for pn in range(PAIRS):
                for _ in qkv_pair_gen(pn):
                    pass
            while ropeq:    # last pair's rope ops
                ropeq.popleft()()
            for tt in range(6):
                for _ in v_unit(tt):
                    pass

            def vdef_gen():
                for tt in range(6, NTT):
                    yield from v_unit(tt)

            fillers.append(["vdef", vdef_gen(), False])
            fillers.append(["projA", proj_stage_gen([0, PAIRS], True), False])
            fillers.append(["projB", proj_stage_gen([1, PAIRS + 1], False), False])
            fillers.append(["projC", proj_stage_gen([2, PAIRS + 2], False), False])
            gate_from.update({"projA": 0, "projB": 1, "projC": 2})

            for p in range(PAIRS):
                cur_pair[0] = p
                led["pe"] = led["act"] = 0.0
                attention_pair(p)
                # pair's transposes are already queued to postq (its last
                # q-chunk flushed pend_av), so FIFO keeps cc after them
                postq.append(("cc", p))
                if p == PAIRS - 2:
                    flush("vdef")  # safety: wv/xT die with the early pool
                    early_ctx.close()

            cur_pair[0] = PAIRS
            while postq:
                pop_post(None)
            for ent in fillers:
                flush(ent[0])

            # ---- projection tail: last pair's f-tiles + combine ----
            # psum cycles all three pools (6 slots): the adds/DMAs pipeline
            for tt in range(NTT):
                sel = tt % 3
                if sel == 0:
                    pss = ps_mm.tile([128, FL], F32, tag="mm", name=f"pf{tt}")
                elif sel == 1:
                    pss = ps_s2.tile([128, 2, QCH], F32, tag="s2",
                                     name=f"pf{tt}")[:, 0, :]
                else:
                    pss = ps_y.tile([128, FL], F32, tag="y", name=f"pf{tt}")
                for i, kf in enumerate([PAIRS - 1, 2 * PAIRS - 1]):
                    nc.tensor.matmul(
                        pss,
                        lhsT=qk_sb[:, kf, tt * 128:(tt + 1) * 128],
                        rhs=wp_sb[:, kf, :],
                        start=(i == 0), stop=(i == 1),
                    )
                    mm_cost(FL)
                ysb = evw.tile([128, FL], F32, tag="ev")
                nc.vector.tensor_add(ysb[:], pss, ppart[:, tt, :])
                nc.sync.dma_start(y_d[tt * 128:(tt + 1) * 128, :], ysb[:])

    nc.compile()
    return nc


def _prep_core_inputs(x, Wqkv, Wproj, q_norm_w, k_norm_w, core):
    b, g = core // 2, core % 2
    bf = ml_dtypes.bfloat16
    xT = np.ascontiguousarray(x[b].T).astype(bf)
    cols = slice(g * FL, (g + 1) * FL)
    wq = Wqkv[:, 0:C][:, cols] * np.tile(q_norm_w, H_LOCAL)[None, :]
    wk = Wqkv[:, C:2 * C][:, cols] * np.tile(k_norm_w, H_LOCAL)[None, :]
    wv = Wqkv[:, 2 * C:3 * C][:, cols]
    wp = Wproj[:, cols]
    return {
        "xT": xT,
        "Wq": np.ascontiguousarray(wq).astype(bf),
        "Wk": np.ascontiguousarray(wk).astype(bf),
        "Wv": np.ascontiguousarray(wv).astype(bf),
        "Wp": np.ascontiguousarray(wp).astype(bf),
    }


def kernel(x, Wqkv, Wproj, q_norm_w, k_norm_w):
    if "nc" not in _cached:
        _cached["nc"] = build_program()
    nc = _cached["nc"]

    x = np.asarray(x, dtype=np.float32)
    Wqkv = np.asarray(Wqkv, dtype=np.float32)
    Wproj = np.asarray(Wproj, dtype=np.float32)
    q_norm_w = np.asarray(q_norm_w, dtype=np.float32)
    k_norm_w = np.asarray(k_norm_w, dtype=np.float32)

    in_maps = [
        _prep_core_inputs(x, Wqkv, Wproj, q_norm_w, k_norm_w, c) for c in range(8)
    ]
    res = run_bass_kernel_spmd(nc, in_maps, list(range(8)))
    outs = res.results

    y = np.empty((B, T, C), dtype=np.float32)
    for b in range(B):
        y[b, :, 0:FL] = outs[2 * b]["y"]
        y[b, :, FL:C] = outs[2 * b + 1]["y"]
    return y


# revision 77
# speedup vs baseline: 1.0223x; 1.0208x over previous
"""Causal self-attention (QK-RMSNorm + RoPE) Trainium2 kernel.

Sharding: 8 cores = 4 batches x 2 head-groups (Megatron-style over heads).
Core c handles batch b=c//2, heads [g*8, g*8+8) with g=c%2.
Each core computes y[b, :, g*512:(g+1)*512] (output-column sharding of the
projection after a pairwise AllGather of attention outputs), so the host
only concatenates slices - no host-side arithmetic.

Perf notes (cost model charges out-free-size rows per matmul, independent of
contraction depth and output-partition count):
- AV is computed in the [q-tokens(part), head-dims(free)] orientation with a
  ones column appended to V per head: the 65-wide moving tensor makes AV cost
  65 rows/tile instead of 128-512, and the softmax denominator accumulates
  for free in column 64.  The division is then a per-partition scalar
  multiply (DVE), and the output is transposed back to [dims, tok] with
  cheap PE transposes (128 rows each) for the AllGather + projection.
- The per-head sum-of-squares for QK-RMSNorm uses one block-diagonal-ones
  matmul covering both packed heads; squares are computed on DVE in bf16
  from a Pool-engine drain of the qkv psum (keeps the scalar engine free
  for the attention exp()s, which are its binding load).
- The in-order PE is kept saturated (and in max p-state) by interleaving
  filler matmuls into the attention loop, driven by a PE-vs-ACT issued-work
  ledger: qkv of the next pair, deferred v-proj tiles, and the partial
  projection of already-gathered f-tiles (staged per AllGather arrival,
  accumulated into a bf16 partial on the Pool engine).  Only the last
  pair's two f-tiles + a DVE add remain after the final AllGather.
- Transpose scratch lives in the s2 (scores) PSUM ring, so the AV
  accumulator ring is released by the division and never serializes
  consecutive q-chunks.
"""


import numpy as np
import ml_dtypes
from collections import deque
from contextlib import ExitStack

import concourse.bass as bass
import concourse.bacc as bacc

# Force all activations into the one table set that covers Exp+Ln+Square+
# Copy+Identity, so the whole kernel needs exactly one ACT_TABLE_LOAD.
import concourse.hw_specs as _hw_specs
_orig_gat = _hw_specs.get_activation_tables

def _gat_one_set(arch):
    t = _orig_gat(arch)
    return {k: (v if k == "natural_log_exp_and_others" else set())
            for k, v in t.items()}

bacc.get_activation_tables = _gat_one_set
import concourse.mybir as mybir
import concourse.tile as tile
from concourse.bass_utils import run_bass_kernel_spmd

BF16 = mybir.dt.bfloat16
F32 = mybir.dt.float32

N_HEAD = 16
HEAD_DIM = 64
EPS = 1e-5
ROPE_BASE = 10000.0

B, T, C = 4, 2048, 1024
H_LOCAL = N_HEAD // 2          # heads per core
PAIRS = H_LOCAL // 2           # head-pairs per core (processed 2-at-a-time)
CT = C // 128                  # contraction tiles over C
FL = H_LOCAL * HEAD_DIM        # local feature width (512)
QCH = 512                      # q-chunk width
NQC = T // QCH                 # q-chunks
NKT = T // 128                 # k tiles
NTT = T // 128                 # token tiles
VW = 2 * (HEAD_DIM + 1)        # per-pair v2 width: [A dims|onesA|B dims|onesB]

_cached = {}


def _reap(ap, dims):
    """Rebuild an AP keeping tensor/offset/partition dim, with free dims
    `dims` given as (stride, size) pairs."""
    return bass.AP(tensor=ap.tensor, offset=ap.offset,
                   ap=[ap.ap[0]] + [list(d) for d in dims])


def _fbcast2(ap):
    """[128, N] AP -> [128, 2, N] with the middle (free) dim broadcast."""
    return bass.AP(
        tensor=ap.tensor, offset=ap.offset, ap=[ap.ap[0], [0, 2], ap.ap[1]]
    )


def _rope_tables():
    inv_freq = 1.0 / (ROPE_BASE ** (np.arange(0, HEAD_DIM, 2, dtype=np.float64) / HEAD_DIM))
    t = np.arange(T, dtype=np.float64)
    freqs = np.outer(t, inv_freq)                       # [T, 32]
    emb = np.concatenate([freqs, freqs], -1)            # [T, 64]
    cos = np.cos(emb).astype(np.float32).T              # [64, T]
    sin = np.sin(emb).astype(np.float32).T              # [64, T]
    cos2 = np.concatenate([cos, cos], 0)                # [128, T] two heads
    sin_s = sin.copy()
    sin_s[0:32] = -sin_s[0:32]                          # rotate-half sign
    sin2 = np.concatenate([sin_s, sin_s], 0)            # [128, T]
    return cos2.astype(ml_dtypes.bfloat16), sin2.astype(ml_dtypes.bfloat16)


def _diag_masks():
    # corner mask: keep where k_partition <= q_col (lower-triangular 128x128)
    p = np.arange(128)[:, None]
    qf = np.arange(128)[None, :]
    m = (p <= qf).astype(np.float32)
    return m.astype(ml_dtypes.bfloat16)                 # [128, 128]


def build_program(no_cc=False):
    nc = bacc.Bacc("TRN2", target_bir_lowering=False, debug=False,
                   num_devices=1 if no_cc else 8)

    xT_d = nc.dram_tensor("xT", [C, T], BF16, kind="ExternalInput")
    wq_d = nc.dram_tensor("Wq", [C, FL], BF16, kind="ExternalInput")
    wk_d = nc.dram_tensor("Wk", [C, FL], BF16, kind="ExternalInput")
    wv_d = nc.dram_tensor("Wv", [C, FL], BF16, kind="ExternalInput")
    wp_d = nc.dram_tensor("Wp", [C, FL], BF16, kind="ExternalInput")
    y_d = nc.dram_tensor("y", [T, FL], F32, kind="ExternalOutput")

    cos2_np, sin2_np = _rope_tables()
    cos_d = nc.inline_tensor(np.ascontiguousarray(cos2_np), "cos2")
    sin_d = nc.inline_tensor(np.ascontiguousarray(sin2_np), "sin2")
    # causal mask as a score bias: out[p,g,f] += mneg[f,p] = -30000*(p>f),
    # added to the diagonal 128x128 block by one PE matmul (keeps the
    # exp->AV chain off the vector engine)
    mneg_np = -30000.0 * (np.arange(128)[None, :] > np.arange(128)[:, None])
    mneg_d = nc.inline_tensor(
        np.ascontiguousarray(mneg_np.astype(ml_dtypes.bfloat16)), "mneg")
    id2_np = np.tile(np.eye(128, dtype=ml_dtypes.bfloat16), (1, 2))
    ident2_d = nc.inline_tensor(np.ascontiguousarray(id2_np), "ident2")
    bd_np = np.zeros((128, 128), dtype=ml_dtypes.bfloat16)
    bd_np[0:64, 0:64] = 1.0
    bd_np[64:128, 64:128] = 1.0
    onesbd_d = nc.inline_tensor(np.ascontiguousarray(bd_np), "onesbd")
    ident_d = nc.inline_tensor(
        np.ascontiguousarray(np.eye(128, dtype=ml_dtypes.bfloat16)), "ident")

    # per-pair exchange buffers
    cc_ins = [nc.dram_tensor(f"cc_in{p}", [128, T], BF16) for p in range(PAIRS)]
    cc_outs = [nc.dram_tensor(f"cc_out{p}", [2, 128, T], BF16) for p in range(PAIRS)]

    # --- PE-vs-ACT issued-work ledger (units: bf16 matmul rows = 0.4167ns) ---
    led = {"pe": 0.0, "act": 0.0}

    def mm_cost(rows):
        led["pe"] += rows

    def act_cost(free):
        led["act"] += 2.0 * free + 444.0

    with tile.TileContext(nc) as tc:
        with (
            tc.tile_pool(name="const", bufs=1) as const,
            tc.tile_pool(name="work", bufs=2) as work,
            tc.tile_pool(name="evw", bufs=6) as evw,
            tc.tile_pool(name="rope", bufs=2) as ropep,
            tc.tile_pool(name="pt", bufs=5) as ptp,
            tc.tile_pool(name="qraw", bufs=3) as qrawp,
            tc.tile_pool(name="ao", bufs=6) as aop,
            tc.tile_pool(name="ps_s2", bufs=2, space="PSUM") as ps_s2,
            tc.tile_pool(name="ps_y", bufs=2, space="PSUM") as ps_y,
            tc.tile_pool(name="ps_mm", bufs=2, space="PSUM") as ps_mm,
        ):
            early_ctx = ExitStack()
            early = early_ctx.enter_context(tc.tile_pool(name="early", bufs=1))

            # ---- inputs: wq/wk/xT first (qkv(0) consumes them first) ----
            wq_sb = early.tile([128, CT, FL], BF16)
            wk_sb = early.tile([128, CT, FL], BF16)
            xT_sb = early.tile([128, CT, T], BF16)
            # few, large DMAs: the HWDGE serializes ~625ns per DMA instruction.
            # xT halved with first halves first: qkv chunk 0/1 start earlier.
            nc.sync.dma_start(wq_sb[:], wq_d[:].rearrange("(k p) f -> p k f", p=128))
            for k in range(CT):
                nc.sync.dma_start(xT_sb[:, k, 0:T // 2],
                                  xT_d[k * 128:(k + 1) * 128, 0:T // 2])
            # wk after xT half-0: the primed q-mtile only needs wq + xT
            nc.sync.dma_start(wk_sb[:], wk_d[:].rearrange("(k p) f -> p k f", p=128))
            for k in range(CT):
                nc.sync.dma_start(xT_sb[:, k, T // 2:T],
                                  xT_d[k * 128:(k + 1) * 128, T // 2:T])
            wv_sb = early.tile([128, CT, FL], BF16)
            nc.sync.dma_start(wv_sb[:], wv_d[:].rearrange("(k p) f -> p k f", p=128))
            cos_sb = early.tile([128, T], BF16)
            nc.sync.dma_start(cos_sb[:], cos_d[:])
            sin_sb = early.tile([128, T], BF16)
            nc.sync.dma_start(sin_sb[:], sin_d[:])
            mneg_sb = const.tile([128, 128], BF16)
            nc.sync.dma_start(mneg_sb[:], mneg_d[:])
            ident2_sb = const.tile([128, 256], BF16)
            nc.sync.dma_start(ident2_sb[:], ident2_d[:])
            ident_sb = const.tile([128, 128], BF16)
            nc.sync.dma_start(ident_sb[:], ident_d[:])
            wp_sb = const.tile([128, CT, FL], BF16)
            nc.sync.dma_start(wp_sb[:], wp_d[:].rearrange("(k p) f -> p k f", p=128))

            # block-diagonal ones: one matmul sums squares of both packed heads
            ones_bd = const.tile([128, 128], BF16)
            nc.sync.dma_start(ones_bd[:], onesbd_d[:])

            qk_sb = const.tile([128, 2 * PAIRS, T], BF16)
            # v2: per tt, per pair: [A dims(64) | onesA | B dims(64) | onesB]
            v2_sb = const.tile([128, NTT, PAIRS * VW], BF16)
            aoT_sb = const.tile([128, PAIRS, T], BF16)
            # bf16 partial of the projection over already-gathered f-tiles
            ppart = const.tile([128, NTT, FL], BF16)

            # ones columns of v2 (once, before any v-proj writes)
            pstr = v2_sb.ap[0][0]
            ones_ap = bass.AP(
                tensor=v2_sb.tensor, offset=v2_sb.offset + HEAD_DIM,
                ap=[[pstr, 128], [PAIRS * VW, NTT], [VW, PAIRS], [HEAD_DIM + 1, 2]])
            nc.vector.memset(ones_ap, 1.0)

            # ---------------- unit generators (PE fillers) ----------------
            def v_unit(tt):
                pss = ps_mm.tile([128, FL], F32, tag="mm", name=f"vp{tt}")
                for k0 in range(0, CT, 2):
                    for k in (k0, k0 + 1):
                        nc.tensor.matmul(
                            pss,
                            lhsT=xT_sb[:, k, tt * 128:(tt + 1) * 128],
                            rhs=wv_sb[:, k, :],
                            start=(k == 0), stop=(k == CT - 1),
                        )
                        mm_cost(FL)
                    yield
                # drain into v2 layout: [128,4,2,64] both sides (DVE --
                # GPSIMD cannot touch PSUM on real hardware)
                src = _reap(pss, [(128, PAIRS), (64, 2), (1, 64)])
                dst = bass.AP(tensor=v2_sb.tensor,
                              offset=v2_sb.offset + tt * PAIRS * VW,
                              ap=[[pstr, 128], [VW, PAIRS], [HEAD_DIM + 1, 2], [1, 64]])
                nc.vector.tensor_copy(dst, src)
                yield

            def qkv_tail(m, pss, sq, n):
                ss = ps_y.tile([128, QCH], F32, tag="y", name=f"ss{m}_{n}")
                nc.tensor.matmul(ss, lhsT=ones_bd[:], rhs=sq[:],
                                 start=True, stop=True, skip_group_check=True)
                mm_cost(QCH)
                # rstd = (ss/64)^(-1/2) = exp(-0.5*ln(ss/64)); eps negligible.
                rr = work.tile([128, QCH], BF16, tag="rr")
                nc.scalar.activation(rr[:], ss,
                                     mybir.ActivationFunctionType.Ln,
                                     scale=1.0 / HEAD_DIM)
                act_cost(QCH)
                rstd = work.tile([128, QCH], BF16, tag="rs")
                nc.scalar.activation(rstd[:], rr[:],
                                     mybir.ActivationFunctionType.Exp,
                                     scale=-0.5)
                act_cost(QCH)
                nc.vector.tensor_mul(qk_sb[:, m, n * QCH:(n + 1) * QCH],
                                     pss, rstd[:])

            # lazy rope: one closure per DVE op, drained one per qkv yield so
            # the qkv norm-muls (psum release) never queue behind a burst
            ropeq = deque()

            def rope_mtile(m):
                src = qk_sb[:, m, :]
                sw = ropep.tile([128, T], BF16, tag="rp")
                for off in (0, 64):
                    ropeq.append(lambda o=off: nc.vector.tensor_copy(
                        sw[o:o + 32, :], src[o + 32:o + 64, :]))
                    ropeq.append(lambda o=off: nc.vector.tensor_copy(
                        sw[o + 32:o + 64, :], src[o:o + 32, :]))
                ropeq.append(lambda: nc.vector.tensor_mul(sw[:], sw[:], sin_sb[:]))
                ropeq.append(lambda: nc.vector.tensor_mul(src, src, cos_sb[:]))
                ropeq.append(lambda: nc.vector.tensor_add(src, src, sw[:]))

            def qkv_unit(m):
                # pss borrows the (pre-phase-idle) s2 slots, ss the y slots:
                # the qkv pipeline gets baseline's 4-bank depth while the
                # attention pools keep their static reservation.
                # private 2-deep pss ring per generator (q-mtiles borrow the
                # pre-phase-idle s2 slots, k-mtiles the mm slots): a slot is
                # reused only 2 chunks later, far past its tail chain
                w_sb = wq_sb if m < PAIRS else wk_sb
                mloc = (m % PAIRS) * 128
                pend = None
                ps2 = None
                for n in range(NQC):
                    if m < PAIRS:
                        # 4-deep ring: two chunks per 2-bank s2 slot
                        if n % 2 == 0:
                            ps2 = ps_s2.tile([128, 2, QCH], F32, tag="s2",
                                             name=f"qk{m}_{n}")
                        pss = ps2[:, n % 2, :]
                    else:
                        pss = ps_mm.tile([128, QCH], F32, tag="mm",
                                         name=f"qk{m}_{n}")
                    for k0 in range(0, CT, 2):
                        for k in (k0, k0 + 1):
                            nc.tensor.matmul(
                                pss,
                                lhsT=w_sb[:, k, mloc:mloc + 128],
                                rhs=xT_sb[:, k, n * QCH:(n + 1) * QCH],
                                start=(k == 0), stop=(k == CT - 1),
                                skip_group_check=True,
                            )
                            mm_cost(QCH)
                        yield
                    # square must be ACT: DVE cannot read two PSUM operands
                    sq = work.tile([128, QCH], BF16, tag="sq")
                    nc.scalar.activation(sq[:], pss,
                                         mybir.ActivationFunctionType.Square)
                    act_cost(QCH)
                    if pend is not None:
                        qkv_tail(m, *pend)
                        yield
                    pend = (pss, sq, n)
                qkv_tail(m, *pend)
                yield
                rope_mtile(m)

            def qkv_pair_gen(pn):
                # alternate q- and k-mtile steps (q primed 2 chunks ahead so
                # the two tails never bunch at the same boundary)
                a, b = qkv_unit(pn), qkv_unit(PAIRS + pn)
                for _ in range(8):
                    try:
                        next(a)
                    except StopIteration:
                        break
                while True:
                    na = nb = False
                    try:
                        next(a)
                    except StopIteration:
                        na = True
                    if ropeq:
                        ropeq.popleft()()
                    try:
                        next(b)
                    except StopIteration:
                        nb = True
                    if ropeq:
                        ropeq.popleft()()
                    if na and nb:
                        return
                    yield

            def proj_stage_gen(kfs, first_stage):
                for tt in range(NTT):
                    pss = ps_mm.tile([128, FL], F32, tag="mm",
                                     name=f"pj{kfs[0]}_{tt}")
                    for i, kf in enumerate(kfs):
                        nc.tensor.matmul(
                            pss,
                            lhsT=qk_sb[:, kf, tt * 128:(tt + 1) * 128],
                            rhs=wp_sb[:, kf, :],
                            start=(i == 0), stop=(i == len(kfs) - 1),
                        )
                        mm_cost(FL)
                        yield
                    pp = ppart[:, tt, :]
                    if first_stage:
                        nc.vector.tensor_copy(pp, pss)
                    else:
                        nc.vector.tensor_add(pp, pss, pp)
                    yield

            # ---------------- filler machinery ----------------
            # list of [label, generator, done]; fill_step picks the first
            # not-done generator whose gate is open (order = priority).
            fillers = []
            gate_from = {}   # label -> cc whose writeback must be issued
            cc_issued = set()
            cur_pair = [0]

            def fill_step():
                for ent in fillers:
                    if ent[2] or gate_from.get(ent[0], -1) not in cc_issued | {-1}:
                        continue
                    try:
                        next(ent[1])
                        return True
                    except StopIteration:
                        ent[2] = True
                        continue
                return False

            FILL_BIAS = 2000.0   # keep PE a bit ahead of ACT (rows-equiv)

            def fill():
                # pair 0: steady front-loaded drip of deferred v-proj tiles
                # (AV(j=tt) consumes v2 tile tt -- the drip beats deadlines)
                if cur_pair[0] == 0 and not fillers[0][2]:
                    for _ in range(2):
                        try:
                            next(fillers[0][1])
                        except StopIteration:
                            fillers[0][2] = True
                            break
                while led["pe"] < led["act"] + FILL_BIAS:
                    if not fill_step():
                        break

            def flush(label):
                for ent in fillers:
                    if ent[0] == label and not ent[2]:
                        for _ in ent[1]:
                            pass
                        ent[2] = True

            # ---------------- attention (flipped AV) ----------------
            # cross-pair queue of deferred work: ("tr", p, ao, qt) transposes
            # and ("cc", p) exchange launches drain one per j-iteration, so
            # cqi/pair epilogues never idle the scalar engine
            postq = deque()

            def do_transpose(p, ao, qt):
                # scratch from the mm ring: a transpose in the s2 ring would
                # halve the QK pipeline's lookahead (the scores ring is the
                # attention loop's binding resource)
                tr_t = ps_mm.tile([128, FL], F32, tag="mm",
                                  name=f"tr{p}_{qt}")
                area = tr_t[:, 0:64].bitcast(BF16)
                nc.tensor.transpose(area, ao[:].opt(), ident_sb[:])
                mm_cost(128)
                nc.vector.tensor_copy(
                    aoT_sb[:, p, qt * 128:(qt + 1) * 128], area)

            def do_cc(p):
                nc.sync.dma_start(cc_ins[p][:], aoT_sb[:, p, :])
                if not no_cc:
                    nc.gpsimd.collective_compute(
                        "AllGather",
                        mybir.AluOpType.bypass,
                        replica_groups=[[0, 1], [2, 3], [4, 5], [6, 7]],
                        ins=[cc_ins[p][:].opt()],
                        outs=[cc_outs[p][:].opt()],
                    )
                # qk slots p and 4+p are dead after attention p: receive the
                # gathered pair there (slot index == global f-tile index)
                if no_cc:
                    nc.sync.dma_start(qk_sb[:, p, :], cc_ins[p][:])
                    nc.sync.dma_start(qk_sb[:, PAIRS + p, :], cc_ins[p][:])
                else:
                    nc.sync.dma_start(qk_sb[:, p, :], cc_outs[p][0])
                    nc.sync.dma_start(qk_sb[:, PAIRS + p, :], cc_outs[p][1])
                cc_issued.add(p)

            def pop_post(curp=None):
                # keep >=3 of the CURRENT pair's transposes pending (their
                # divs need DVE time); older pairs' items drain immediately
                if not postq:
                    return False
                if len(postq) <= 3 and postq[0][1] == curp:
                    return False
                it = postq.popleft()
                if it[0] == "tr":
                    do_transpose(*it[1:])
                else:
                    do_cc(it[1])
                return True

            # AV issuance lags QK/exp by 2 k-tiles and carries across
            # q-chunk and pair boundaries: the scalar engine's exp stream
            # never waits for an epilogue
            pend_av = deque()

            def attention_pair(p):
                qT = qk_sb[:, p, :]
                kT = qk_sb[:, PAIRS + p, :]
                vbase = p * VW

                def do_qk(cqi, j):
                    qg0 = cqi * 4
                    jr = j - qg0
                    q0 = max(jr, 0) * 128
                    sq_sl = slice(cqi * QCH + q0, (cqi + 1) * QCH)
                    s2 = ps_s2.tile([128, 2, QCH], F32, tag="s2",
                                    name=f"s2_{p}_{cqi}_{j}")
                    nc.tensor.matmul(s2[:, 0, q0:QCH],
                                     lhsT=kT[0:64, j * 128:(j + 1) * 128],
                                     rhs=qT[0:64, sq_sl], start=True,
                                     stop=(jr < 0),
                                     skip_group_check=True)
                    mm_cost(QCH - q0)
                    nc.tensor.matmul(s2[:, 1, q0:QCH],
                                     lhsT=kT[64:128, j * 128:(j + 1) * 128],
                                     rhs=qT[64:128, sq_sl], start=True,
                                     stop=(jr < 0),
                                     skip_group_check=True)
                    mm_cost(QCH - q0)
                    if jr >= 0:
                        # causal bias on the diagonal block (per head --
                        # a matmul output must stay inside one psum bank)
                        for h in range(2):
                            nc.tensor.matmul(
                                s2[:, h, q0:q0 + 128], lhsT=mneg_sb[:],
                                rhs=ident2_sb[:, 0:128],
                                start=False, stop=(h == 1),
                                skip_group_check=True)
                            mm_cost(128)
                    pt = ptp.tile([128, 2, QCH], BF16, tag="pt")
                    nc.scalar.activation(pt[:, :, q0:QCH], s2[:, :, q0:QCH],
                                         mybir.ActivationFunctionType.Exp,
                                         scale=0.125)
                    act_cost(2 * (QCH - q0))
                    return pt

                for cqi in range(NQC):
                    qg0 = cqi * 4          # first global qtile of this chunk
                    kmax = qg0 + 4
                    y_t = [ps_y.tile([128, 2 * VW], F32, tag="y",
                                     name=f"yp{p}_{cqi}_{h}")
                           for h in range(2)]

                    def finalize(qt, yt, base):
                        # dens at cols base+64 and base+129
                        recip2 = work.tile([128, 2], F32, tag="rc",
                                           name=f"rc{p}_{qt}")
                        den_ap = _reap(yt[:, base + 64], [(HEAD_DIM + 1, 2)])
                        nc.vector.reciprocal_approx_fast(recip2[:], den_ap)
                        ao = aop.tile([128, 2, 64], BF16, tag="ao",
                                      name=f"ao{p}_{qt}")
                        for h in range(2):
                            nc.vector.tensor_scalar_mul(
                                ao[:, h, :],
                                _reap(yt[:, base + h * (HEAD_DIM + 1)], [(1, 64)]),
                                recip2[:, h:h + 1])
                        postq.append(("tr", p, ao, qt))

                    def issue_av(pt, j, qg0=qg0, y_t=y_t):
                        # default args bind THIS q-chunk's state: entries are
                        # popped after the loop variables have moved on
                        jr = j - qg0
                        for qtl in range(max(jr, 0), 4):
                            qt = qg0 + qtl
                            yt = y_t[qtl // 2]
                            base = (qtl % 2) * VW
                            for h in range(2):
                                o = h * (HEAD_DIM + 1)
                                # psum 'start' poisons the whole zero-region
                                # (bank): only the FIRST matmul touching each
                                # y tile may set it; the lazy zero-on-write
                                # initializes the other head/qt sub-groups
                                nc.tensor.matmul(
                                    yt[:, base + o: base + o + HEAD_DIM + 1],
                                    lhsT=pt[:, h, qtl * 128:(qtl + 1) * 128],
                                    rhs=v2_sb[:, j, vbase + o: vbase + o + HEAD_DIM + 1],
                                    start=(j == 0 and h == 0 and qtl % 2 == 0),
                                    stop=(j == qt),
                                    skip_group_check=True,
                                )
                                mm_cost(HEAD_DIM + 1)
                            if j == qt:
                                finalize(qt, yt, base)

                    for j in range(kmax):
                        pt = do_qk(cqi, j)
                        pend_av.append((issue_av, pt, j))
                        # one deferred transpose/cc per iteration
                        pop_post(p)
                        # fillers go BEFORE the lagged AV so the in-order PE
                        # chews them while exp(j-2) finishes
                        fill()
                        if len(pend_av) > 2:
                            fn, pt_, j_ = pend_av.popleft()
                            fn(pt_, j_)
                    # q-chunk epilogue: the accumulator ring recycles at the
                    # next y_t alloc, so all its AVs must be issued NOW (the
                    # transposes/cc stay deferred -- they use fresh scratch)
                    while pend_av:
                        fn, pt_, j_ = pend_av.popleft()
                        fn(pt_, j_)

            # ================= program =================
            # pre-phase: ALL qkv+norm+rope (their ACT/DVE tails hide under
            # the PE-dense mains), then v-proj tiles 0-3.  v tiles 4-15 and
            # the staged projection are the attention fillers: pure PE work
            # with no cross-engine chains to tangle with the attention loop.
            for pn in range(PAIRS):
                for _ in qkv_pair_gen(pn):
                    pass
            while ropeq:    # last pair's rope ops
                ropeq.popleft()()
            for tt in range(6):
                for _ in v_unit(tt):
                    pass

            def vdef_gen():
                for tt in range(6, NTT):
                    yield from v_unit(tt)

            fillers.append(["vdef", vdef_gen(), False])
            fillers.append(["projA", proj_stage_gen([0, PAIRS], True), False])
            fillers.append(["projB", proj_stage_gen([1, PAIRS + 1], False), False])
            fillers.append(["projC", proj_stage_gen([2, PAIRS + 2], False), False])
            gate_from.update({"projA": 0, "projB": 1, "projC": 2})

            for p in range(PAIRS):
                cur_pair[0] = p
                led["pe"] = led["act"] = 0.0
                attention_pair(p)
                # pair's transposes are already queued to postq (its last
                # q-chunk flushed pend_av), so FIFO keeps cc after them
                postq.append(("cc", p))
                if p == PAIRS - 2:
                    flush("vdef")  # safety: wv/xT die with the early pool
                    early_ctx.close()

            cur_pair[0] = PAIRS
            while postq:
                pop_post(None)
            for ent in fillers:
                flush(ent[0])

            # ---- projection tail: last pair's f-tiles + combine ----
            # psum cycles all three pools (6 slots): the adds/DMAs pipeline
            for tt in range(NTT):
                sel = tt % 3
                if sel == 0:
                    pss = ps_mm.tile([128, FL], F32, tag="mm", name=f"pf{tt}")
                elif sel == 1:
                    pss = ps_s2.tile([128, 2, QCH], F32, tag="s2",
                                     name=f"pf{tt}")[:, 0, :]
                else:
                    pss = ps_y.tile([128, FL], F32, tag="y", name=f"pf{tt}")
                for i, kf in enumerate([PAIRS - 1, 2 * PAIRS - 1]):
                    nc.tensor.matmul(
                        pss,
                        lhsT=qk_sb[:, kf, tt * 128:(tt + 1) * 128],
                        rhs=wp_sb[:, kf, :],
                        start=(i == 0), stop=(i == 1),
                    )
                    mm_cost(FL)
                ysb = evw.tile([128, FL], F32, tag="ev")
                nc.vector.tensor_add(ysb[:], pss, ppart[:, tt, :])
                nc.sync.dma_start(y_d[tt * 128:(tt + 1) * 128, :], ysb[:])

    nc.compile()
    return nc


def _prep_core_inputs(x, Wqkv, Wproj, q_norm_w, k_norm_w, core):
    b, g = core // 2, core % 2
    bf = ml_dtypes.bfloat16
    xT = np.ascontiguousarray(x[b].T).astype(bf)
    cols = slice(g * FL, (g + 1) * FL)
    wq = Wqkv[:, 0:C][:, cols] * np.tile(q_norm_w, H_LOCAL)[None, :]
    wk = Wqkv[:, C:2 * C][:, cols] * np.tile(k_norm_w, H_LOCAL)[None, :]
    wv = Wqkv[:, 2 * C:3 * C][:, cols]
    wp = Wproj[:, cols]
    return {
        "xT": xT,
        "Wq": np.ascontiguousarray(wq).astype(bf),
        "Wk": np.ascontiguousarray(wk).astype(bf),
        "Wv": np.ascontiguousarray(wv).astype(bf),
        "Wp": np.ascontiguousarray(wp).astype(bf),
    }


def kernel(x, Wqkv, Wproj, q_norm_w, k_norm_w):
    if "nc" not in _cached:
        _cached["nc"] = build_program()
    nc = _cached["nc"]

    x = np.asarray(x, dtype=np.float32)
    Wqkv = np.asarray(Wqkv, dtype=np.float32)
    Wproj = np.asarray(Wproj, dtype=np.float32)
    q_norm_w = np.asarray(q_norm_w, dtype=np.float32)
    k_norm_w = np.asarray(k_norm_w, dtype=np.float32)

    in_maps = [
        _prep_core_inputs(x, Wqkv, Wproj, q_norm_w, k_norm_w, c) for c in range(8)
    ]
    res = run_bass_kernel_spmd(nc, in_maps, list(range(8)))
    outs = res.results

    y = np.empty((B, T, C), dtype=np.float32)
    for b in range(B):
        y[b, :, 0:FL] = outs[2 * b]["y"]
        y[b, :, FL:C] = outs[2 * b + 1]["y"]
    return y


# revision 78
# speedup vs baseline: 1.0287x; 1.0062x over previous
"""Causal self-attention (QK-RMSNorm + RoPE) Trainium2 kernel.

Sharding: 8 cores = 4 batches x 2 head-groups (Megatron-style over heads).
Core c handles batch b=c//2, heads [g*8, g*8+8) with g=c%2.
Each core computes y[b, :, g*512:(g+1)*512] (output-column sharding of the
projection after a pairwise AllGather of attention outputs), so the host
only concatenates slices - no host-side arithmetic.

Perf notes (cost model charges out-free-size rows per matmul, independent of
contraction depth and output-partition count):
- AV is computed in the [q-tokens(part), head-dims(free)] orientation with a
  ones column appended to V per head: the 65-wide moving tensor makes AV cost
  65 rows/tile instead of 128-512, and the softmax denominator accumulates
  for free in column 64.  The division is then a per-partition scalar
  multiply (DVE), and the output is transposed back to [dims, tok] with
  cheap PE transposes (128 rows each) for the AllGather + projection.
- The per-head sum-of-squares for QK-RMSNorm uses one block-diagonal-ones
  matmul covering both packed heads; squares are computed on DVE in bf16
  from a Pool-engine drain of the qkv psum (keeps the scalar engine free
  for the attention exp()s, which are its binding load).
- The in-order PE is kept saturated (and in max p-state) by interleaving
  filler matmuls into the attention loop, driven by a PE-vs-ACT issued-work
  ledger: qkv of the next pair, deferred v-proj tiles, and the partial
  projection of already-gathered f-tiles (staged per AllGather arrival,
  accumulated into a bf16 partial on the Pool engine).  Only the last
  pair's two f-tiles + a DVE add remain after the final AllGather.
- Transpose scratch lives in the s2 (scores) PSUM ring, so the AV
  accumulator ring is released by the division and never serializes
  consecutive q-chunks.
"""


import numpy as np
import ml_dtypes
from collections import deque
from contextlib import ExitStack

import concourse.bass as bass
import concourse.bacc as bacc

# Force all activations into the one table set that covers Exp+Ln+Square+
# Copy+Identity, so the whole kernel needs exactly one ACT_TABLE_LOAD.
import concourse.hw_specs as _hw_specs
_orig_gat = _hw_specs.get_activation_tables

def _gat_one_set(arch):
    t = _orig_gat(arch)
    return {k: (v if k == "natural_log_exp_and_others" else set())
            for k, v in t.items()}

bacc.get_activation_tables = _gat_one_set
import concourse.mybir as mybir
import concourse.tile as tile
from concourse.bass_utils import run_bass_kernel_spmd

BF16 = mybir.dt.bfloat16
F32 = mybir.dt.float32

N_HEAD = 16
HEAD_DIM = 64
EPS = 1e-5
ROPE_BASE = 10000.0

B, T, C = 4, 2048, 1024
H_LOCAL = N_HEAD // 2          # heads per core
PAIRS = H_LOCAL // 2           # head-pairs per core (processed 2-at-a-time)
CT = C // 128                  # contraction tiles over C
FL = H_LOCAL * HEAD_DIM        # local feature width (512)
QCH = 512                      # q-chunk width
NQC = T // QCH                 # q-chunks
NKT = T // 128                 # k tiles
NTT = T // 128                 # token tiles
VW = 2 * (HEAD_DIM + 1)        # per-pair v2 width: [A dims|onesA|B dims|onesB]

_cached = {}


def _reap(ap, dims):
    """Rebuild an AP keeping tensor/offset/partition dim, with free dims
    `dims` given as (stride, size) pairs."""
    return bass.AP(tensor=ap.tensor, offset=ap.offset,
                   ap=[ap.ap[0]] + [list(d) for d in dims])


def _fbcast2(ap):
    """[128, N] AP -> [128, 2, N] with the middle (free) dim broadcast."""
    return bass.AP(
        tensor=ap.tensor, offset=ap.offset, ap=[ap.ap[0], [0, 2], ap.ap[1]]
    )


def _rope_tables():
    inv_freq = 1.0 / (ROPE_BASE ** (np.arange(0, HEAD_DIM, 2, dtype=np.float64) / HEAD_DIM))
    t = np.arange(T, dtype=np.float64)
    freqs = np.outer(t, inv_freq)                       # [T, 32]
    emb = np.concatenate([freqs, freqs], -1)            # [T, 64]
    cos = np.cos(emb).astype(np.float32).T              # [64, T]
    sin = np.sin(emb).astype(np.float32).T              # [64, T]
    cos2 = np.concatenate([cos, cos], 0)                # [128, T] two heads
    sin_s = sin.copy()
    sin_s[0:32] = -sin_s[0:32]                          # rotate-half sign
    sin2 = np.concatenate([sin_s, sin_s], 0)            # [128, T]
    return cos2.astype(ml_dtypes.bfloat16), sin2.astype(ml_dtypes.bfloat16)


def _diag_masks():
    # corner mask: keep where k_partition <= q_col (lower-triangular 128x128)
    p = np.arange(128)[:, None]
    qf = np.arange(128)[None, :]
    m = (p <= qf).astype(np.float32)
    return m.astype(ml_dtypes.bfloat16)                 # [128, 128]


def build_program(no_cc=False):
    nc = bacc.Bacc("TRN2", target_bir_lowering=False, debug=False,
                   num_devices=1 if no_cc else 8)

    xT_d = nc.dram_tensor("xT", [C, T], BF16, kind="ExternalInput")
    wq_d = nc.dram_tensor("Wq", [C, FL], BF16, kind="ExternalInput")
    wk_d = nc.dram_tensor("Wk", [C, FL], BF16, kind="ExternalInput")
    wv_d = nc.dram_tensor("Wv", [C, FL], BF16, kind="ExternalInput")
    wp_d = nc.dram_tensor("Wp", [C, FL], BF16, kind="ExternalInput")
    y_d = nc.dram_tensor("y", [T, FL], F32, kind="ExternalOutput")

    cos2_np, sin2_np = _rope_tables()
    cos_d = nc.inline_tensor(np.ascontiguousarray(cos2_np), "cos2")
    sin_d = nc.inline_tensor(np.ascontiguousarray(sin2_np), "sin2")
    # causal mask as a score bias: out[p,g,f] += mneg[f,p] = -30000*(p>f),
    # added to the diagonal 128x128 block by one PE matmul (keeps the
    # exp->AV chain off the vector engine)
    mneg_np = -30000.0 * (np.arange(128)[None, :] > np.arange(128)[:, None])
    mneg_d = nc.inline_tensor(
        np.ascontiguousarray(mneg_np.astype(ml_dtypes.bfloat16)), "mneg")
    id2_np = np.tile(np.eye(128, dtype=ml_dtypes.bfloat16), (1, 2))
    ident2_d = nc.inline_tensor(np.ascontiguousarray(id2_np), "ident2")
    bd_np = np.zeros((128, 128), dtype=ml_dtypes.bfloat16)
    bd_np[0:64, 0:64] = 1.0
    bd_np[64:128, 64:128] = 1.0
    onesbd_d = nc.inline_tensor(np.ascontiguousarray(bd_np), "onesbd")
    ident_d = nc.inline_tensor(
        np.ascontiguousarray(np.eye(128, dtype=ml_dtypes.bfloat16)), "ident")

    # per-pair exchange buffers
    cc_ins = [nc.dram_tensor(f"cc_in{p}", [128, T], BF16) for p in range(PAIRS)]
    cc_outs = [nc.dram_tensor(f"cc_out{p}", [2, 128, T], BF16) for p in range(PAIRS)]

    # --- PE-vs-ACT issued-work ledger (units: bf16 matmul rows = 0.4167ns) ---
    led = {"pe": 0.0, "act": 0.0}

    def mm_cost(rows):
        led["pe"] += rows

    def act_cost(free):
        led["act"] += 2.0 * free + 444.0

    with tile.TileContext(nc) as tc:
        with (
            tc.tile_pool(name="const", bufs=1) as const,
            tc.tile_pool(name="work", bufs=2) as work,
            tc.tile_pool(name="evw", bufs=6) as evw,
            tc.tile_pool(name="rope", bufs=2) as ropep,
            tc.tile_pool(name="pt", bufs=6) as ptp,
            tc.tile_pool(name="qraw", bufs=3) as qrawp,
            tc.tile_pool(name="ao", bufs=6) as aop,
            tc.tile_pool(name="ps_s2", bufs=2, space="PSUM") as ps_s2,
            tc.tile_pool(name="ps_y", bufs=2, space="PSUM") as ps_y,
            tc.tile_pool(name="ps_mm", bufs=2, space="PSUM") as ps_mm,
        ):
            early_ctx = ExitStack()
            early = early_ctx.enter_context(tc.tile_pool(name="early", bufs=1))

            # ---- inputs: wq/wk/xT first (qkv(0) consumes them first) ----
            wq_sb = early.tile([128, CT, FL], BF16)
            wk_sb = early.tile([128, CT, FL], BF16)
            xT_sb = early.tile([128, CT, T], BF16)
            # few, large DMAs: the HWDGE serializes ~625ns per DMA instruction.
            # xT halved with first halves first: qkv chunk 0/1 start earlier.
            nc.sync.dma_start(wq_sb[:], wq_d[:].rearrange("(k p) f -> p k f", p=128))
            for k in range(CT):
                nc.sync.dma_start(xT_sb[:, k, 0:T // 2],
                                  xT_d[k * 128:(k + 1) * 128, 0:T // 2])
            # wk after xT half-0: the primed q-mtile only needs wq + xT
            nc.sync.dma_start(wk_sb[:], wk_d[:].rearrange("(k p) f -> p k f", p=128))
            for k in range(CT):
                nc.sync.dma_start(xT_sb[:, k, T // 2:T],
                                  xT_d[k * 128:(k + 1) * 128, T // 2:T])
            wv_sb = early.tile([128, CT, FL], BF16)
            nc.sync.dma_start(wv_sb[:], wv_d[:].rearrange("(k p) f -> p k f", p=128))
            cos_sb = early.tile([128, T], BF16)
            nc.sync.dma_start(cos_sb[:], cos_d[:])
            sin_sb = early.tile([128, T], BF16)
            nc.sync.dma_start(sin_sb[:], sin_d[:])
            mneg_sb = const.tile([128, 128], BF16)
            nc.sync.dma_start(mneg_sb[:], mneg_d[:])
            ident2_sb = const.tile([128, 256], BF16)
            nc.sync.dma_start(ident2_sb[:], ident2_d[:])
            ident_sb = const.tile([128, 128], BF16)
            nc.sync.dma_start(ident_sb[:], ident_d[:])
            wp_sb = const.tile([128, CT, FL], BF16)
            nc.sync.dma_start(wp_sb[:], wp_d[:].rearrange("(k p) f -> p k f", p=128))

            # block-diagonal ones: one matmul sums squares of both packed heads
            ones_bd = const.tile([128, 128], BF16)
            nc.sync.dma_start(ones_bd[:], onesbd_d[:])

            qk_sb = const.tile([128, 2 * PAIRS, T], BF16)
            # v2: per tt, per pair: [A dims(64) | onesA | B dims(64) | onesB]
            v2_sb = const.tile([128, NTT, PAIRS * VW], BF16)
            aoT_sb = const.tile([128, PAIRS, T], BF16)
            # bf16 partial of the projection over already-gathered f-tiles
            ppart = const.tile([128, NTT, FL], BF16)

            # ones columns of v2 (once, before any v-proj writes)
            pstr = v2_sb.ap[0][0]
            ones_ap = bass.AP(
                tensor=v2_sb.tensor, offset=v2_sb.offset + HEAD_DIM,
                ap=[[pstr, 128], [PAIRS * VW, NTT], [VW, PAIRS], [HEAD_DIM + 1, 2]])
            nc.vector.memset(ones_ap, 1.0)

            # ---------------- unit generators (PE fillers) ----------------
            def v_unit(tt):
                pss = ps_mm.tile([128, FL], F32, tag="mm", name=f"vp{tt}")
                for k0 in range(0, CT, 2):
                    for k in (k0, k0 + 1):
                        nc.tensor.matmul(
                            pss,
                            lhsT=xT_sb[:, k, tt * 128:(tt + 1) * 128],
                            rhs=wv_sb[:, k, :],
                            start=(k == 0), stop=(k == CT - 1),
                        )
                        mm_cost(FL)
                    yield
                # drain into v2 layout: [128,4,2,64] both sides (DVE --
                # GPSIMD cannot touch PSUM on real hardware)
                src = _reap(pss, [(128, PAIRS), (64, 2), (1, 64)])
                dst = bass.AP(tensor=v2_sb.tensor,
                              offset=v2_sb.offset + tt * PAIRS * VW,
                              ap=[[pstr, 128], [VW, PAIRS], [HEAD_DIM + 1, 2], [1, 64]])
                nc.vector.tensor_copy(dst, src)
                yield

            def qkv_tail(m, pss, sq, n):
                ss = ps_y.tile([128, QCH], F32, tag="y", name=f"ss{m}_{n}")
                nc.tensor.matmul(ss, lhsT=ones_bd[:], rhs=sq[:],
                                 start=True, stop=True, skip_group_check=True)
                mm_cost(QCH)
                # rstd = (ss/64)^(-1/2) = exp(-0.5*ln(ss/64)); eps negligible.
                rr = work.tile([128, QCH], BF16, tag="rr")
                nc.scalar.activation(rr[:], ss,
                                     mybir.ActivationFunctionType.Ln,
                                     scale=1.0 / HEAD_DIM)
                act_cost(QCH)
                rstd = work.tile([128, QCH], BF16, tag="rs")
                nc.scalar.activation(rstd[:], rr[:],
                                     mybir.ActivationFunctionType.Exp,
                                     scale=-0.5)
                act_cost(QCH)
                nc.vector.tensor_mul(qk_sb[:, m, n * QCH:(n + 1) * QCH],
                                     pss, rstd[:])

            # lazy rope: one closure per DVE op, drained one per qkv yield so
            # the qkv norm-muls (psum release) never queue behind a burst
            ropeq = deque()

            def rope_mtile(m):
                src = qk_sb[:, m, :]
                sw = ropep.tile([128, T], BF16, tag="rp")
                for off in (0, 64):
                    ropeq.append(lambda o=off: nc.vector.tensor_copy(
                        sw[o:o + 32, :], src[o + 32:o + 64, :]))
                    ropeq.append(lambda o=off: nc.vector.tensor_copy(
                        sw[o + 32:o + 64, :], src[o:o + 32, :]))
                ropeq.append(lambda: nc.vector.tensor_mul(sw[:], sw[:], sin_sb[:]))
                ropeq.append(lambda: nc.vector.tensor_mul(src, src, cos_sb[:]))
                ropeq.append(lambda: nc.vector.tensor_add(src, src, sw[:]))

            def qkv_unit(m):
                # pss borrows the (pre-phase-idle) s2 slots, ss the y slots:
                # the qkv pipeline gets baseline's 4-bank depth while the
                # attention pools keep their static reservation.
                # private 2-deep pss ring per generator (q-mtiles borrow the
                # pre-phase-idle s2 slots, k-mtiles the mm slots): a slot is
                # reused only 2 chunks later, far past its tail chain
                w_sb = wq_sb if m < PAIRS else wk_sb
                mloc = (m % PAIRS) * 128
                pend = None
                ps2 = None
                for n in range(NQC):
                    if m < PAIRS:
                        # 4-deep ring: two chunks per 2-bank s2 slot
                        if n % 2 == 0:
                            ps2 = ps_s2.tile([128, 2, QCH], F32, tag="s2",
                                             name=f"qk{m}_{n}")
                        pss = ps2[:, n % 2, :]
                    else:
                        pss = ps_mm.tile([128, QCH], F32, tag="mm",
                                         name=f"qk{m}_{n}")
                    for k0 in range(0, CT, 2):
                        for k in (k0, k0 + 1):
                            nc.tensor.matmul(
                                pss,
                                lhsT=w_sb[:, k, mloc:mloc + 128],
                                rhs=xT_sb[:, k, n * QCH:(n + 1) * QCH],
                                start=(k == 0), stop=(k == CT - 1),
                                skip_group_check=True,
                            )
                            mm_cost(QCH)
                        yield
                    # square must be ACT: DVE cannot read two PSUM operands
                    sq = work.tile([128, QCH], BF16, tag="sq")
                    nc.scalar.activation(sq[:], pss,
                                         mybir.ActivationFunctionType.Square)
                    act_cost(QCH)
                    if pend is not None:
                        qkv_tail(m, *pend)
                        yield
                    pend = (pss, sq, n)
                qkv_tail(m, *pend)
                yield
                rope_mtile(m)

            def qkv_pair_gen(pn):
                # alternate q- and k-mtile steps (q primed 2 chunks ahead so
                # the two tails never bunch at the same boundary)
                a, b = qkv_unit(pn), qkv_unit(PAIRS + pn)
                for _ in range(8):
                    try:
                        next(a)
                    except StopIteration:
                        break
                while True:
                    na = nb = False
                    try:
                        next(a)
                    except StopIteration:
                        na = True
                    if ropeq:
                        ropeq.popleft()()
                    try:
                        next(b)
                    except StopIteration:
                        nb = True
                    if ropeq:
                        ropeq.popleft()()
                    if na and nb:
                        return
                    yield

            def proj_stage_gen(kfs, first_stage):
                for tt in range(NTT):
                    pss = ps_mm.tile([128, FL], F32, tag="mm",
                                     name=f"pj{kfs[0]}_{tt}")
                    for i, kf in enumerate(kfs):
                        nc.tensor.matmul(
                            pss,
                            lhsT=qk_sb[:, kf, tt * 128:(tt + 1) * 128],
                            rhs=wp_sb[:, kf, :],
                            start=(i == 0), stop=(i == len(kfs) - 1),
                        )
                        mm_cost(FL)
                        yield
                    pp = ppart[:, tt, :]
                    if first_stage:
                        nc.vector.tensor_copy(pp, pss)
                    else:
                        nc.vector.tensor_add(pp, pss, pp)
                    yield

            # ---------------- filler machinery ----------------
            # list of [label, generator, done]; fill_step picks the first
            # not-done generator whose gate is open (order = priority).
            fillers = []
            gate_from = {}   # label -> cc whose writeback must be issued
            cc_issued = set()
            cur_pair = [0]

            def fill_step():
                for ent in fillers:
                    if ent[2] or gate_from.get(ent[0], -1) not in cc_issued | {-1}:
                        continue
                    try:
                        next(ent[1])
                        return True
                    except StopIteration:
                        ent[2] = True
                        continue
                return False

            FILL_BIAS = 2000.0   # keep PE a bit ahead of ACT (rows-equiv)

            def fill():
                # pair 0: steady front-loaded drip of deferred v-proj tiles
                # (AV(j=tt) consumes v2 tile tt -- the drip beats deadlines)
                if cur_pair[0] == 0 and not fillers[0][2]:
                    for _ in range(2):
                        try:
                            next(fillers[0][1])
                        except StopIteration:
                            fillers[0][2] = True
                            break
                while led["pe"] < led["act"] + FILL_BIAS:
                    if not fill_step():
                        break

            def flush(label):
                for ent in fillers:
                    if ent[0] == label and not ent[2]:
                        for _ in ent[1]:
                            pass
                        ent[2] = True

            # ---------------- attention (flipped AV) ----------------
            # cross-pair queue of deferred work: ("tr", p, ao, qt) transposes
            # and ("cc", p) exchange launches drain one per j-iteration, so
            # cqi/pair epilogues never idle the scalar engine
            postq = deque()

            def do_transpose(p, ao, qt):
                # scratch from the mm ring: a transpose in the s2 ring would
                # halve the QK pipeline's lookahead (the scores ring is the
                # attention loop's binding resource)
                tr_t = ps_mm.tile([128, FL], F32, tag="mm",
                                  name=f"tr{p}_{qt}")
                area = tr_t[:, 0:64].bitcast(BF16)
                nc.tensor.transpose(area, ao[:].opt(), ident_sb[:])
                mm_cost(128)
                nc.vector.tensor_copy(
                    aoT_sb[:, p, qt * 128:(qt + 1) * 128], area)

            def do_cc(p):
                nc.sync.dma_start(cc_ins[p][:], aoT_sb[:, p, :])
                if not no_cc:
                    nc.gpsimd.collective_compute(
                        "AllGather",
                        mybir.AluOpType.bypass,
                        replica_groups=[[0, 1], [2, 3], [4, 5], [6, 7]],
                        ins=[cc_ins[p][:].opt()],
                        outs=[cc_outs[p][:].opt()],
                    )
                # qk slots p and 4+p are dead after attention p: receive the
                # gathered pair there (slot index == global f-tile index)
                if no_cc:
                    nc.sync.dma_start(qk_sb[:, p, :], cc_ins[p][:])
                    nc.sync.dma_start(qk_sb[:, PAIRS + p, :], cc_ins[p][:])
                else:
                    nc.sync.dma_start(qk_sb[:, p, :], cc_outs[p][0])
                    nc.sync.dma_start(qk_sb[:, PAIRS + p, :], cc_outs[p][1])
                cc_issued.add(p)

            def pop_post(curp=None):
                # keep >=3 of the CURRENT pair's transposes pending (their
                # divs need DVE time); older pairs' items drain immediately
                if not postq:
                    return False
                if len(postq) <= 3 and postq[0][1] == curp:
                    return False
                it = postq.popleft()
                if it[0] == "tr":
                    do_transpose(*it[1:])
                else:
                    do_cc(it[1])
                return True

            # AV issuance lags QK/exp by 2 k-tiles and carries across
            # q-chunk and pair boundaries: the scalar engine's exp stream
            # never waits for an epilogue
            pend_av = deque()

            def attention_pair(p):
                qT = qk_sb[:, p, :]
                kT = qk_sb[:, PAIRS + p, :]
                vbase = p * VW

                def do_qk(cqi, j):
                    qg0 = cqi * 4
                    jr = j - qg0
                    q0 = max(jr, 0) * 128
                    sq_sl = slice(cqi * QCH + q0, (cqi + 1) * QCH)
                    s2 = ps_s2.tile([128, 2, QCH], F32, tag="s2",
                                    name=f"s2_{p}_{cqi}_{j}")
                    nc.tensor.matmul(s2[:, 0, q0:QCH],
                                     lhsT=kT[0:64, j * 128:(j + 1) * 128],
                                     rhs=qT[0:64, sq_sl], start=True,
                                     stop=(jr < 0),
                                     skip_group_check=True)
                    mm_cost(QCH - q0)
                    nc.tensor.matmul(s2[:, 1, q0:QCH],
                                     lhsT=kT[64:128, j * 128:(j + 1) * 128],
                                     rhs=qT[64:128, sq_sl], start=True,
                                     stop=(jr < 0),
                                     skip_group_check=True)
                    mm_cost(QCH - q0)
                    if jr >= 0:
                        # causal bias on the diagonal block (per head --
                        # a matmul output must stay inside one psum bank)
                        for h in range(2):
                            nc.tensor.matmul(
                                s2[:, h, q0:q0 + 128], lhsT=mneg_sb[:],
                                rhs=ident2_sb[:, 0:128],
                                start=False, stop=(h == 1),
                                skip_group_check=True)
                            mm_cost(128)
                    pt = ptp.tile([128, 2, QCH], BF16, tag="pt")
                    nc.scalar.activation(pt[:, :, q0:QCH], s2[:, :, q0:QCH],
                                         mybir.ActivationFunctionType.Exp,
                                         scale=0.125)
                    act_cost(2 * (QCH - q0))
                    return pt

                for cqi in range(NQC):
                    qg0 = cqi * 4          # first global qtile of this chunk
                    kmax = qg0 + 4
                    y_t = [ps_y.tile([128, 2 * VW], F32, tag="y",
                                     name=f"yp{p}_{cqi}_{h}")
                           for h in range(2)]

                    def finalize(qt, yt, base):
                        # dens at cols base+64 and base+129
                        recip2 = work.tile([128, 2], F32, tag="rc",
                                           name=f"rc{p}_{qt}")
                        den_ap = _reap(yt[:, base + 64], [(HEAD_DIM + 1, 2)])
                        nc.vector.reciprocal_approx_fast(recip2[:], den_ap)
                        ao = aop.tile([128, 2, 64], BF16, tag="ao",
                                      name=f"ao{p}_{qt}")
                        for h in range(2):
                            nc.vector.tensor_scalar_mul(
                                ao[:, h, :],
                                _reap(yt[:, base + h * (HEAD_DIM + 1)], [(1, 64)]),
                                recip2[:, h:h + 1])
                        postq.append(("tr", p, ao, qt))

                    def issue_av(pt, j, qg0=qg0, y_t=y_t):
                        # default args bind THIS q-chunk's state: entries are
                        # popped after the loop variables have moved on
                        jr = j - qg0
                        for qtl in range(max(jr, 0), 4):
                            qt = qg0 + qtl
                            yt = y_t[qtl // 2]
                            base = (qtl % 2) * VW
                            for h in range(2):
                                o = h * (HEAD_DIM + 1)
                                # psum 'start' poisons the whole zero-region
                                # (bank): only the FIRST matmul touching each
                                # y tile may set it; the lazy zero-on-write
                                # initializes the other head/qt sub-groups
                                nc.tensor.matmul(
                                    yt[:, base + o: base + o + HEAD_DIM + 1],
                                    lhsT=pt[:, h, qtl * 128:(qtl + 1) * 128],
                                    rhs=v2_sb[:, j, vbase + o: vbase + o + HEAD_DIM + 1],
                                    start=(j == 0 and h == 0 and qtl % 2 == 0),
                                    stop=(j == qt),
                                    skip_group_check=True,
                                )
                                mm_cost(HEAD_DIM + 1)
                            if j == qt:
                                finalize(qt, yt, base)

                    for j in range(kmax):
                        pt = do_qk(cqi, j)
                        pend_av.append((issue_av, pt, j))
                        # one deferred transpose/cc per iteration
                        pop_post(p)
                        # fillers go BEFORE the lagged AV so the in-order PE
                        # chews them while exp(j-2) finishes
                        fill()
                        if len(pend_av) > 2:
                            fn, pt_, j_ = pend_av.popleft()
                            fn(pt_, j_)
                    # q-chunk epilogue: the accumulator ring recycles at the
                    # next y_t alloc, so all its AVs must be issued NOW (the
                    # transposes/cc stay deferred -- they use fresh scratch)
                    while pend_av:
                        fn, pt_, j_ = pend_av.popleft()
                        fn(pt_, j_)

            # ================= program =================
            # pre-phase: ALL qkv+norm+rope (their ACT/DVE tails hide under
            # the PE-dense mains), then v-proj tiles 0-3.  v tiles 4-15 and
            # the staged projection are the attention fillers: pure PE work
            # with no cross-engine chains to tangle with the attention loop.
            for pn in range(PAIRS):
                for _ in qkv_pair_gen(pn):
                    pass
            while ropeq:    # last pair's rope ops
                ropeq.popleft()()
            for tt in range(6):
                for _ in v_unit(tt):
                    pass

            def vdef_gen():
                for tt in range(6, NTT):
                    yield from v_unit(tt)

            fillers.append(["vdef", vdef_gen(), False])
            fillers.append(["projA", proj_stage_gen([0, PAIRS], True), False])
            fillers.append(["projB", proj_stage_gen([1, PAIRS + 1], False), False])
            fillers.append(["projC", proj_stage_gen([2, PAIRS + 2], False), False])
            gate_from.update({"projA": 0, "projB": 1, "projC": 2})

            for p in range(PAIRS):
                cur_pair[0] = p
                led["pe"] = led["act"] = 0.0
                attention_pair(p)
                # pair's transposes are already queued to postq (its last
                # q-chunk flushed pend_av), so FIFO keeps cc after them
                postq.append(("cc", p))
                if p == PAIRS - 2:
                    flush("vdef")  # safety: wv/xT die with the early pool
                    early_ctx.close()

            cur_pair[0] = PAIRS
            while postq:
                pop_post(None)
            for ent in fillers:
                flush(ent[0])

            # ---- projection tail: last pair's f-tiles + combine ----
            # psum cycles all three pools (6 slots): the adds/DMAs pipeline
            for tt in range(NTT):
                sel = tt % 3
                if sel == 0:
                    pss = ps_mm.tile([128, FL], F32, tag="mm", name=f"pf{tt}")
                elif sel == 1:
                    pss = ps_s2.tile([128, 2, QCH], F32, tag="s2",
                                     name=f"pf{tt}")[:, 0, :]
                else:
                    pss = ps_y.tile([128, FL], F32, tag="y", name=f"pf{tt}")
                for i, kf in enumerate([PAIRS - 1, 2 * PAIRS - 1]):
                    nc.tensor.matmul(
                        pss,
                        lhsT=qk_sb[:, kf, tt * 128:(tt + 1) * 128],
                        rhs=wp_sb[:, kf, :],
                        start=(i == 0), stop=(i == 1),
                    )
                    mm_cost(FL)
                ysb = evw.tile([128, FL], F32, tag="ev")
                nc.vector.tensor_add(ysb[:], pss, ppart[:, tt, :])
                nc.sync.dma_start(y_d[tt * 128:(tt + 1) * 128, :], ysb[:])

    nc.compile()
    return nc


def _prep_core_inputs(x, Wqkv, Wproj, q_norm_w, k_norm_w, core):
    b, g = core // 2, core % 2
    bf = ml_dtypes.bfloat16
    xT = np.ascontiguousarray(x[b].T).astype(bf)
    cols = slice(g * FL, (g + 1) * FL)
    wq = Wqkv[:, 0:C][:, cols] * np.tile(q_norm_w, H_LOCAL)[None, :]
    wk = Wqkv[:, C:2 * C][:, cols] * np.tile(k_norm_w, H_LOCAL)[None, :]
    wv = Wqkv[:, 2 * C:3 * C][:, cols]
    wp = Wproj[:, cols]
    return {
        "xT": xT,
        "Wq": np.ascontiguousarray(wq).astype(bf),
        "Wk": np.ascontiguousarray(wk).astype(bf),
        "Wv": np.ascontiguousarray(wv).astype(bf),
        "Wp": np.ascontiguousarray(wp).astype(bf),
    }


def kernel(x, Wqkv, Wproj, q_norm_w, k_norm_w):
    if "nc" not in _cached:
        _cached["nc"] = build_program()
    nc = _cached["nc"]

    x = np.asarray(x, dtype=np.float32)
    Wqkv = np.asarray(Wqkv, dtype=np.float32)
    Wproj = np.asarray(Wproj, dtype=np.float32)
    q_norm_w = np.asarray(q_norm_w, dtype=np.float32)
    k_norm_w = np.asarray(k_norm_w, dtype=np.float32)

    in_maps = [
        _prep_core_inputs(x, Wqkv, Wproj, q_norm_w, k_norm_w, c) for c in range(8)
    ]
    res = run_bass_kernel_spmd(nc, in_maps, list(range(8)))
    outs = res.results

    y = np.empty((B, T, C), dtype=np.float32)
    for b in range(B):
        y[b, :, 0:FL] = outs[2 * b]["y"]
        y[b, :, FL:C] = outs[2 * b + 1]["y"]
    return y


# revision 79
# speedup vs baseline: 1.0385x; 1.0095x over previous
"""Causal self-attention (QK-RMSNorm + RoPE) Trainium2 kernel.

Sharding: 8 cores = 4 batches x 2 head-groups (Megatron-style over heads).
Core c handles batch b=c//2, heads [g*8, g*8+8) with g=c%2.
Each core computes y[b, :, g*512:(g+1)*512] (output-column sharding of the
projection after a pairwise AllGather of attention outputs), so the host
only concatenates slices - no host-side arithmetic.

Perf notes (cost model charges out-free-size rows per matmul, independent of
contraction depth and output-partition count):
- AV is computed in the [q-tokens(part), head-dims(free)] orientation with a
  ones column appended to V per head: the 65-wide moving tensor makes AV cost
  65 rows/tile instead of 128-512, and the softmax denominator accumulates
  for free in column 64.  The division is then a per-partition scalar
  multiply (DVE), and the output is transposed back to [dims, tok] with
  cheap PE transposes (128 rows each) for the AllGather + projection.
- The per-head sum-of-squares for QK-RMSNorm uses one block-diagonal-ones
  matmul covering both packed heads; squares are computed on DVE in bf16
  from a Pool-engine drain of the qkv psum (keeps the scalar engine free
  for the attention exp()s, which are its binding load).
- The in-order PE is kept saturated (and in max p-state) by interleaving
  filler matmuls into the attention loop, driven by a PE-vs-ACT issued-work
  ledger: qkv of the next pair, deferred v-proj tiles, and the partial
  projection of already-gathered f-tiles (staged per AllGather arrival,
  accumulated into a bf16 partial on the Pool engine).  Only the last
  pair's two f-tiles + a DVE add remain after the final AllGather.
- Transpose scratch lives in the s2 (scores) PSUM ring, so the AV
  accumulator ring is released by the division and never serializes
  consecutive q-chunks.
"""


import numpy as np
import ml_dtypes
from collections import deque
from contextlib import ExitStack

import concourse.bass as bass
import concourse.bacc as bacc

# Force all activations into the one table set that covers Exp+Ln+Square+
# Copy+Identity, so the whole kernel needs exactly one ACT_TABLE_LOAD.
import concourse.hw_specs as _hw_specs
_orig_gat = _hw_specs.get_activation_tables

def _gat_one_set(arch):
    t = _orig_gat(arch)
    return {k: (v if k == "natural_log_exp_and_others" else set())
            for k, v in t.items()}

bacc.get_activation_tables = _gat_one_set
import concourse.mybir as mybir
import concourse.tile as tile
from concourse.bass_utils import run_bass_kernel_spmd

BF16 = mybir.dt.bfloat16
F32 = mybir.dt.float32

N_HEAD = 16
HEAD_DIM = 64
EPS = 1e-5
ROPE_BASE = 10000.0

B, T, C = 4, 2048, 1024
H_LOCAL = N_HEAD // 2          # heads per core
PAIRS = H_LOCAL // 2           # head-pairs per core (processed 2-at-a-time)
CT = C // 128                  # contraction tiles over C
FL = H_LOCAL * HEAD_DIM        # local feature width (512)
QCH = 512                      # q-chunk width
NQC = T // QCH                 # q-chunks
NKT = T // 128                 # k tiles
NTT = T // 128                 # token tiles
VW = 2 * (HEAD_DIM + 1)        # per-pair v2 width: [A dims|onesA|B dims|onesB]

_cached = {}


def _reap(ap, dims):
    """Rebuild an AP keeping tensor/offset/partition dim, with free dims
    `dims` given as (stride, size) pairs."""
    return bass.AP(tensor=ap.tensor, offset=ap.offset,
                   ap=[ap.ap[0]] + [list(d) for d in dims])


def _fbcast2(ap):
    """[128, N] AP -> [128, 2, N] with the middle (free) dim broadcast."""
    return bass.AP(
        tensor=ap.tensor, offset=ap.offset, ap=[ap.ap[0], [0, 2], ap.ap[1]]
    )


def _rope_tables():
    inv_freq = 1.0 / (ROPE_BASE ** (np.arange(0, HEAD_DIM, 2, dtype=np.float64) / HEAD_DIM))
    t = np.arange(T, dtype=np.float64)
    freqs = np.outer(t, inv_freq)                       # [T, 32]
    emb = np.concatenate([freqs, freqs], -1)            # [T, 64]
    cos = np.cos(emb).astype(np.float32).T              # [64, T]
    sin = np.sin(emb).astype(np.float32).T              # [64, T]
    cos2 = np.concatenate([cos, cos], 0)                # [128, T] two heads
    sin_s = sin.copy()
    sin_s[0:32] = -sin_s[0:32]                          # rotate-half sign
    sin2 = np.concatenate([sin_s, sin_s], 0)            # [128, T]
    return cos2.astype(ml_dtypes.bfloat16), sin2.astype(ml_dtypes.bfloat16)


def _diag_masks():
    # corner mask: keep where k_partition <= q_col (lower-triangular 128x128)
    p = np.arange(128)[:, None]
    qf = np.arange(128)[None, :]
    m = (p <= qf).astype(np.float32)
    return m.astype(ml_dtypes.bfloat16)                 # [128, 128]


def build_program(no_cc=False):
    nc = bacc.Bacc("TRN2", target_bir_lowering=False, debug=False,
                   num_devices=1 if no_cc else 8)

    xT_d = nc.dram_tensor("xT", [C, T], BF16, kind="ExternalInput")
    wq_d = nc.dram_tensor("Wq", [C, FL], BF16, kind="ExternalInput")
    wk_d = nc.dram_tensor("Wk", [C, FL], BF16, kind="ExternalInput")
    wv_d = nc.dram_tensor("Wv", [C, FL], BF16, kind="ExternalInput")
    wp_d = nc.dram_tensor("Wp", [C, FL], BF16, kind="ExternalInput")
    y_d = nc.dram_tensor("y", [T, FL], F32, kind="ExternalOutput")

    cos2_np, sin2_np = _rope_tables()
    cos_d = nc.inline_tensor(np.ascontiguousarray(cos2_np), "cos2")
    sin_d = nc.inline_tensor(np.ascontiguousarray(sin2_np), "sin2")
    # causal mask as a score bias: out[p,g,f] += mneg[f,p] = -30000*(p>f),
    # added to the diagonal 128x128 block by one PE matmul (keeps the
    # exp->AV chain off the vector engine)
    mneg_np = -30000.0 * (np.arange(128)[None, :] > np.arange(128)[:, None])
    mneg_d = nc.inline_tensor(
        np.ascontiguousarray(mneg_np.astype(ml_dtypes.bfloat16)), "mneg")
    id2_np = np.tile(np.eye(128, dtype=ml_dtypes.bfloat16), (1, 2))
    ident2_d = nc.inline_tensor(np.ascontiguousarray(id2_np), "ident2")
    bd_np = np.zeros((128, 128), dtype=ml_dtypes.bfloat16)
    bd_np[0:64, 0:64] = 1.0
    bd_np[64:128, 64:128] = 1.0
    onesbd_d = nc.inline_tensor(np.ascontiguousarray(bd_np), "onesbd")
    ident_d = nc.inline_tensor(
        np.ascontiguousarray(np.eye(128, dtype=ml_dtypes.bfloat16)), "ident")

    # per-pair exchange buffers
    cc_ins = [nc.dram_tensor(f"cc_in{p}", [128, T], BF16) for p in range(PAIRS)]
    cc_outs = [nc.dram_tensor(f"cc_out{p}", [2, 128, T], BF16) for p in range(PAIRS)]

    # --- PE-vs-ACT issued-work ledger (units: bf16 matmul rows = 0.4167ns) ---
    led = {"pe": 0.0, "act": 0.0}

    def mm_cost(rows):
        led["pe"] += rows

    def act_cost(free):
        led["act"] += 2.0 * free + 444.0

    with tile.TileContext(nc) as tc:
        with (
            tc.tile_pool(name="const", bufs=1) as const,
            tc.tile_pool(name="work", bufs=2) as work,
            tc.tile_pool(name="evw", bufs=6) as evw,
            tc.tile_pool(name="rope", bufs=2) as ropep,
            tc.tile_pool(name="pt", bufs=7) as ptp,
            tc.tile_pool(name="qraw", bufs=3) as qrawp,
            tc.tile_pool(name="ao", bufs=6) as aop,
            tc.tile_pool(name="ps_s2", bufs=2, space="PSUM") as ps_s2,
            tc.tile_pool(name="ps_y", bufs=2, space="PSUM") as ps_y,
            tc.tile_pool(name="ps_mm", bufs=2, space="PSUM") as ps_mm,
        ):
            early_ctx = ExitStack()
            early = early_ctx.enter_context(tc.tile_pool(name="early", bufs=1))

            # ---- inputs: wq/wk/xT first (qkv(0) consumes them first) ----
            wq_sb = early.tile([128, CT, FL], BF16)
            wk_sb = early.tile([128, CT, FL], BF16)
            xT_sb = early.tile([128, CT, T], BF16)
            # few, large DMAs: the HWDGE serializes ~625ns per DMA instruction.
            # xT halved with first halves first: qkv chunk 0/1 start earlier.
            nc.sync.dma_start(wq_sb[:], wq_d[:].rearrange("(k p) f -> p k f", p=128))
            for k in range(CT):
                nc.sync.dma_start(xT_sb[:, k, 0:T // 2],
                                  xT_d[k * 128:(k + 1) * 128, 0:T // 2])
            # wk after xT half-0: the primed q-mtile only needs wq + xT
            nc.sync.dma_start(wk_sb[:], wk_d[:].rearrange("(k p) f -> p k f", p=128))
            for k in range(CT):
                nc.sync.dma_start(xT_sb[:, k, T // 2:T],
                                  xT_d[k * 128:(k + 1) * 128, T // 2:T])
            wv_sb = early.tile([128, CT, FL], BF16)
            nc.sync.dma_start(wv_sb[:], wv_d[:].rearrange("(k p) f -> p k f", p=128))
            cos_sb = early.tile([128, T], BF16)
            nc.sync.dma_start(cos_sb[:], cos_d[:])
            sin_sb = early.tile([128, T], BF16)
            nc.sync.dma_start(sin_sb[:], sin_d[:])
            mneg_sb = const.tile([128, 128], BF16)
            nc.sync.dma_start(mneg_sb[:], mneg_d[:])
            ident2_sb = const.tile([128, 256], BF16)
            nc.sync.dma_start(ident2_sb[:], ident2_d[:])
            ident_sb = const.tile([128, 128], BF16)
            nc.sync.dma_start(ident_sb[:], ident_d[:])
            wp_sb = const.tile([128, CT, FL], BF16)
            nc.sync.dma_start(wp_sb[:], wp_d[:].rearrange("(k p) f -> p k f", p=128))

            # block-diagonal ones: one matmul sums squares of both packed heads
            ones_bd = const.tile([128, 128], BF16)
            nc.sync.dma_start(ones_bd[:], onesbd_d[:])

            qk_sb = const.tile([128, 2 * PAIRS, T], BF16)
            # v2: per tt, per pair: [A dims(64) | onesA | B dims(64) | onesB]
            v2_sb = const.tile([128, NTT, PAIRS * VW], BF16)
            aoT_sb = const.tile([128, PAIRS, T], BF16)
            # bf16 partial of the projection over already-gathered f-tiles
            ppart = const.tile([128, NTT, FL], BF16)

            # ones columns of v2 (once, before any v-proj writes)
            pstr = v2_sb.ap[0][0]
            ones_ap = bass.AP(
                tensor=v2_sb.tensor, offset=v2_sb.offset + HEAD_DIM,
                ap=[[pstr, 128], [PAIRS * VW, NTT], [VW, PAIRS], [HEAD_DIM + 1, 2]])
            nc.vector.memset(ones_ap, 1.0)

            # ---------------- unit generators (PE fillers) ----------------
            def v_unit(tt):
                pss = ps_mm.tile([128, FL], F32, tag="mm", name=f"vp{tt}")
                for k0 in range(0, CT, 2):
                    for k in (k0, k0 + 1):
                        nc.tensor.matmul(
                            pss,
                            lhsT=xT_sb[:, k, tt * 128:(tt + 1) * 128],
                            rhs=wv_sb[:, k, :],
                            start=(k == 0), stop=(k == CT - 1),
                        )
                        mm_cost(FL)
                    yield
                # drain into v2 layout: [128,4,2,64] both sides (DVE --
                # GPSIMD cannot touch PSUM on real hardware)
                src = _reap(pss, [(128, PAIRS), (64, 2), (1, 64)])
                dst = bass.AP(tensor=v2_sb.tensor,
                              offset=v2_sb.offset + tt * PAIRS * VW,
                              ap=[[pstr, 128], [VW, PAIRS], [HEAD_DIM + 1, 2], [1, 64]])
                nc.vector.tensor_copy(dst, src)
                yield

            def qkv_tail(m, pss, sq, n):
                ss = ps_y.tile([128, QCH], F32, tag="y", name=f"ss{m}_{n}")
                nc.tensor.matmul(ss, lhsT=ones_bd[:], rhs=sq[:],
                                 start=True, stop=True, skip_group_check=True)
                mm_cost(QCH)
                # rstd = (ss/64)^(-1/2) = exp(-0.5*ln(ss/64)); eps negligible.
                rr = work.tile([128, QCH], BF16, tag="rr")
                nc.scalar.activation(rr[:], ss,
                                     mybir.ActivationFunctionType.Ln,
                                     scale=1.0 / HEAD_DIM)
                act_cost(QCH)
                rstd = work.tile([128, QCH], BF16, tag="rs")
                nc.scalar.activation(rstd[:], rr[:],
                                     mybir.ActivationFunctionType.Exp,
                                     scale=-0.5)
                act_cost(QCH)
                nc.vector.tensor_mul(qk_sb[:, m, n * QCH:(n + 1) * QCH],
                                     pss, rstd[:])

            # lazy rope: one closure per DVE op, drained one per qkv yield so
            # the qkv norm-muls (psum release) never queue behind a burst
            ropeq = deque()

            def rope_mtile(m):
                src = qk_sb[:, m, :]
                sw = ropep.tile([128, T], BF16, tag="rp")
                for off in (0, 64):
                    ropeq.append(lambda o=off: nc.vector.tensor_copy(
                        sw[o:o + 32, :], src[o + 32:o + 64, :]))
                    ropeq.append(lambda o=off: nc.vector.tensor_copy(
                        sw[o + 32:o + 64, :], src[o:o + 32, :]))
                ropeq.append(lambda: nc.vector.tensor_mul(sw[:], sw[:], sin_sb[:]))
                ropeq.append(lambda: nc.vector.tensor_mul(src, src, cos_sb[:]))
                ropeq.append(lambda: nc.vector.tensor_add(src, src, sw[:]))

            def qkv_unit(m):
                # pss borrows the (pre-phase-idle) s2 slots, ss the y slots:
                # the qkv pipeline gets baseline's 4-bank depth while the
                # attention pools keep their static reservation.
                # private 2-deep pss ring per generator (q-mtiles borrow the
                # pre-phase-idle s2 slots, k-mtiles the mm slots): a slot is
                # reused only 2 chunks later, far past its tail chain
                w_sb = wq_sb if m < PAIRS else wk_sb
                mloc = (m % PAIRS) * 128
                pend = None
                ps2 = None
                for n in range(NQC):
                    if m < PAIRS:
                        # 4-deep ring: two chunks per 2-bank s2 slot
                        if n % 2 == 0:
                            ps2 = ps_s2.tile([128, 2, QCH], F32, tag="s2",
                                             name=f"qk{m}_{n}")
                        pss = ps2[:, n % 2, :]
                    else:
                        pss = ps_mm.tile([128, QCH], F32, tag="mm",
                                         name=f"qk{m}_{n}")
                    for k0 in range(0, CT, 2):
                        for k in (k0, k0 + 1):
                            nc.tensor.matmul(
                                pss,
                                lhsT=w_sb[:, k, mloc:mloc + 128],
                                rhs=xT_sb[:, k, n * QCH:(n + 1) * QCH],
                                start=(k == 0), stop=(k == CT - 1),
                                skip_group_check=True,
                            )
                            mm_cost(QCH)
                        yield
                    # square must be ACT: DVE cannot read two PSUM operands
                    sq = work.tile([128, QCH], BF16, tag="sq")
                    nc.scalar.activation(sq[:], pss,
                                         mybir.ActivationFunctionType.Square)
                    act_cost(QCH)
                    if pend is not None:
                        qkv_tail(m, *pend)
                        yield
                    pend = (pss, sq, n)
                qkv_tail(m, *pend)
                yield
                rope_mtile(m)

            def qkv_pair_gen(pn):
                # alternate q- and k-mtile steps (q primed 2 chunks ahead so
                # the two tails never bunch at the same boundary)
                a, b = qkv_unit(pn), qkv_unit(PAIRS + pn)
                for _ in range(8):
                    try:
                        next(a)
                    except StopIteration:
                        break
                while True:
                    na = nb = False
                    try:
                        next(a)
                    except StopIteration:
                        na = True
                    if ropeq:
                        ropeq.popleft()()
                    try:
                        next(b)
                    except StopIteration:
                        nb = True
                    if ropeq:
                        ropeq.popleft()()
                    if na and nb:
                        return
                    yield

            def proj_stage_gen(kfs, first_stage):
                for tt in range(NTT):
                    pss = ps_mm.tile([128, FL], F32, tag="mm",
                                     name=f"pj{kfs[0]}_{tt}")
                    for i, kf in enumerate(kfs):
                        nc.tensor.matmul(
                            pss,
                            lhsT=qk_sb[:, kf, tt * 128:(tt + 1) * 128],
                            rhs=wp_sb[:, kf, :],
                            start=(i == 0), stop=(i == len(kfs) - 1),
                        )
                        mm_cost(FL)
                        yield
                    pp = ppart[:, tt, :]
                    if first_stage:
                        nc.vector.tensor_copy(pp, pss)
                    else:
                        nc.vector.tensor_add(pp, pss, pp)
                    yield

            # ---------------- filler machinery ----------------
            # list of [label, generator, done]; fill_step picks the first
            # not-done generator whose gate is open (order = priority).
            fillers = []
            gate_from = {}   # label -> cc whose writeback must be issued
            cc_issued = set()
            cur_pair = [0]

            def fill_step():
                for ent in fillers:
                    if ent[2] or gate_from.get(ent[0], -1) not in cc_issued | {-1}:
                        continue
                    try:
                        next(ent[1])
                        return True
                    except StopIteration:
                        ent[2] = True
                        continue
                return False

            FILL_BIAS = 2000.0   # keep PE a bit ahead of ACT (rows-equiv)

            def fill():
                # pair 0: steady front-loaded drip of deferred v-proj tiles
                # (AV(j=tt) consumes v2 tile tt -- the drip beats deadlines)
                if cur_pair[0] == 0 and not fillers[0][2]:
                    for _ in range(2):
                        try:
                            next(fillers[0][1])
                        except StopIteration:
                            fillers[0][2] = True
                            break
                while led["pe"] < led["act"] + FILL_BIAS:
                    if not fill_step():
                        break

            def flush(label):
                for ent in fillers:
                    if ent[0] == label and not ent[2]:
                        for _ in ent[1]:
                            pass
                        ent[2] = True

            # ---------------- attention (flipped AV) ----------------
            # cross-pair queue of deferred work: ("tr", p, ao, qt) transposes
            # and ("cc", p) exchange launches drain one per j-iteration, so
            # cqi/pair epilogues never idle the scalar engine
            postq = deque()

            def do_transpose(p, ao, qt):
                # scratch from the mm ring: a transpose in the s2 ring would
                # halve the QK pipeline's lookahead (the scores ring is the
                # attention loop's binding resource)
                tr_t = ps_mm.tile([128, FL], F32, tag="mm",
                                  name=f"tr{p}_{qt}")
                area = tr_t[:, 0:64].bitcast(BF16)
                nc.tensor.transpose(area, ao[:].opt(), ident_sb[:])
                mm_cost(128)
                nc.vector.tensor_copy(
                    aoT_sb[:, p, qt * 128:(qt + 1) * 128], area)

            def do_cc(p):
                nc.sync.dma_start(cc_ins[p][:], aoT_sb[:, p, :])
                if not no_cc:
                    nc.gpsimd.collective_compute(
                        "AllGather",
                        mybir.AluOpType.bypass,
                        replica_groups=[[0, 1], [2, 3], [4, 5], [6, 7]],
                        ins=[cc_ins[p][:].opt()],
                        outs=[cc_outs[p][:].opt()],
                    )
                # qk slots p and 4+p are dead after attention p: receive the
                # gathered pair there (slot index == global f-tile index)
                if no_cc:
                    nc.sync.dma_start(qk_sb[:, p, :], cc_ins[p][:])
                    nc.sync.dma_start(qk_sb[:, PAIRS + p, :], cc_ins[p][:])
                else:
                    nc.sync.dma_start(qk_sb[:, p, :], cc_outs[p][0])
                    nc.sync.dma_start(qk_sb[:, PAIRS + p, :], cc_outs[p][1])
                cc_issued.add(p)

            def pop_post(curp=None):
                # keep >=3 of the CURRENT pair's transposes pending (their
                # divs need DVE time); older pairs' items drain immediately
                if not postq:
                    return False
                if len(postq) <= 3 and postq[0][1] == curp:
                    return False
                it = postq.popleft()
                if it[0] == "tr":
                    do_transpose(*it[1:])
                else:
                    do_cc(it[1])
                return True

            # AV issuance lags QK/exp by 2 k-tiles and carries across
            # q-chunk and pair boundaries: the scalar engine's exp stream
            # never waits for an epilogue
            pend_av = deque()

            def attention_pair(p):
                qT = qk_sb[:, p, :]
                kT = qk_sb[:, PAIRS + p, :]
                vbase = p * VW

                def do_qk(cqi, j):
                    qg0 = cqi * 4
                    jr = j - qg0
                    q0 = max(jr, 0) * 128
                    sq_sl = slice(cqi * QCH + q0, (cqi + 1) * QCH)
                    s2 = ps_s2.tile([128, 2, QCH], F32, tag="s2",
                                    name=f"s2_{p}_{cqi}_{j}")
                    nc.tensor.matmul(s2[:, 0, q0:QCH],
                                     lhsT=kT[0:64, j * 128:(j + 1) * 128],
                                     rhs=qT[0:64, sq_sl], start=True,
                                     stop=(jr < 0),
                                     skip_group_check=True)
                    mm_cost(QCH - q0)
                    nc.tensor.matmul(s2[:, 1, q0:QCH],
                                     lhsT=kT[64:128, j * 128:(j + 1) * 128],
                                     rhs=qT[64:128, sq_sl], start=True,
                                     stop=(jr < 0),
                                     skip_group_check=True)
                    mm_cost(QCH - q0)
                    if jr >= 0:
                        # causal bias on the diagonal block (per head --
                        # a matmul output must stay inside one psum bank)
                        for h in range(2):
                            nc.tensor.matmul(
                                s2[:, h, q0:q0 + 128], lhsT=mneg_sb[:],
                                rhs=ident2_sb[:, 0:128],
                                start=False, stop=(h == 1),
                                skip_group_check=True)
                            mm_cost(128)
                    pt = ptp.tile([128, 2, QCH], BF16, tag="pt")
                    nc.scalar.activation(pt[:, :, q0:QCH], s2[:, :, q0:QCH],
                                         mybir.ActivationFunctionType.Exp,
                                         scale=0.125)
                    act_cost(2 * (QCH - q0))
                    return pt

                for cqi in range(NQC):
                    qg0 = cqi * 4          # first global qtile of this chunk
                    kmax = qg0 + 4
                    y_t = [ps_y.tile([128, 2 * VW], F32, tag="y",
                                     name=f"yp{p}_{cqi}_{h}")
                           for h in range(2)]

                    def finalize(qt, yt, base):
                        # dens at cols base+64 and base+129
                        recip2 = work.tile([128, 2], F32, tag="rc",
                                           name=f"rc{p}_{qt}")
                        den_ap = _reap(yt[:, base + 64], [(HEAD_DIM + 1, 2)])
                        nc.vector.reciprocal_approx_fast(recip2[:], den_ap)
                        ao = aop.tile([128, 2, 64], BF16, tag="ao",
                                      name=f"ao{p}_{qt}")
                        for h in range(2):
                            nc.vector.tensor_scalar_mul(
                                ao[:, h, :],
                                _reap(yt[:, base + h * (HEAD_DIM + 1)], [(1, 64)]),
                                recip2[:, h:h + 1])
                        postq.append(("tr", p, ao, qt))

                    def issue_av(pt, j, qg0=qg0, y_t=y_t):
                        # default args bind THIS q-chunk's state: entries are
                        # popped after the loop variables have moved on
                        jr = j - qg0
                        for qtl in range(max(jr, 0), 4):
                            qt = qg0 + qtl
                            yt = y_t[qtl // 2]
                            base = (qtl % 2) * VW
                            for h in range(2):
                                o = h * (HEAD_DIM + 1)
                                # psum 'start' poisons the whole zero-region
                                # (bank): only the FIRST matmul touching each
                                # y tile may set it; the lazy zero-on-write
                                # initializes the other head/qt sub-groups
                                nc.tensor.matmul(
                                    yt[:, base + o: base + o + HEAD_DIM + 1],
                                    lhsT=pt[:, h, qtl * 128:(qtl + 1) * 128],
                                    rhs=v2_sb[:, j, vbase + o: vbase + o + HEAD_DIM + 1],
                                    start=(j == 0 and h == 0 and qtl % 2 == 0),
                                    stop=(j == qt),
                                    skip_group_check=True,
                                )
                                mm_cost(HEAD_DIM + 1)
                            if j == qt:
                                finalize(qt, yt, base)

                    for j in range(kmax):
                        pt = do_qk(cqi, j)
                        pend_av.append((issue_av, pt, j))
                        # one deferred transpose/cc per iteration
                        pop_post(p)
                        # fillers go BEFORE the lagged AV so the in-order PE
                        # chews them while exp(j-2) finishes
                        fill()
                        if len(pend_av) > 2:
                            fn, pt_, j_ = pend_av.popleft()
                            fn(pt_, j_)
                    # q-chunk epilogue: the accumulator ring recycles at the
                    # next y_t alloc, so all its AVs must be issued NOW (the
                    # transposes/cc stay deferred -- they use fresh scratch)
                    while pend_av:
                        fn, pt_, j_ = pend_av.popleft()
                        fn(pt_, j_)

            # ================= program =================
            # pre-phase: ALL qkv+norm+rope (their ACT/DVE tails hide under
            # the PE-dense mains), then v-proj tiles 0-3.  v tiles 4-15 and
            # the staged projection are the attention fillers: pure PE work
            # with no cross-engine chains to tangle with the attention loop.
            for pn in range(PAIRS):
                for _ in qkv_pair_gen(pn):
                    pass
            while ropeq:    # last pair's rope ops
                ropeq.popleft()()
            for tt in range(6):
                for _ in v_unit(tt):
                    pass

            def vdef_gen():
                for tt in range(6, NTT):
                    yield from v_unit(tt)

            fillers.append(["vdef", vdef_gen(), False])
            fillers.append(["projA", proj_stage_gen([0, PAIRS], True), False])
            fillers.append(["projB", proj_stage_gen([1, PAIRS + 1], False), False])
            fillers.append(["projC", proj_stage_gen([2, PAIRS + 2], False), False])
            gate_from.update({"projA": 0, "projB": 1, "projC": 2})

            for p in range(PAIRS):
                cur_pair[0] = p
                led["pe"] = led["act"] = 0.0
                attention_pair(p)
                # pair's transposes are already queued to postq (its last
                # q-chunk flushed pend_av), so FIFO keeps cc after them
                postq.append(("cc", p))
                if p == PAIRS - 2:
                    flush("vdef")  # safety: wv/xT die with the early pool
                    early_ctx.close()

            cur_pair[0] = PAIRS
            while postq:
                pop_post(None)
            for ent in fillers:
                flush(ent[0])

            # ---- projection tail: last pair's f-tiles + combine ----
            # psum cycles all three pools (6 slots): the adds/DMAs pipeline
            for tt in range(NTT):
                sel = tt % 3
                if sel == 0:
                    pss = ps_mm.tile([128, FL], F32, tag="mm", name=f"pf{tt}")
                elif sel == 1:
                    pss = ps_s2.tile([128, 2, QCH], F32, tag="s2",
                                     name=f"pf{tt}")[:, 0, :]
                else:
                    pss = ps_y.tile([128, FL], F32, tag="y", name=f"pf{tt}")
                for i, kf in enumerate([PAIRS - 1, 2 * PAIRS - 1]):
                    nc.tensor.matmul(
                        pss,
                        lhsT=qk_sb[:, kf, tt * 128:(tt + 1) * 128],
                        rhs=wp_sb[:, kf, :],
                        start=(i == 0), stop=(i == 1),
                    )
                    mm_cost(FL)
                ysb = evw.tile([128, FL], F32, tag="ev")
                nc.vector.tensor_add(ysb[:], pss, ppart[:, tt, :])
                nc.sync.dma_start(y_d[tt * 128:(tt + 1) * 128, :], ysb[:])

    nc.compile()
    return nc


def _prep_core_inputs(x, Wqkv, Wproj, q_norm_w, k_norm_w, core):
    b, g = core // 2, core % 2
    bf = ml_dtypes.bfloat16
    xT = np.ascontiguousarray(x[b].T).astype(bf)
    cols = slice(g * FL, (g + 1) * FL)
    wq = Wqkv[:, 0:C][:, cols] * np.tile(q_norm_w, H_LOCAL)[None, :]
    wk = Wqkv[:, C:2 * C][:, cols] * np.tile(k_norm_w, H_LOCAL)[None, :]
    wv = Wqkv[:, 2 * C:3 * C][:, cols]
    wp = Wproj[:, cols]
    return {
        "xT": xT,
        "Wq": np.ascontiguousarray(wq).astype(bf),
        "Wk": np.ascontiguousarray(wk).astype(bf),
        "Wv": np.ascontiguousarray(wv).astype(bf),
        "Wp": np.ascontiguousarray(wp).astype(bf),
    }


def kernel(x, Wqkv, Wproj, q_norm_w, k_norm_w):
    if "nc" not in _cached:
        _cached["nc"] = build_program()
    nc = _cached["nc"]

    x = np.asarray(x, dtype=np.float32)
    Wqkv = np.asarray(Wqkv, dtype=np.float32)
    Wproj = np.asarray(Wproj, dtype=np.float32)
    q_norm_w = np.asarray(q_norm_w, dtype=np.float32)
    k_norm_w = np.asarray(k_norm_w, dtype=np.float32)

    in_maps = [
        _prep_core_inputs(x, Wqkv, Wproj, q_norm_w, k_norm_w, c) for c in range(8)
    ]
    res = run_bass_kernel_spmd(nc, in_maps, list(range(8)))
    outs = res.results

    y = np.empty((B, T, C), dtype=np.float32)
    for b in range(B):
        y[b, :, 0:FL] = outs[2 * b]["y"]
        y[b, :, FL:C] = outs[2 * b + 1]["y"]
    return y


# revision 80
# speedup vs baseline: 1.0482x; 1.0094x over previous
"""Causal self-attention (QK-RMSNorm + RoPE) Trainium2 kernel.

Sharding: 8 cores = 4 batches x 2 head-groups (Megatron-style over heads).
Core c handles batch b=c//2, heads [g*8, g*8+8) with g=c%2.
Each core computes y[b, :, g*512:(g+1)*512] (output-column sharding of the
projection after a pairwise AllGather of attention outputs), so the host
only concatenates slices - no host-side arithmetic.

Perf notes (cost model charges out-free-size rows per matmul, independent of
contraction depth and output-partition count):
- AV is computed in the [q-tokens(part), head-dims(free)] orientation with a
  ones column appended to V per head: the 65-wide moving tensor makes AV cost
  65 rows/tile instead of 128-512, and the softmax denominator accumulates
  for free in column 64.  The division is then a per-partition scalar
  multiply (DVE), and the output is transposed back to [dims, tok] with
  cheap PE transposes (128 rows each) for the AllGather + projection.
- The per-head sum-of-squares for QK-RMSNorm uses one block-diagonal-ones
  matmul covering both packed heads; squares are computed on DVE in bf16
  from a Pool-engine drain of the qkv psum (keeps the scalar engine free
  for the attention exp()s, which are its binding load).
- The in-order PE is kept saturated (and in max p-state) by interleaving
  filler matmuls into the attention loop, driven by a PE-vs-ACT issued-work
  ledger: qkv of the next pair, deferred v-proj tiles, and the partial
  projection of already-gathered f-tiles (staged per AllGather arrival,
  accumulated into a bf16 partial on the Pool engine).  Only the last
  pair's two f-tiles + a DVE add remain after the final AllGather.
- Transpose scratch lives in the s2 (scores) PSUM ring, so the AV
  accumulator ring is released by the division and never serializes
  consecutive q-chunks.
"""


import numpy as np
import ml_dtypes
from collections import deque
from contextlib import ExitStack

import concourse.bass as bass
import concourse.bacc as bacc

# Force all activations into the one table set that covers Exp+Ln+Square+
# Copy+Identity, so the whole kernel needs exactly one ACT_TABLE_LOAD.
import concourse.hw_specs as _hw_specs
_orig_gat = _hw_specs.get_activation_tables

def _gat_one_set(arch):
    t = _orig_gat(arch)
    return {k: (v if k == "natural_log_exp_and_others" else set())
            for k, v in t.items()}

bacc.get_activation_tables = _gat_one_set
import concourse.mybir as mybir
import concourse.tile as tile
from concourse.bass_utils import run_bass_kernel_spmd

BF16 = mybir.dt.bfloat16
F32 = mybir.dt.float32

N_HEAD = 16
HEAD_DIM = 64
EPS = 1e-5
ROPE_BASE = 10000.0

B, T, C = 4, 2048, 1024
H_LOCAL = N_HEAD // 2          # heads per core
PAIRS = H_LOCAL // 2           # head-pairs per core (processed 2-at-a-time)
CT = C // 128                  # contraction tiles over C
FL = H_LOCAL * HEAD_DIM        # local feature width (512)
QCH = 512                      # q-chunk width
NQC = T // QCH                 # q-chunks
NKT = T // 128                 # k tiles
NTT = T // 128                 # token tiles
VW = 2 * (HEAD_DIM + 1)        # per-pair v2 width: [A dims|onesA|B dims|onesB]

_cached = {}


def _reap(ap, dims):
    """Rebuild an AP keeping tensor/offset/partition dim, with free dims
    `dims` given as (stride, size) pairs."""
    return bass.AP(tensor=ap.tensor, offset=ap.offset,
                   ap=[ap.ap[0]] + [list(d) for d in dims])


def _fbcast2(ap):
    """[128, N] AP -> [128, 2, N] with the middle (free) dim broadcast."""
    return bass.AP(
        tensor=ap.tensor, offset=ap.offset, ap=[ap.ap[0], [0, 2], ap.ap[1]]
    )


def _rope_tables():
    inv_freq = 1.0 / (ROPE_BASE ** (np.arange(0, HEAD_DIM, 2, dtype=np.float64) / HEAD_DIM))
    t = np.arange(T, dtype=np.float64)
    freqs = np.outer(t, inv_freq)                       # [T, 32]
    emb = np.concatenate([freqs, freqs], -1)            # [T, 64]
    cos = np.cos(emb).astype(np.float32).T              # [64, T]
    sin = np.sin(emb).astype(np.float32).T              # [64, T]
    cos2 = np.concatenate([cos, cos], 0)                # [128, T] two heads
    sin_s = sin.copy()
    sin_s[0:32] = -sin_s[0:32]                          # rotate-half sign
    sin2 = np.concatenate([sin_s, sin_s], 0)            # [128, T]
    return cos2.astype(ml_dtypes.bfloat16), sin2.astype(ml_dtypes.bfloat16)


def _diag_masks():
    # corner mask: keep where k_partition <= q_col (lower-triangular 128x128)
    p = np.arange(128)[:, None]
    qf = np.arange(128)[None, :]
    m = (p <= qf).astype(np.float32)
    return m.astype(ml_dtypes.bfloat16)                 # [128, 128]


def build_program(no_cc=False):
    nc = bacc.Bacc("TRN2", target_bir_lowering=False, debug=False,
                   num_devices=1 if no_cc else 8)

    xT_d = nc.dram_tensor("xT", [C, T], BF16, kind="ExternalInput")
    wq_d = nc.dram_tensor("Wq", [C, FL], BF16, kind="ExternalInput")
    wk_d = nc.dram_tensor("Wk", [C, FL], BF16, kind="ExternalInput")
    wv_d = nc.dram_tensor("Wv", [C, FL], BF16, kind="ExternalInput")
    wp_d = nc.dram_tensor("Wp", [C, FL], BF16, kind="ExternalInput")
    y_d = nc.dram_tensor("y", [T, FL], F32, kind="ExternalOutput")

    cos2_np, sin2_np = _rope_tables()
    cos_d = nc.inline_tensor(np.ascontiguousarray(cos2_np), "cos2")
    sin_d = nc.inline_tensor(np.ascontiguousarray(sin2_np), "sin2")
    # causal mask as a score bias: out[p,g,f] += mneg[f,p] = -30000*(p>f),
    # added to the diagonal 128x128 block by one PE matmul (keeps the
    # exp->AV chain off the vector engine)
    mneg_np = -30000.0 * (np.arange(128)[None, :] > np.arange(128)[:, None])
    mneg_d = nc.inline_tensor(
        np.ascontiguousarray(mneg_np.astype(ml_dtypes.bfloat16)), "mneg")
    id2_np = np.tile(np.eye(128, dtype=ml_dtypes.bfloat16), (1, 2))
    ident2_d = nc.inline_tensor(np.ascontiguousarray(id2_np), "ident2")
    bd_np = np.zeros((128, 128), dtype=ml_dtypes.bfloat16)
    bd_np[0:64, 0:64] = 1.0
    bd_np[64:128, 64:128] = 1.0
    onesbd_d = nc.inline_tensor(np.ascontiguousarray(bd_np), "onesbd")
    ident_d = nc.inline_tensor(
        np.ascontiguousarray(np.eye(128, dtype=ml_dtypes.bfloat16)), "ident")

    # per-pair exchange buffers
    cc_ins = [nc.dram_tensor(f"cc_in{p}", [128, T], BF16) for p in range(PAIRS)]
    cc_outs = [nc.dram_tensor(f"cc_out{p}", [2, 128, T], BF16) for p in range(PAIRS)]

    # --- PE-vs-ACT issued-work ledger (units: bf16 matmul rows = 0.4167ns) ---
    led = {"pe": 0.0, "act": 0.0}

    def mm_cost(rows):
        led["pe"] += rows

    def act_cost(free):
        led["act"] += 2.0 * free + 444.0

    with tile.TileContext(nc) as tc:
        with (
            tc.tile_pool(name="const", bufs=1) as const,
            tc.tile_pool(name="work", bufs=2) as work,
            tc.tile_pool(name="evw", bufs=6) as evw,
            tc.tile_pool(name="rope", bufs=2) as ropep,
            tc.tile_pool(name="pt", bufs=8) as ptp,
            tc.tile_pool(name="qraw", bufs=3) as qrawp,
            tc.tile_pool(name="ao", bufs=6) as aop,
            tc.tile_pool(name="ps_s2", bufs=2, space="PSUM") as ps_s2,
            tc.tile_pool(name="ps_y", bufs=2, space="PSUM") as ps_y,
            tc.tile_pool(name="ps_mm", bufs=2, space="PSUM") as ps_mm,
        ):
            early_ctx = ExitStack()
            early = early_ctx.enter_context(tc.tile_pool(name="early", bufs=1))

            # ---- inputs: wq/wk/xT first (qkv(0) consumes them first) ----
            wq_sb = early.tile([128, CT, FL], BF16)
            wk_sb = early.tile([128, CT, FL], BF16)
            xT_sb = early.tile([128, CT, T], BF16)
            # few, large DMAs: the HWDGE serializes ~625ns per DMA instruction.
            # xT halved with first halves first: qkv chunk 0/1 start earlier.
            nc.sync.dma_start(wq_sb[:], wq_d[:].rearrange("(k p) f -> p k f", p=128))
            for k in range(CT):
                nc.sync.dma_start(xT_sb[:, k, 0:T // 2],
                                  xT_d[k * 128:(k + 1) * 128, 0:T // 2])
            # wk after xT half-0: the primed q-mtile only needs wq + xT
            nc.sync.dma_start(wk_sb[:], wk_d[:].rearrange("(k p) f -> p k f", p=128))
            for k in range(CT):
                nc.sync.dma_start(xT_sb[:, k, T // 2:T],
                                  xT_d[k * 128:(k + 1) * 128, T // 2:T])
            wv_sb = early.tile([128, CT, FL], BF16)
            nc.sync.dma_start(wv_sb[:], wv_d[:].rearrange("(k p) f -> p k f", p=128))
            cos_sb = early.tile([128, T], BF16)
            nc.sync.dma_start(cos_sb[:], cos_d[:])
            sin_sb = early.tile([128, T], BF16)
            nc.sync.dma_start(sin_sb[:], sin_d[:])
            mneg_sb = const.tile([128, 128], BF16)
            nc.sync.dma_start(mneg_sb[:], mneg_d[:])
            ident2_sb = const.tile([128, 256], BF16)
            nc.sync.dma_start(ident2_sb[:], ident2_d[:])
            ident_sb = const.tile([128, 128], BF16)
            nc.sync.dma_start(ident_sb[:], ident_d[:])
            wp_sb = const.tile([128, CT, FL], BF16)
            nc.sync.dma_start(wp_sb[:], wp_d[:].rearrange("(k p) f -> p k f", p=128))

            # block-diagonal ones: one matmul sums squares of both packed heads
            ones_bd = const.tile([128, 128], BF16)
            nc.sync.dma_start(ones_bd[:], onesbd_d[:])

            qk_sb = const.tile([128, 2 * PAIRS, T], BF16)
            # v2: per tt, per pair: [A dims(64) | onesA | B dims(64) | onesB]
            v2_sb = const.tile([128, NTT, PAIRS * VW], BF16)
            aoT_sb = const.tile([128, PAIRS, T], BF16)
            # bf16 partial of the projection over already-gathered f-tiles
            ppart = const.tile([128, NTT, FL], BF16)

            # ones columns of v2 (once, before any v-proj writes)
            pstr = v2_sb.ap[0][0]
            ones_ap = bass.AP(
                tensor=v2_sb.tensor, offset=v2_sb.offset + HEAD_DIM,
                ap=[[pstr, 128], [PAIRS * VW, NTT], [VW, PAIRS], [HEAD_DIM + 1, 2]])
            nc.vector.memset(ones_ap, 1.0)

            # ---------------- unit generators (PE fillers) ----------------
            def v_unit(tt):
                pss = ps_mm.tile([128, FL], F32, tag="mm", name=f"vp{tt}")
                for k0 in range(0, CT, 2):
                    for k in (k0, k0 + 1):
                        nc.tensor.matmul(
                            pss,
                            lhsT=xT_sb[:, k, tt * 128:(tt + 1) * 128],
                            rhs=wv_sb[:, k, :],
                            start=(k == 0), stop=(k == CT - 1),
                        )
                        mm_cost(FL)
                    yield
                # drain into v2 layout: [128,4,2,64] both sides (DVE --
                # GPSIMD cannot touch PSUM on real hardware)
                src = _reap(pss, [(128, PAIRS), (64, 2), (1, 64)])
                dst = bass.AP(tensor=v2_sb.tensor,
                              offset=v2_sb.offset + tt * PAIRS * VW,
                              ap=[[pstr, 128], [VW, PAIRS], [HEAD_DIM + 1, 2], [1, 64]])
                nc.vector.tensor_copy(dst, src)
                yield

            def qkv_tail(m, pss, sq, n):
                ss = ps_y.tile([128, QCH], F32, tag="y", name=f"ss{m}_{n}")
                nc.tensor.matmul(ss, lhsT=ones_bd[:], rhs=sq[:],
                                 start=True, stop=True, skip_group_check=True)
                mm_cost(QCH)
                # rstd = (ss/64)^(-1/2) = exp(-0.5*ln(ss/64)); eps negligible.
                rr = work.tile([128, QCH], BF16, tag="rr")
                nc.scalar.activation(rr[:], ss,
                                     mybir.ActivationFunctionType.Ln,
                                     scale=1.0 / HEAD_DIM)
                act_cost(QCH)
                rstd = work.tile([128, QCH], BF16, tag="rs")
                nc.scalar.activation(rstd[:], rr[:],
                                     mybir.ActivationFunctionType.Exp,
                                     scale=-0.5)
                act_cost(QCH)
                nc.vector.tensor_mul(qk_sb[:, m, n * QCH:(n + 1) * QCH],
                                     pss, rstd[:])

            # lazy rope: one closure per DVE op, drained one per qkv yield so
            # the qkv norm-muls (psum release) never queue behind a burst
            ropeq = deque()

            def rope_mtile(m):
                src = qk_sb[:, m, :]
                sw = ropep.tile([128, T], BF16, tag="rp")
                for off in (0, 64):
                    ropeq.append(lambda o=off: nc.vector.tensor_copy(
                        sw[o:o + 32, :], src[o + 32:o + 64, :]))
                    ropeq.append(lambda o=off: nc.vector.tensor_copy(
                        sw[o + 32:o + 64, :], src[o:o + 32, :]))
                ropeq.append(lambda: nc.vector.tensor_mul(sw[:], sw[:], sin_sb[:]))
                ropeq.append(lambda: nc.vector.tensor_mul(src, src, cos_sb[:]))
                ropeq.append(lambda: nc.vector.tensor_add(src, src, sw[:]))

            def qkv_unit(m):
                # pss borrows the (pre-phase-idle) s2 slots, ss the y slots:
                # the qkv pipeline gets baseline's 4-bank depth while the
                # attention pools keep their static reservation.
                # private 2-deep pss ring per generator (q-mtiles borrow the
                # pre-phase-idle s2 slots, k-mtiles the mm slots): a slot is
                # reused only 2 chunks later, far past its tail chain
                w_sb = wq_sb if m < PAIRS else wk_sb
                mloc = (m % PAIRS) * 128
                pend = None
                ps2 = None
                for n in range(NQC):
                    if m < PAIRS:
                        # 4-deep ring: two chunks per 2-bank s2 slot
                        if n % 2 == 0:
                            ps2 = ps_s2.tile([128, 2, QCH], F32, tag="s2",
                                             name=f"qk{m}_{n}")
                        pss = ps2[:, n % 2, :]
                    else:
                        pss = ps_mm.tile([128, QCH], F32, tag="mm",
                                         name=f"qk{m}_{n}")
                    for k0 in range(0, CT, 2):
                        for k in (k0, k0 + 1):
                            nc.tensor.matmul(
                                pss,
                                lhsT=w_sb[:, k, mloc:mloc + 128],
                                rhs=xT_sb[:, k, n * QCH:(n + 1) * QCH],
                                start=(k == 0), stop=(k == CT - 1),
                                skip_group_check=True,
                            )
                            mm_cost(QCH)
                        yield
                    # square must be ACT: DVE cannot read two PSUM operands
                    sq = work.tile([128, QCH], BF16, tag="sq")
                    nc.scalar.activation(sq[:], pss,
                                         mybir.ActivationFunctionType.Square)
                    act_cost(QCH)
                    if pend is not None:
                        qkv_tail(m, *pend)
                        yield
                    pend = (pss, sq, n)
                qkv_tail(m, *pend)
                yield
                rope_mtile(m)

            def qkv_pair_gen(pn):
                # alternate q- and k-mtile steps (q primed 2 chunks ahead so
                # the two tails never bunch at the same boundary)
                a, b = qkv_unit(pn), qkv_unit(PAIRS + pn)
                for _ in range(8):
                    try:
                        next(a)
                    except StopIteration:
                        break
                while True:
                    na = nb = False
                    try:
                        next(a)
                    except StopIteration:
                        na = True
                    if ropeq:
                        ropeq.popleft()()
                    try:
                        next(b)
                    except StopIteration:
                        nb = True
                    if ropeq:
                        ropeq.popleft()()
                    if na and nb:
                        return
                    yield

            def proj_stage_gen(kfs, first_stage):
                for tt in range(NTT):
                    pss = ps_mm.tile([128, FL], F32, tag="mm",
                                     name=f"pj{kfs[0]}_{tt}")
                    for i, kf in enumerate(kfs):
                        nc.tensor.matmul(
                            pss,
                            lhsT=qk_sb[:, kf, tt * 128:(tt + 1) * 128],
                            rhs=wp_sb[:, kf, :],
                            start=(i == 0), stop=(i == len(kfs) - 1),
                        )
                        mm_cost(FL)
                        yield
                    pp = ppart[:, tt, :]
                    if first_stage:
                        nc.vector.tensor_copy(pp, pss)
                    else:
                        nc.vector.tensor_add(pp, pss, pp)
                    yield

            # ---------------- filler machinery ----------------
            # list of [label, generator, done]; fill_step picks the first
            # not-done generator whose gate is open (order = priority).
            fillers = []
            gate_from = {}   # label -> cc whose writeback must be issued
            cc_issued = set()
            cur_pair = [0]

            def fill_step():
                for ent in fillers:
                    if ent[2] or gate_from.get(ent[0], -1) not in cc_issued | {-1}:
                        continue
                    try:
                        next(ent[1])
                        return True
                    except StopIteration:
                        ent[2] = True
                        continue
                return False

            FILL_BIAS = 2000.0   # keep PE a bit ahead of ACT (rows-equiv)

            def fill():
                # pair 0: steady front-loaded drip of deferred v-proj tiles
                # (AV(j=tt) consumes v2 tile tt -- the drip beats deadlines)
                if cur_pair[0] == 0 and not fillers[0][2]:
                    for _ in range(2):
                        try:
                            next(fillers[0][1])
                        except StopIteration:
                            fillers[0][2] = True
                            break
                while led["pe"] < led["act"] + FILL_BIAS:
                    if not fill_step():
                        break

            def flush(label):
                for ent in fillers:
                    if ent[0] == label and not ent[2]:
                        for _ in ent[1]:
                            pass
                        ent[2] = True

            # ---------------- attention (flipped AV) ----------------
            # cross-pair queue of deferred work: ("tr", p, ao, qt) transposes
            # and ("cc", p) exchange launches drain one per j-iteration, so
            # cqi/pair epilogues never idle the scalar engine
            postq = deque()

            def do_transpose(p, ao, qt):
                # scratch from the mm ring: a transpose in the s2 ring would
                # halve the QK pipeline's lookahead (the scores ring is the
                # attention loop's binding resource)
                tr_t = ps_mm.tile([128, FL], F32, tag="mm",
                                  name=f"tr{p}_{qt}")
                area = tr_t[:, 0:64].bitcast(BF16)
                nc.tensor.transpose(area, ao[:].opt(), ident_sb[:])
                mm_cost(128)
                nc.vector.tensor_copy(
                    aoT_sb[:, p, qt * 128:(qt + 1) * 128], area)

            def do_cc(p):
                nc.sync.dma_start(cc_ins[p][:], aoT_sb[:, p, :])
                if not no_cc:
                    nc.gpsimd.collective_compute(
                        "AllGather",
                        mybir.AluOpType.bypass,
                        replica_groups=[[0, 1], [2, 3], [4, 5], [6, 7]],
                        ins=[cc_ins[p][:].opt()],
                        outs=[cc_outs[p][:].opt()],
                    )
                # qk slots p and 4+p are dead after attention p: receive the
                # gathered pair there (slot index == global f-tile index)
                if no_cc:
                    nc.sync.dma_start(qk_sb[:, p, :], cc_ins[p][:])
                    nc.sync.dma_start(qk_sb[:, PAIRS + p, :], cc_ins[p][:])
                else:
                    nc.sync.dma_start(qk_sb[:, p, :], cc_outs[p][0])
                    nc.sync.dma_start(qk_sb[:, PAIRS + p, :], cc_outs[p][1])
                cc_issued.add(p)

            def pop_post(curp=None):
                # keep >=3 of the CURRENT pair's transposes pending (their
                # divs need DVE time); older pairs' items drain immediately
                if not postq:
                    return False
                if len(postq) <= 3 and postq[0][1] == curp:
                    return False
                it = postq.popleft()
                if it[0] == "tr":
                    do_transpose(*it[1:])
                else:
                    do_cc(it[1])
                return True

            # AV issuance lags QK/exp by 2 k-tiles and carries across
            # q-chunk and pair boundaries: the scalar engine's exp stream
            # never waits for an epilogue
            pend_av = deque()

            def attention_pair(p):
                qT = qk_sb[:, p, :]
                kT = qk_sb[:, PAIRS + p, :]
                vbase = p * VW

                def do_qk(cqi, j):
                    qg0 = cqi * 4
                    jr = j - qg0
                    q0 = max(jr, 0) * 128
                    sq_sl = slice(cqi * QCH + q0, (cqi + 1) * QCH)
                    s2 = ps_s2.tile([128, 2, QCH], F32, tag="s2",
                                    name=f"s2_{p}_{cqi}_{j}")
                    nc.tensor.matmul(s2[:, 0, q0:QCH],
                                     lhsT=kT[0:64, j * 128:(j + 1) * 128],
                                     rhs=qT[0:64, sq_sl], start=True,
                                     stop=(jr < 0),
                                     skip_group_check=True)
                    mm_cost(QCH - q0)
                    nc.tensor.matmul(s2[:, 1, q0:QCH],
                                     lhsT=kT[64:128, j * 128:(j + 1) * 128],
                                     rhs=qT[64:128, sq_sl], start=True,
                                     stop=(jr < 0),
                                     skip_group_check=True)
                    mm_cost(QCH - q0)
                    if jr >= 0:
                        # causal bias on the diagonal block (per head --
                        # a matmul output must stay inside one psum bank)
                        for h in range(2):
                            nc.tensor.matmul(
                                s2[:, h, q0:q0 + 128], lhsT=mneg_sb[:],
                                rhs=ident2_sb[:, 0:128],
                                start=False, stop=(h == 1),
                                skip_group_check=True)
                            mm_cost(128)
                    pt = ptp.tile([128, 2, QCH], BF16, tag="pt")
                    nc.scalar.activation(pt[:, :, q0:QCH], s2[:, :, q0:QCH],
                                         mybir.ActivationFunctionType.Exp,
                                         scale=0.125)
                    act_cost(2 * (QCH - q0))
                    return pt

                for cqi in range(NQC):
                    qg0 = cqi * 4          # first global qtile of this chunk
                    kmax = qg0 + 4
                    y_t = [ps_y.tile([128, 2 * VW], F32, tag="y",
                                     name=f"yp{p}_{cqi}_{h}")
                           for h in range(2)]

                    def finalize(qt, yt, base):
                        # dens at cols base+64 and base+129
                        recip2 = work.tile([128, 2], F32, tag="rc",
                                           name=f"rc{p}_{qt}")
                        den_ap = _reap(yt[:, base + 64], [(HEAD_DIM + 1, 2)])
                        nc.vector.reciprocal_approx_fast(recip2[:], den_ap)
                        ao = aop.tile([128, 2, 64], BF16, tag="ao",
                                      name=f"ao{p}_{qt}")
                        for h in range(2):
                            nc.vector.tensor_scalar_mul(
                                ao[:, h, :],
                                _reap(yt[:, base + h * (HEAD_DIM + 1)], [(1, 64)]),
                                recip2[:, h:h + 1])
                        postq.append(("tr", p, ao, qt))

                    def issue_av(pt, j, qg0=qg0, y_t=y_t):
                        # default args bind THIS q-chunk's state: entries are
                        # popped after the loop variables have moved on
                        jr = j - qg0
                        for qtl in range(max(jr, 0), 4):
                            qt = qg0 + qtl
                            yt = y_t[qtl // 2]
                            base = (qtl % 2) * VW
                            for h in range(2):
                                o = h * (HEAD_DIM + 1)
                                # psum 'start' poisons the whole zero-region
                                # (bank): only the FIRST matmul touching each
                                # y tile may set it; the lazy zero-on-write
                                # initializes the other head/qt sub-groups
                                nc.tensor.matmul(
                                    yt[:, base + o: base + o + HEAD_DIM + 1],
                                    lhsT=pt[:, h, qtl * 128:(qtl + 1) * 128],
                                    rhs=v2_sb[:, j, vbase + o: vbase + o + HEAD_DIM + 1],
                                    start=(j == 0 and h == 0 and qtl % 2 == 0),
                                    stop=(j == qt),
                                    skip_group_check=True,
                                )
                                mm_cost(HEAD_DIM + 1)
                            if j == qt:
                                finalize(qt, yt, base)

                    for j in range(kmax):
                        pt = do_qk(cqi, j)
                        pend_av.append((issue_av, pt, j))
                        # one deferred transpose/cc per iteration
                        pop_post(p)
                        # fillers go BEFORE the lagged AV so the in-order PE
                        # chews them while exp(j-2) finishes
                        fill()
                        if len(pend_av) > 2:
                            fn, pt_, j_ = pend_av.popleft()
                            fn(pt_, j_)
                    # q-chunk epilogue: the accumulator ring recycles at the
                    # next y_t alloc, so all its AVs must be issued NOW (the
                    # transposes/cc stay deferred -- they use fresh scratch)
                    while pend_av:
                        fn, pt_, j_ = pend_av.popleft()
                        fn(pt_, j_)

            # ================= program =================
            # pre-phase: ALL qkv+norm+rope (their ACT/DVE tails hide under
            # the PE-dense mains), then v-proj tiles 0-3.  v tiles 4-15 and
            # the staged projection are the attention fillers: pure PE work
            # with no cross-engine chains to tangle with the attention loop.
            for pn in range(PAIRS):
                for _ in qkv_pair_gen(pn):
                    pass
            while ropeq:    # last pair's rope ops
                ropeq.popleft()()
            for tt in range(6):
                for _ in v_unit(tt):
                    pass

            def vdef_gen():
                for tt in range(6, NTT):
                    yield from v_unit(tt)

            fillers.append(["vdef", vdef_gen(), False])
            fillers.append(["projA", proj_stage_gen([0, PAIRS], True), False])
            fillers.append(["projB", proj_stage_gen([1, PAIRS + 1], False), False])
            fillers.append(["projC", proj_stage_gen([2, PAIRS + 2], False), False])
            gate_from.update({"projA": 0, "projB": 1, "projC": 2})

            for p in range(PAIRS):
                cur_pair[0] = p
                led["pe"] = led["act"] = 0.0
                attention_pair(p)
                # pair's transposes are already queued to postq (its last
                # q-chunk flushed pend_av), so FIFO keeps cc after them
                postq.append(("cc", p))
                if p == PAIRS - 2:
                    flush("vdef")  # safety: wv/xT die with the early pool
                    early_ctx.close()

            cur_pair[0] = PAIRS
            while postq:
                pop_post(None)
            for ent in fillers:
                flush(ent[0])

            # ---- projection tail: last pair's f-tiles + combine ----
            # psum cycles all three pools (6 slots): the adds/DMAs pipeline
            for tt in range(NTT):
                sel = tt % 3
                if sel == 0:
                    pss = ps_mm.tile([128, FL], F32, tag="mm", name=f"pf{tt}")
                elif sel == 1:
                    pss = ps_s2.tile([128, 2, QCH], F32, tag="s2",
                                     name=f"pf{tt}")[:, 0, :]
                else:
                    pss = ps_y.tile([128, FL], F32, tag="y", name=f"pf{tt}")
                for i, kf in enumerate([PAIRS - 1, 2 * PAIRS - 1]):
                    nc.tensor.matmul(
                        pss,
                        lhsT=qk_sb[:, kf, tt * 128:(tt + 1) * 128],
                        rhs=wp_sb[:, kf, :],
                        start=(i == 0), stop=(i == 1),
                    )
                    mm_cost(FL)
                ysb = evw.tile([128, FL], F32, tag="ev")
                nc.vector.tensor_add(ysb[:], pss, ppart[:, tt, :])
                nc.sync.dma_start(y_d[tt * 128:(tt + 1) * 128, :], ysb[:])

    nc.compile()
    return nc


def _prep_core_inputs(x, Wqkv, Wproj, q_norm_w, k_norm_w, core):
    b, g = core // 2, core % 2
    bf = ml_dtypes.bfloat16
    xT = np.ascontiguousarray(x[b].T).astype(bf)
    cols = slice(g * FL, (g + 1) * FL)
    wq = Wqkv[:, 0:C][:, cols] * np.tile(q_norm_w, H_LOCAL)[None, :]
    wk = Wqkv[:, C:2 * C][:, cols] * np.tile(k_norm_w, H_LOCAL)[None, :]
    wv = Wqkv[:, 2 * C:3 * C][:, cols]
    wp = Wproj[:, cols]
    return {
        "xT": xT,
        "Wq": np.ascontiguousarray(wq).astype(bf),
        "Wk": np.ascontiguousarray(wk).astype(bf),
        "Wv": np.ascontiguousarray(wv).astype(bf),
        "Wp": np.ascontiguousarray(wp).astype(bf),
    }


def kernel(x, Wqkv, Wproj, q_norm_w, k_norm_w):
    if "nc" not in _cached:
        _cached["nc"] = build_program()
    nc = _cached["nc"]

    x = np.asarray(x, dtype=np.float32)
    Wqkv = np.asarray(Wqkv, dtype=np.float32)
    Wproj = np.asarray(Wproj, dtype=np.float32)
    q_norm_w = np.asarray(q_norm_w, dtype=np.float32)
    k_norm_w = np.asarray(k_norm_w, dtype=np.float32)

    in_maps = [
        _prep_core_inputs(x, Wqkv, Wproj, q_norm_w, k_norm_w, c) for c in range(8)
    ]
    res = run_bass_kernel_spmd(nc, in_maps, list(range(8)))
    outs = res.results

    y = np.empty((B, T, C), dtype=np.float32)
    for b in range(B):
        y[b, :, 0:FL] = outs[2 * b]["y"]
        y[b, :, FL:C] = outs[2 * b + 1]["y"]
    return y


# revision 81
# speedup vs baseline: 1.0508x; 1.0024x over previous
"""Causal self-attention (QK-RMSNorm + RoPE) Trainium2 kernel.

Sharding: 8 cores = 4 batches x 2 head-groups (Megatron-style over heads).
Core c handles batch b=c//2, heads [g*8, g*8+8) with g=c%2.
Each core computes y[b, :, g*512:(g+1)*512] (output-column sharding of the
projection after a pairwise AllGather of attention outputs), so the host
only concatenates slices - no host-side arithmetic.

Perf notes (cost model charges out-free-size rows per matmul, independent of
contraction depth and output-partition count):
- AV is computed in the [q-tokens(part), head-dims(free)] orientation with a
  ones column appended to V per head: the 65-wide moving tensor makes AV cost
  65 rows/tile instead of 128-512, and the softmax denominator accumulates
  for free in column 64.  The division is then a per-partition scalar
  multiply (DVE), and the output is transposed back to [dims, tok] with
  cheap PE transposes (128 rows each) for the AllGather + projection.
- The per-head sum-of-squares for QK-RMSNorm uses one block-diagonal-ones
  matmul covering both packed heads; squares are computed on DVE in bf16
  from a Pool-engine drain of the qkv psum (keeps the scalar engine free
  for the attention exp()s, which are its binding load).
- The in-order PE is kept saturated (and in max p-state) by interleaving
  filler matmuls into the attention loop, driven by a PE-vs-ACT issued-work
  ledger: qkv of the next pair, deferred v-proj tiles, and the partial
  projection of already-gathered f-tiles (staged per AllGather arrival,
  accumulated into a bf16 partial on the Pool engine).  Only the last
  pair's two f-tiles + a DVE add remain after the final AllGather.
- Transpose scratch lives in the s2 (scores) PSUM ring, so the AV
  accumulator ring is released by the division and never serializes
  consecutive q-chunks.
"""


import numpy as np
import ml_dtypes
from collections import deque
from contextlib import ExitStack

import concourse.bass as bass
import concourse.bacc as bacc

# Force all activations into the one table set that covers Exp+Ln+Square+
# Copy+Identity, so the whole kernel needs exactly one ACT_TABLE_LOAD.
import concourse.hw_specs as _hw_specs
_orig_gat = _hw_specs.get_activation_tables

def _gat_one_set(arch):
    t = _orig_gat(arch)
    return {k: (v if k == "natural_log_exp_and_others" else set())
            for k, v in t.items()}

bacc.get_activation_tables = _gat_one_set
import concourse.mybir as mybir
import concourse.tile as tile
from concourse.bass_utils import run_bass_kernel_spmd

BF16 = mybir.dt.bfloat16
F32 = mybir.dt.float32

N_HEAD = 16
HEAD_DIM = 64
EPS = 1e-5
ROPE_BASE = 10000.0

B, T, C = 4, 2048, 1024
H_LOCAL = N_HEAD // 2          # heads per core
PAIRS = H_LOCAL // 2           # head-pairs per core (processed 2-at-a-time)
CT = C // 128                  # contraction tiles over C
FL = H_LOCAL * HEAD_DIM        # local feature width (512)
QCH = 512                      # q-chunk width
NQC = T // QCH                 # q-chunks
NKT = T // 128                 # k tiles
NTT = T // 128                 # token tiles
VW = 2 * (HEAD_DIM + 1)        # per-pair v2 width: [A dims|onesA|B dims|onesB]

_cached = {}


def _reap(ap, dims):
    """Rebuild an AP keeping tensor/offset/partition dim, with free dims
    `dims` given as (stride, size) pairs."""
    return bass.AP(tensor=ap.tensor, offset=ap.offset,
                   ap=[ap.ap[0]] + [list(d) for d in dims])


def _fbcast2(ap):
    """[128, N] AP -> [128, 2, N] with the middle (free) dim broadcast."""
    return bass.AP(
        tensor=ap.tensor, offset=ap.offset, ap=[ap.ap[0], [0, 2], ap.ap[1]]
    )


def _rope_tables():
    inv_freq = 1.0 / (ROPE_BASE ** (np.arange(0, HEAD_DIM, 2, dtype=np.float64) / HEAD_DIM))
    t = np.arange(T, dtype=np.float64)
    freqs = np.outer(t, inv_freq)                       # [T, 32]
    emb = np.concatenate([freqs, freqs], -1)            # [T, 64]
    cos = np.cos(emb).astype(np.float32).T              # [64, T]
    sin = np.sin(emb).astype(np.float32).T              # [64, T]
    cos2 = np.concatenate([cos, cos], 0)                # [128, T] two heads
    sin_s = sin.copy()
    sin_s[0:32] = -sin_s[0:32]                          # rotate-half sign
    sin2 = np.concatenate([sin_s, sin_s], 0)            # [128, T]
    return cos2.astype(ml_dtypes.bfloat16), sin2.astype(ml_dtypes.bfloat16)


def _diag_masks():
    # corner mask: keep where k_partition <= q_col (lower-triangular 128x128)
    p = np.arange(128)[:, None]
    qf = np.arange(128)[None, :]
    m = (p <= qf).astype(np.float32)
    return m.astype(ml_dtypes.bfloat16)                 # [128, 128]


def build_program(no_cc=False):
    nc = bacc.Bacc("TRN2", target_bir_lowering=False, debug=False,
                   num_devices=1 if no_cc else 8)

    xT_d = nc.dram_tensor("xT", [C, T], BF16, kind="ExternalInput")
    wq_d = nc.dram_tensor("Wq", [C, FL], BF16, kind="ExternalInput")
    wk_d = nc.dram_tensor("Wk", [C, FL], BF16, kind="ExternalInput")
    wv_d = nc.dram_tensor("Wv", [C, FL], BF16, kind="ExternalInput")
    wp_d = nc.dram_tensor("Wp", [C, FL], BF16, kind="ExternalInput")
    y_d = nc.dram_tensor("y", [T, FL], F32, kind="ExternalOutput")

    cos2_np, sin2_np = _rope_tables()
    cos_d = nc.inline_tensor(np.ascontiguousarray(cos2_np), "cos2")
    sin_d = nc.inline_tensor(np.ascontiguousarray(sin2_np), "sin2")
    # causal mask as a score bias: out[p,g,f] += mneg[f,p] = -30000*(p>f),
    # added to the diagonal 128x128 block by one PE matmul (keeps the
    # exp->AV chain off the vector engine)
    mneg_np = -30000.0 * (np.arange(128)[None, :] > np.arange(128)[:, None])
    mneg_d = nc.inline_tensor(
        np.ascontiguousarray(mneg_np.astype(ml_dtypes.bfloat16)), "mneg")
    id2_np = np.tile(np.eye(128, dtype=ml_dtypes.bfloat16), (1, 2))
    ident2_d = nc.inline_tensor(np.ascontiguousarray(id2_np), "ident2")
    bd_np = np.zeros((128, 128), dtype=ml_dtypes.bfloat16)
    bd_np[0:64, 0:64] = 1.0
    bd_np[64:128, 64:128] = 1.0
    onesbd_d = nc.inline_tensor(np.ascontiguousarray(bd_np), "onesbd")
    ident_d = nc.inline_tensor(
        np.ascontiguousarray(np.eye(128, dtype=ml_dtypes.bfloat16)), "ident")

    # per-pair exchange buffers
    cc_ins = [nc.dram_tensor(f"cc_in{p}", [128, T], BF16) for p in range(PAIRS)]
    cc_outs = [nc.dram_tensor(f"cc_out{p}", [2, 128, T], BF16) for p in range(PAIRS)]

    # --- PE-vs-ACT issued-work ledger (units: bf16 matmul rows = 0.4167ns) ---
    led = {"pe": 0.0, "act": 0.0}

    def mm_cost(rows):
        led["pe"] += rows

    def act_cost(free):
        led["act"] += 2.0 * free + 444.0

    with tile.TileContext(nc) as tc:
        with (
            tc.tile_pool(name="const", bufs=1) as const,
            tc.tile_pool(name="work", bufs=2) as work,
            tc.tile_pool(name="evw", bufs=6) as evw,
            tc.tile_pool(name="rope", bufs=2) as ropep,
            tc.tile_pool(name="pt", bufs=10) as ptp,
            tc.tile_pool(name="qraw", bufs=3) as qrawp,
            tc.tile_pool(name="ao", bufs=6) as aop,
            tc.tile_pool(name="ps_s2", bufs=2, space="PSUM") as ps_s2,
            tc.tile_pool(name="ps_y", bufs=2, space="PSUM") as ps_y,
            tc.tile_pool(name="ps_mm", bufs=2, space="PSUM") as ps_mm,
        ):
            early_ctx = ExitStack()
            early = early_ctx.enter_context(tc.tile_pool(name="early", bufs=1))

            # ---- inputs: wq/wk/xT first (qkv(0) consumes them first) ----
            wq_sb = early.tile([128, CT, FL], BF16)
            wk_sb = early.tile([128, CT, FL], BF16)
            xT_sb = early.tile([128, CT, T], BF16)
            # few, large DMAs: the HWDGE serializes ~625ns per DMA instruction.
            # xT halved with first halves first: qkv chunk 0/1 start earlier.
            nc.sync.dma_start(wq_sb[:], wq_d[:].rearrange("(k p) f -> p k f", p=128))
            for k in range(CT):
                nc.sync.dma_start(xT_sb[:, k, 0:T // 2],
                                  xT_d[k * 128:(k + 1) * 128, 0:T // 2])
            # wk after xT half-0: the primed q-mtile only needs wq + xT
            nc.sync.dma_start(wk_sb[:], wk_d[:].rearrange("(k p) f -> p k f", p=128))
            for k in range(CT):
                nc.sync.dma_start(xT_sb[:, k, T // 2:T],
                                  xT_d[k * 128:(k + 1) * 128, T // 2:T])
            wv_sb = early.tile([128, CT, FL], BF16)
            nc.sync.dma_start(wv_sb[:], wv_d[:].rearrange("(k p) f -> p k f", p=128))
            cos_sb = early.tile([128, T], BF16)
            nc.sync.dma_start(cos_sb[:], cos_d[:])
            sin_sb = early.tile([128, T], BF16)
            nc.sync.dma_start(sin_sb[:], sin_d[:])
            mneg_sb = const.tile([128, 128], BF16)
            nc.sync.dma_start(mneg_sb[:], mneg_d[:])
            ident2_sb = const.tile([128, 256], BF16)
            nc.sync.dma_start(ident2_sb[:], ident2_d[:])
            ident_sb = const.tile([128, 128], BF16)
            nc.sync.dma_start(ident_sb[:], ident_d[:])
            wp_sb = const.tile([128, CT, FL], BF16)
            nc.sync.dma_start(wp_sb[:], wp_d[:].rearrange("(k p) f -> p k f", p=128))

            # block-diagonal ones: one matmul sums squares of both packed heads
            ones_bd = const.tile([128, 128], BF16)
            nc.sync.dma_start(ones_bd[:], onesbd_d[:])

            qk_sb = const.tile([128, 2 * PAIRS, T], BF16)
            # v2: per tt, per pair: [A dims(64) | onesA | B dims(64) | onesB]
            v2_sb = const.tile([128, NTT, PAIRS * VW], BF16)
            aoT_sb = const.tile([128, PAIRS, T], BF16)
            # bf16 partial of the projection over already-gathered f-tiles
            ppart = const.tile([128, NTT, FL], BF16)

            # ones columns of v2 (once, before any v-proj writes)
            pstr = v2_sb.ap[0][0]
            ones_ap = bass.AP(
                tensor=v2_sb.tensor, offset=v2_sb.offset + HEAD_DIM,
                ap=[[pstr, 128], [PAIRS * VW, NTT], [VW, PAIRS], [HEAD_DIM + 1, 2]])
            nc.vector.memset(ones_ap, 1.0)

            # ---------------- unit generators (PE fillers) ----------------
            def v_unit(tt):
                pss = ps_mm.tile([128, FL], F32, tag="mm", name=f"vp{tt}")
                for k0 in range(0, CT, 2):
                    for k in (k0, k0 + 1):
                        nc.tensor.matmul(
                            pss,
                            lhsT=xT_sb[:, k, tt * 128:(tt + 1) * 128],
                            rhs=wv_sb[:, k, :],
                            start=(k == 0), stop=(k == CT - 1),
                        )
                        mm_cost(FL)
                    yield
                # drain into v2 layout: [128,4,2,64] both sides (DVE --
                # GPSIMD cannot touch PSUM on real hardware)
                src = _reap(pss, [(128, PAIRS), (64, 2), (1, 64)])
                dst = bass.AP(tensor=v2_sb.tensor,
                              offset=v2_sb.offset + tt * PAIRS * VW,
                              ap=[[pstr, 128], [VW, PAIRS], [HEAD_DIM + 1, 2], [1, 64]])
                nc.vector.tensor_copy(dst, src)
                yield

            def qkv_tail(m, pss, sq, n):
                ss = ps_y.tile([128, QCH], F32, tag="y", name=f"ss{m}_{n}")
                nc.tensor.matmul(ss, lhsT=ones_bd[:], rhs=sq[:],
                                 start=True, stop=True, skip_group_check=True)
                mm_cost(QCH)
                # rstd = (ss/64)^(-1/2) = exp(-0.5*ln(ss/64)); eps negligible.
                rr = work.tile([128, QCH], BF16, tag="rr")
                nc.scalar.activation(rr[:], ss,
                                     mybir.ActivationFunctionType.Ln,
                                     scale=1.0 / HEAD_DIM)
                act_cost(QCH)
                rstd = work.tile([128, QCH], BF16, tag="rs")
                nc.scalar.activation(rstd[:], rr[:],
                                     mybir.ActivationFunctionType.Exp,
                                     scale=-0.5)
                act_cost(QCH)
                nc.vector.tensor_mul(qk_sb[:, m, n * QCH:(n + 1) * QCH],
                                     pss, rstd[:])

            # lazy rope: one closure per DVE op, drained one per qkv yield so
            # the qkv norm-muls (psum release) never queue behind a burst
            ropeq = deque()

            def rope_mtile(m):
                src = qk_sb[:, m, :]
                sw = ropep.tile([128, T], BF16, tag="rp")
                for off in (0, 64):
                    ropeq.append(lambda o=off: nc.vector.tensor_copy(
                        sw[o:o + 32, :], src[o + 32:o + 64, :]))
                    ropeq.append(lambda o=off: nc.vector.tensor_copy(
                        sw[o + 32:o + 64, :], src[o:o + 32, :]))
                ropeq.append(lambda: nc.vector.tensor_mul(sw[:], sw[:], sin_sb[:]))
                ropeq.append(lambda: nc.vector.tensor_mul(src, src, cos_sb[:]))
                ropeq.append(lambda: nc.vector.tensor_add(src, src, sw[:]))

            def qkv_unit(m):
                # pss borrows the (pre-phase-idle) s2 slots, ss the y slots:
                # the qkv pipeline gets baseline's 4-bank depth while the
                # attention pools keep their static reservation.
                # private 2-deep pss ring per generator (q-mtiles borrow the
                # pre-phase-idle s2 slots, k-mtiles the mm slots): a slot is
                # reused only 2 chunks later, far past its tail chain
                w_sb = wq_sb if m < PAIRS else wk_sb
                mloc = (m % PAIRS) * 128
                pend = None
                ps2 = None
                for n in range(NQC):
                    if m < PAIRS:
                        # 4-deep ring: two chunks per 2-bank s2 slot
                        if n % 2 == 0:
                            ps2 = ps_s2.tile([128, 2, QCH], F32, tag="s2",
                                             name=f"qk{m}_{n}")
                        pss = ps2[:, n % 2, :]
                    else:
                        pss = ps_mm.tile([128, QCH], F32, tag="mm",
                                         name=f"qk{m}_{n}")
                    for k0 in range(0, CT, 2):
                        for k in (k0, k0 + 1):
                            nc.tensor.matmul(
                                pss,
                                lhsT=w_sb[:, k, mloc:mloc + 128],
                                rhs=xT_sb[:, k, n * QCH:(n + 1) * QCH],
                                start=(k == 0), stop=(k == CT - 1),
                                skip_group_check=True,
                            )
                            mm_cost(QCH)
                        yield
                    # square must be ACT: DVE cannot read two PSUM operands
                    sq = work.tile([128, QCH], BF16, tag="sq")
                    nc.scalar.activation(sq[:], pss,
                                         mybir.ActivationFunctionType.Square)
                    act_cost(QCH)
                    if pend is not None:
                        qkv_tail(m, *pend)
                        yield
                    pend = (pss, sq, n)
                qkv_tail(m, *pend)
                yield
                rope_mtile(m)

            def qkv_pair_gen(pn):
                # alternate q- and k-mtile steps (q primed 2 chunks ahead so
                # the two tails never bunch at the same boundary)
                a, b = qkv_unit(pn), qkv_unit(PAIRS + pn)
                for _ in range(8):
                    try:
                        next(a)
                    except StopIteration:
                        break
                while True:
                    na = nb = False
                    try:
                        next(a)
                    except StopIteration:
                        na = True
                    if ropeq:
                        ropeq.popleft()()
                    try:
                        next(b)
                    except StopIteration:
                        nb = True
                    if ropeq:
                        ropeq.popleft()()
                    if na and nb:
                        return
                    yield

            def proj_stage_gen(kfs, first_stage):
                for tt in range(NTT):
                    pss = ps_mm.tile([128, FL], F32, tag="mm",
                                     name=f"pj{kfs[0]}_{tt}")
                    for i, kf in enumerate(kfs):
                        nc.tensor.matmul(
                            pss,
                            lhsT=qk_sb[:, kf, tt * 128:(tt + 1) * 128],
                            rhs=wp_sb[:, kf, :],
                            start=(i == 0), stop=(i == len(kfs) - 1),
                        )
                        mm_cost(FL)
                        yield
                    pp = ppart[:, tt, :]
                    if first_stage:
                        nc.vector.tensor_copy(pp, pss)
                    else:
                        nc.vector.tensor_add(pp, pss, pp)
                    yield

            # ---------------- filler machinery ----------------
            # list of [label, generator, done]; fill_step picks the first
            # not-done generator whose gate is open (order = priority).
            fillers = []
            gate_from = {}   # label -> cc whose writeback must be issued
            cc_issued = set()
            cur_pair = [0]

            def fill_step():
                for ent in fillers:
                    if ent[2] or gate_from.get(ent[0], -1) not in cc_issued | {-1}:
                        continue
                    try:
                        next(ent[1])
                        return True
                    except StopIteration:
                        ent[2] = True
                        continue
                return False

            FILL_BIAS = 2000.0   # keep PE a bit ahead of ACT (rows-equiv)

            def fill():
                # pair 0: steady front-loaded drip of deferred v-proj tiles
                # (AV(j=tt) consumes v2 tile tt -- the drip beats deadlines)
                if cur_pair[0] == 0 and not fillers[0][2]:
                    for _ in range(2):
                        try:
                            next(fillers[0][1])
                        except StopIteration:
                            fillers[0][2] = True
                            break
                while led["pe"] < led["act"] + FILL_BIAS:
                    if not fill_step():
                        break

            def flush(label):
                for ent in fillers:
                    if ent[0] == label and not ent[2]:
                        for _ in ent[1]:
                            pass
                        ent[2] = True

            # ---------------- attention (flipped AV) ----------------
            # cross-pair queue of deferred work: ("tr", p, ao, qt) transposes
            # and ("cc", p) exchange launches drain one per j-iteration, so
            # cqi/pair epilogues never idle the scalar engine
            postq = deque()

            def do_transpose(p, ao, qt):
                # scratch from the mm ring: a transpose in the s2 ring would
                # halve the QK pipeline's lookahead (the scores ring is the
                # attention loop's binding resource)
                tr_t = ps_mm.tile([128, FL], F32, tag="mm",
                                  name=f"tr{p}_{qt}")
                area = tr_t[:, 0:64].bitcast(BF16)
                nc.tensor.transpose(area, ao[:].opt(), ident_sb[:])
                mm_cost(128)
                nc.vector.tensor_copy(
                    aoT_sb[:, p, qt * 128:(qt + 1) * 128], area)

            def do_cc(p):
                nc.sync.dma_start(cc_ins[p][:], aoT_sb[:, p, :])
                if not no_cc:
                    nc.gpsimd.collective_compute(
                        "AllGather",
                        mybir.AluOpType.bypass,
                        replica_groups=[[0, 1], [2, 3], [4, 5], [6, 7]],
                        ins=[cc_ins[p][:].opt()],
                        outs=[cc_outs[p][:].opt()],
                    )
                # qk slots p and 4+p are dead after attention p: receive the
                # gathered pair there (slot index == global f-tile index)
                if no_cc:
                    nc.sync.dma_start(qk_sb[:, p, :], cc_ins[p][:])
                    nc.sync.dma_start(qk_sb[:, PAIRS + p, :], cc_ins[p][:])
                else:
                    nc.sync.dma_start(qk_sb[:, p, :], cc_outs[p][0])
                    nc.sync.dma_start(qk_sb[:, PAIRS + p, :], cc_outs[p][1])
                cc_issued.add(p)

            def pop_post(curp=None):
                # keep >=3 of the CURRENT pair's transposes pending (their
                # divs need DVE time); older pairs' items drain immediately
                if not postq:
                    return False
                if len(postq) <= 3 and postq[0][1] == curp:
                    return False
                it = postq.popleft()
                if it[0] == "tr":
                    do_transpose(*it[1:])
                else:
                    do_cc(it[1])
                return True

            # AV issuance lags QK/exp by 2 k-tiles and carries across
            # q-chunk and pair boundaries: the scalar engine's exp stream
            # never waits for an epilogue
            pend_av = deque()

            def attention_pair(p):
                qT = qk_sb[:, p, :]
                kT = qk_sb[:, PAIRS + p, :]
                vbase = p * VW

                def do_qk(cqi, j):
                    qg0 = cqi * 4
                    jr = j - qg0
                    q0 = max(jr, 0) * 128
                    sq_sl = slice(cqi * QCH + q0, (cqi + 1) * QCH)
                    s2 = ps_s2.tile([128, 2, QCH], F32, tag="s2",
                                    name=f"s2_{p}_{cqi}_{j}")
                    nc.tensor.matmul(s2[:, 0, q0:QCH],
                                     lhsT=kT[0:64, j * 128:(j + 1) * 128],
                                     rhs=qT[0:64, sq_sl], start=True,
                                     stop=(jr < 0),
                                     skip_group_check=True)
                    mm_cost(QCH - q0)
                    nc.tensor.matmul(s2[:, 1, q0:QCH],
                                     lhsT=kT[64:128, j * 128:(j + 1) * 128],
                                     rhs=qT[64:128, sq_sl], start=True,
                                     stop=(jr < 0),
                                     skip_group_check=True)
                    mm_cost(QCH - q0)
                    if jr >= 0:
                        # causal bias on the diagonal block (per head --
                        # a matmul output must stay inside one psum bank)
                        for h in range(2):
                            nc.tensor.matmul(
                                s2[:, h, q0:q0 + 128], lhsT=mneg_sb[:],
                                rhs=ident2_sb[:, 0:128],
                                start=False, stop=(h == 1),
                                skip_group_check=True)
                            mm_cost(128)
                    pt = ptp.tile([128, 2, QCH], BF16, tag="pt")
                    nc.scalar.activation(pt[:, :, q0:QCH], s2[:, :, q0:QCH],
                                         mybir.ActivationFunctionType.Exp,
                                         scale=0.125)
                    act_cost(2 * (QCH - q0))
                    return pt

                for cqi in range(NQC):
                    qg0 = cqi * 4          # first global qtile of this chunk
                    kmax = qg0 + 4
                    y_t = [ps_y.tile([128, 2 * VW], F32, tag="y",
                                     name=f"yp{p}_{cqi}_{h}")
                           for h in range(2)]

                    def finalize(qt, yt, base):
                        # dens at cols base+64 and base+129
                        recip2 = work.tile([128, 2], F32, tag="rc",
                                           name=f"rc{p}_{qt}")
                        den_ap = _reap(yt[:, base + 64], [(HEAD_DIM + 1, 2)])
                        nc.vector.reciprocal_approx_fast(recip2[:], den_ap)
                        ao = aop.tile([128, 2, 64], BF16, tag="ao",
                                      name=f"ao{p}_{qt}")
                        for h in range(2):
                            nc.vector.tensor_scalar_mul(
                                ao[:, h, :],
                                _reap(yt[:, base + h * (HEAD_DIM + 1)], [(1, 64)]),
                                recip2[:, h:h + 1])
                        postq.append(("tr", p, ao, qt))

                    def issue_av(pt, j, qg0=qg0, y_t=y_t):
                        # default args bind THIS q-chunk's state: entries are
                        # popped after the loop variables have moved on
                        jr = j - qg0
                        for qtl in range(max(jr, 0), 4):
                            qt = qg0 + qtl
                            yt = y_t[qtl // 2]
                            base = (qtl % 2) * VW
                            for h in range(2):
                                o = h * (HEAD_DIM + 1)
                                # psum 'start' poisons the whole zero-region
                                # (bank): only the FIRST matmul touching each
                                # y tile may set it; the lazy zero-on-write
                                # initializes the other head/qt sub-groups
                                nc.tensor.matmul(
                                    yt[:, base + o: base + o + HEAD_DIM + 1],
                                    lhsT=pt[:, h, qtl * 128:(qtl + 1) * 128],
                                    rhs=v2_sb[:, j, vbase + o: vbase + o + HEAD_DIM + 1],
                                    start=(j == 0 and h == 0 and qtl % 2 == 0),
                                    stop=(j == qt),
                                    skip_group_check=True,
                                )
                                mm_cost(HEAD_DIM + 1)
                            if j == qt:
                                finalize(qt, yt, base)

                    for j in range(kmax):
                        pt = do_qk(cqi, j)
                        pend_av.append((issue_av, pt, j))
                        # one deferred transpose/cc per iteration
                        pop_post(p)
                        # fillers go BEFORE the lagged AV so the in-order PE
                        # chews them while exp(j-2) finishes
                        fill()
                        if len(pend_av) > 2:
                            fn, pt_, j_ = pend_av.popleft()
                            fn(pt_, j_)
                    # q-chunk epilogue: the accumulator ring recycles at the
                    # next y_t alloc, so all its AVs must be issued NOW (the
                    # transposes/cc stay deferred -- they use fresh scratch)
                    while pend_av:
                        fn, pt_, j_ = pend_av.popleft()
                        fn(pt_, j_)

            # ================= program =================
            # pre-phase: ALL qkv+norm+rope (their ACT/DVE tails hide under
            # the PE-dense mains), then v-proj tiles 0-3.  v tiles 4-15 and
            # the staged projection are the attention fillers: pure PE work
            # with no cross-engine chains to tangle with the attention loop.
            for pn in range(PAIRS):
                for _ in qkv_pair_gen(pn):
                    pass
            while ropeq:    # last pair's rope ops
                ropeq.popleft()()
            for tt in range(6):
                for _ in v_unit(tt):
                    pass

            def vdef_gen():
                for tt in range(6, NTT):
                    yield from v_unit(tt)

            fillers.append(["vdef", vdef_gen(), False])
            fillers.append(["projA", proj_stage_gen([0, PAIRS], True), False])
            fillers.append(["projB", proj_stage_gen([1, PAIRS + 1], False), False])
            fillers.append(["projC", proj_stage_gen([2, PAIRS + 2], False), False])
            gate_from.update({"projA": 0, "projB": 1, "projC": 2})

            for p in range(PAIRS):
                cur_pair[0] = p
                led["pe"] = led["act"] = 0.0
                attention_pair(p)
                # pair's transposes are already queued to postq (its last
                # q-chunk flushed pend_av), so FIFO keeps cc after them
                postq.append(("cc", p))
                if p == PAIRS - 2:
                    flush("vdef")  # safety: wv/xT die with the early pool
                    early_ctx.close()

            cur_pair[0] = PAIRS
            while postq:
                pop_post(None)
            for ent in fillers:
                flush(ent[0])

            # ---- projection tail: last pair's f-tiles + combine ----
            # psum cycles all three pools (6 slots): the adds/DMAs pipeline
            for tt in range(NTT):
                sel = tt % 3
                if sel == 0:
                    pss = ps_mm.tile([128, FL], F32, tag="mm", name=f"pf{tt}")
                elif sel == 1:
                    pss = ps_s2.tile([128, 2, QCH], F32, tag="s2",
                                     name=f"pf{tt}")[:, 0, :]
                else:
                    pss = ps_y.tile([128, FL], F32, tag="y", name=f"pf{tt}")
                for i, kf in enumerate([PAIRS - 1, 2 * PAIRS - 1]):
                    nc.tensor.matmul(
                        pss,
                        lhsT=qk_sb[:, kf, tt * 128:(tt + 1) * 128],
                        rhs=wp_sb[:, kf, :],
                        start=(i == 0), stop=(i == 1),
                    )
                    mm_cost(FL)
                ysb = evw.tile([128, FL], F32, tag="ev")
                nc.vector.tensor_add(ysb[:], pss, ppart[:, tt, :])
                nc.sync.dma_start(y_d[tt * 128:(tt + 1) * 128, :], ysb[:])

    nc.compile()
    return nc


def _prep_core_inputs(x, Wqkv, Wproj, q_norm_w, k_norm_w, core):
    b, g = core // 2, core % 2
    bf = ml_dtypes.bfloat16
    xT = np.ascontiguousarray(x[b].T).astype(bf)
    cols = slice(g * FL, (g + 1) * FL)
    wq = Wqkv[:, 0:C][:, cols] * np.tile(q_norm_w, H_LOCAL)[None, :]
    wk = Wqkv[:, C:2 * C][:, cols] * np.tile(k_norm_w, H_LOCAL)[None, :]
    wv = Wqkv[:, 2 * C:3 * C][:, cols]
    wp = Wproj[:, cols]
    return {
        "xT": xT,
        "Wq": np.ascontiguousarray(wq).astype(bf),
        "Wk": np.ascontiguousarray(wk).astype(bf),
        "Wv": np.ascontiguousarray(wv).astype(bf),
        "Wp": np.ascontiguousarray(wp).astype(bf),
    }


def kernel(x, Wqkv, Wproj, q_norm_w, k_norm_w):
    if "nc" not in _cached:
        _cached["nc"] = build_program()
    nc = _cached["nc"]

    x = np.asarray(x, dtype=np.float32)
    Wqkv = np.asarray(Wqkv, dtype=np.float32)
    Wproj = np.asarray(Wproj, dtype=np.float32)
    q_norm_w = np.asarray(q_norm_w, dtype=np.float32)
    k_norm_w = np.asarray(k_norm_w, dtype=np.float32)

    in_maps = [
        _prep_core_inputs(x, Wqkv, Wproj, q_norm_w, k_norm_w, c) for c in range(8)
    ]
    res = run_bass_kernel_spmd(nc, in_maps, list(range(8)))
    outs = res.results

    y = np.empty((B, T, C), dtype=np.float32)
    for b in range(B):
        y[b, :, 0:FL] = outs[2 * b]["y"]
        y[b, :, FL:C] = outs[2 * b + 1]["y"]
    return y


# revision 82
# speedup vs baseline: 1.0645x; 1.0130x over previous
"""Causal self-attention (QK-RMSNorm + RoPE) Trainium2 kernel.

Sharding: 8 cores = 4 batches x 2 head-groups (Megatron-style over heads).
Core c handles batch b=c//2, heads [g*8, g*8+8) with g=c%2.
Each core computes y[b, :, g*512:(g+1)*512] (output-column sharding of the
projection after a pairwise AllGather of attention outputs), so the host
only concatenates slices - no host-side arithmetic.

Perf notes (cost model charges out-free-size rows per matmul, independent of
contraction depth and output-partition count):
- AV is computed in the [q-tokens(part), head-dims(free)] orientation with a
  ones column appended to V per head: the 65-wide moving tensor makes AV cost
  65 rows/tile instead of 128-512, and the softmax denominator accumulates
  for free in column 64.  The division is then a per-partition scalar
  multiply (DVE), and the output is transposed back to [dims, tok] with
  cheap PE transposes (128 rows each) for the AllGather + projection.
- The per-head sum-of-squares for QK-RMSNorm uses one block-diagonal-ones
  matmul covering both packed heads; squares are computed on DVE in bf16
  from a Pool-engine drain of the qkv psum (keeps the scalar engine free
  for the attention exp()s, which are its binding load).
- The in-order PE is kept saturated (and in max p-state) by interleaving
  filler matmuls into the attention loop, driven by a PE-vs-ACT issued-work
  ledger: qkv of the next pair, deferred v-proj tiles, and the partial
  projection of already-gathered f-tiles (staged per AllGather arrival,
  accumulated into a bf16 partial on the Pool engine).  Only the last
  pair's two f-tiles + a DVE add remain after the final AllGather.
- Transpose scratch lives in the s2 (scores) PSUM ring, so the AV
  accumulator ring is released by the division and never serializes
  consecutive q-chunks.
"""


import numpy as np
import ml_dtypes
from collections import deque
from contextlib import ExitStack

import concourse.bass as bass
import concourse.bacc as bacc

# Force all activations into the one table set that covers Exp+Ln+Square+
# Copy+Identity, so the whole kernel needs exactly one ACT_TABLE_LOAD.
import concourse.hw_specs as _hw_specs
_orig_gat = _hw_specs.get_activation_tables

def _gat_one_set(arch):
    t = _orig_gat(arch)
    return {k: (v if k == "natural_log_exp_and_others" else set())
            for k, v in t.items()}

bacc.get_activation_tables = _gat_one_set
import concourse.mybir as mybir
import concourse.tile as tile
from concourse.bass_utils import run_bass_kernel_spmd

BF16 = mybir.dt.bfloat16
F32 = mybir.dt.float32

N_HEAD = 16
HEAD_DIM = 64
EPS = 1e-5
ROPE_BASE = 10000.0

B, T, C = 4, 2048, 1024
H_LOCAL = N_HEAD // 2          # heads per core
PAIRS = H_LOCAL // 2           # head-pairs per core (processed 2-at-a-time)
CT = C // 128                  # contraction tiles over C
FL = H_LOCAL * HEAD_DIM        # local feature width (512)
QCH = 512                      # q-chunk width
NQC = T // QCH                 # q-chunks
NKT = T // 128                 # k tiles
NTT = T // 128                 # token tiles
VW = 2 * (HEAD_DIM + 1)        # per-pair v2 width: [A dims|onesA|B dims|onesB]

_cached = {}


def _reap(ap, dims):
    """Rebuild an AP keeping tensor/offset/partition dim, with free dims
    `dims` given as (stride, size) pairs."""
    return bass.AP(tensor=ap.tensor, offset=ap.offset,
                   ap=[ap.ap[0]] + [list(d) for d in dims])


def _fbcast2(ap):
    """[128, N] AP -> [128, 2, N] with the middle (free) dim broadcast."""
    return bass.AP(
        tensor=ap.tensor, offset=ap.offset, ap=[ap.ap[0], [0, 2], ap.ap[1]]
    )


def _rope_tables():
    inv_freq = 1.0 / (ROPE_BASE ** (np.arange(0, HEAD_DIM, 2, dtype=np.float64) / HEAD_DIM))
    t = np.arange(T, dtype=np.float64)
    freqs = np.outer(t, inv_freq)                       # [T, 32]
    emb = np.concatenate([freqs, freqs], -1)            # [T, 64]
    cos = np.cos(emb).astype(np.float32).T              # [64, T]
    sin = np.sin(emb).astype(np.float32).T              # [64, T]
    cos2 = np.concatenate([cos, cos], 0)                # [128, T] two heads
    sin_s = sin.copy()
    sin_s[0:32] = -sin_s[0:32]                          # rotate-half sign
    sin2 = np.concatenate([sin_s, sin_s], 0)            # [128, T]
    return cos2.astype(ml_dtypes.bfloat16), sin2.astype(ml_dtypes.bfloat16)


def _diag_masks():
    # corner mask: keep where k_partition <= q_col (lower-triangular 128x128)
    p = np.arange(128)[:, None]
    qf = np.arange(128)[None, :]
    m = (p <= qf).astype(np.float32)
    return m.astype(ml_dtypes.bfloat16)                 # [128, 128]


def build_program(no_cc=False):
    nc = bacc.Bacc("TRN2", target_bir_lowering=False, debug=False,
                   num_devices=1 if no_cc else 8)

    xT_d = nc.dram_tensor("xT", [C, T], BF16, kind="ExternalInput")
    wq_d = nc.dram_tensor("Wq", [C, FL], BF16, kind="ExternalInput")
    wk_d = nc.dram_tensor("Wk", [C, FL], BF16, kind="ExternalInput")
    wv_d = nc.dram_tensor("Wv", [C, FL], BF16, kind="ExternalInput")
    wp_d = nc.dram_tensor("Wp", [C, FL], BF16, kind="ExternalInput")
    y_d = nc.dram_tensor("y", [T, FL], F32, kind="ExternalOutput")

    cos2_np, sin2_np = _rope_tables()
    cos_d = nc.inline_tensor(np.ascontiguousarray(cos2_np), "cos2")
    sin_d = nc.inline_tensor(np.ascontiguousarray(sin2_np), "sin2")
    # causal mask as a score bias: out[p,g,f] += mneg[f,p] = -30000*(p>f),
    # added to the diagonal 128x128 block by one PE matmul (keeps the
    # exp->AV chain off the vector engine)
    mneg_np = -30000.0 * (np.arange(128)[None, :] > np.arange(128)[:, None])
    mneg_d = nc.inline_tensor(
        np.ascontiguousarray(mneg_np.astype(ml_dtypes.bfloat16)), "mneg")
    id2_np = np.tile(np.eye(128, dtype=ml_dtypes.bfloat16), (1, 2))
    ident2_d = nc.inline_tensor(np.ascontiguousarray(id2_np), "ident2")
    bd_np = np.zeros((128, 128), dtype=ml_dtypes.bfloat16)
    bd_np[0:64, 0:64] = 1.0
    bd_np[64:128, 64:128] = 1.0
    onesbd_d = nc.inline_tensor(np.ascontiguousarray(bd_np), "onesbd")
    ident_d = nc.inline_tensor(
        np.ascontiguousarray(np.eye(128, dtype=ml_dtypes.bfloat16)), "ident")

    # per-pair exchange buffers
    cc_ins = [nc.dram_tensor(f"cc_in{p}", [128, T], BF16) for p in range(PAIRS)]
    cc_outs = [nc.dram_tensor(f"cc_out{p}", [2, 128, T], BF16) for p in range(PAIRS)]

    # --- PE-vs-ACT issued-work ledger (units: bf16 matmul rows = 0.4167ns) ---
    led = {"pe": 0.0, "act": 0.0}

    def mm_cost(rows):
        led["pe"] += rows

    def act_cost(free):
        led["act"] += 2.0 * free + 444.0

    with tile.TileContext(nc) as tc:
        with (
            tc.tile_pool(name="const", bufs=1) as const,
            tc.tile_pool(name="work", bufs=3) as work,
            tc.tile_pool(name="evw", bufs=6) as evw,
            tc.tile_pool(name="rope", bufs=2) as ropep,
            tc.tile_pool(name="pt", bufs=10) as ptp,
            tc.tile_pool(name="qraw", bufs=3) as qrawp,
            tc.tile_pool(name="ao", bufs=8) as aop,
            tc.tile_pool(name="ps_s2", bufs=2, space="PSUM") as ps_s2,
            tc.tile_pool(name="ps_y", bufs=2, space="PSUM") as ps_y,
            tc.tile_pool(name="ps_mm", bufs=2, space="PSUM") as ps_mm,
        ):
            early_ctx = ExitStack()
            early = early_ctx.enter_context(tc.tile_pool(name="early", bufs=1))

            # ---- inputs: wq/wk/xT first (qkv(0) consumes them first) ----
            wq_sb = early.tile([128, CT, FL], BF16)
            wk_sb = early.tile([128, CT, FL], BF16)
            xT_sb = early.tile([128, CT, T], BF16)
            # few, large DMAs: the HWDGE serializes ~625ns per DMA instruction.
            # xT halved with first halves first: qkv chunk 0/1 start earlier.
            nc.sync.dma_start(wq_sb[:], wq_d[:].rearrange("(k p) f -> p k f", p=128))
            for k in range(CT):
                nc.sync.dma_start(xT_sb[:, k, 0:T // 2],
                                  xT_d[k * 128:(k + 1) * 128, 0:T // 2])
            # wk after xT half-0: the primed q-mtile only needs wq + xT
            nc.sync.dma_start(wk_sb[:], wk_d[:].rearrange("(k p) f -> p k f", p=128))
            for k in range(CT):
                nc.sync.dma_start(xT_sb[:, k, T // 2:T],
                                  xT_d[k * 128:(k + 1) * 128, T // 2:T])
            wv_sb = early.tile([128, CT, FL], BF16)
            nc.sync.dma_start(wv_sb[:], wv_d[:].rearrange("(k p) f -> p k f", p=128))
            cos_sb = early.tile([128, T], BF16)
            nc.sync.dma_start(cos_sb[:], cos_d[:])
            sin_sb = early.tile([128, T], BF16)
            nc.sync.dma_start(sin_sb[:], sin_d[:])
            mneg_sb = const.tile([128, 128], BF16)
            nc.sync.dma_start(mneg_sb[:], mneg_d[:])
            ident2_sb = const.tile([128, 256], BF16)
            nc.sync.dma_start(ident2_sb[:], ident2_d[:])
            ident_sb = const.tile([128, 128], BF16)
            nc.sync.dma_start(ident_sb[:], ident_d[:])
            wp_sb = const.tile([128, CT, FL], BF16)
            nc.sync.dma_start(wp_sb[:], wp_d[:].rearrange("(k p) f -> p k f", p=128))

            # block-diagonal ones: one matmul sums squares of both packed heads
            ones_bd = const.tile([128, 128], BF16)
            nc.sync.dma_start(ones_bd[:], onesbd_d[:])

            qk_sb = const.tile([128, 2 * PAIRS, T], BF16)
            # v2: per tt, per pair: [A dims(64) | onesA | B dims(64) | onesB]
            v2_sb = const.tile([128, NTT, PAIRS * VW], BF16)
            aoT_sb = const.tile([128, PAIRS, T], BF16)
            # bf16 partial of the projection over already-gathered f-tiles
            ppart = const.tile([128, NTT, FL], BF16)

            # ones columns of v2 (once, before any v-proj writes)
            pstr = v2_sb.ap[0][0]
            ones_ap = bass.AP(
                tensor=v2_sb.tensor, offset=v2_sb.offset + HEAD_DIM,
                ap=[[pstr, 128], [PAIRS * VW, NTT], [VW, PAIRS], [HEAD_DIM + 1, 2]])
            nc.vector.memset(ones_ap, 1.0)

            # ---------------- unit generators (PE fillers) ----------------
            def v_unit(tt):
                pss = ps_mm.tile([128, FL], F32, tag="mm", name=f"vp{tt}")
                for k0 in range(0, CT, 2):
                    for k in (k0, k0 + 1):
                        nc.tensor.matmul(
                            pss,
                            lhsT=xT_sb[:, k, tt * 128:(tt + 1) * 128],
                            rhs=wv_sb[:, k, :],
                            start=(k == 0), stop=(k == CT - 1),
                        )
                        mm_cost(FL)
                    yield
                # drain into v2 layout: [128,4,2,64] both sides (DVE --
                # GPSIMD cannot touch PSUM on real hardware)
                src = _reap(pss, [(128, PAIRS), (64, 2), (1, 64)])
                dst = bass.AP(tensor=v2_sb.tensor,
                              offset=v2_sb.offset + tt * PAIRS * VW,
                              ap=[[pstr, 128], [VW, PAIRS], [HEAD_DIM + 1, 2], [1, 64]])
                nc.vector.tensor_copy(dst, src)
                yield

            def qkv_tail(m, pss, sq, n):
                ss = ps_y.tile([128, QCH], F32, tag="y", name=f"ss{m}_{n}")
                nc.tensor.matmul(ss, lhsT=ones_bd[:], rhs=sq[:],
                                 start=True, stop=True, skip_group_check=True)
                mm_cost(QCH)
                # rstd = (ss/64)^(-1/2) = exp(-0.5*ln(ss/64)); eps negligible.
                rr = work.tile([128, QCH], BF16, tag="rr")
                nc.scalar.activation(rr[:], ss,
                                     mybir.ActivationFunctionType.Ln,
                                     scale=1.0 / HEAD_DIM)
                act_cost(QCH)
                rstd = work.tile([128, QCH], BF16, tag="rs")
                nc.scalar.activation(rstd[:], rr[:],
                                     mybir.ActivationFunctionType.Exp,
                                     scale=-0.5)
                act_cost(QCH)
                nc.vector.tensor_mul(qk_sb[:, m, n * QCH:(n + 1) * QCH],
                                     pss, rstd[:])

            # lazy rope: one closure per DVE op, drained one per qkv yield so
            # the qkv norm-muls (psum release) never queue behind a burst
            ropeq = deque()

            def rope_mtile(m):
                src = qk_sb[:, m, :]
                sw = ropep.tile([128, T], BF16, tag="rp")
                for off in (0, 64):
                    ropeq.append(lambda o=off: nc.vector.tensor_copy(
                        sw[o:o + 32, :], src[o + 32:o + 64, :]))
                    ropeq.append(lambda o=off: nc.vector.tensor_copy(
                        sw[o + 32:o + 64, :], src[o:o + 32, :]))
                ropeq.append(lambda: nc.vector.tensor_mul(sw[:], sw[:], sin_sb[:]))
                ropeq.append(lambda: nc.vector.tensor_mul(src, src, cos_sb[:]))
                ropeq.append(lambda: nc.vector.tensor_add(src, src, sw[:]))

            def qkv_unit(m):
                # pss borrows the (pre-phase-idle) s2 slots, ss the y slots:
                # the qkv pipeline gets baseline's 4-bank depth while the
                # attention pools keep their static reservation.
                # private 2-deep pss ring per generator (q-mtiles borrow the
                # pre-phase-idle s2 slots, k-mtiles the mm slots): a slot is
                # reused only 2 chunks later, far past its tail chain
                w_sb = wq_sb if m < PAIRS else wk_sb
                mloc = (m % PAIRS) * 128
                pend = None
                ps2 = None
                for n in range(NQC):
                    if m < PAIRS:
                        # 4-deep ring: two chunks per 2-bank s2 slot
                        if n % 2 == 0:
                            ps2 = ps_s2.tile([128, 2, QCH], F32, tag="s2",
                                             name=f"qk{m}_{n}")
                        pss = ps2[:, n % 2, :]
                    else:
                        pss = ps_mm.tile([128, QCH], F32, tag="mm",
                                         name=f"qk{m}_{n}")
                    for k0 in range(0, CT, 2):
                        for k in (k0, k0 + 1):
                            nc.tensor.matmul(
                                pss,
                                lhsT=w_sb[:, k, mloc:mloc + 128],
                                rhs=xT_sb[:, k, n * QCH:(n + 1) * QCH],
                                start=(k == 0), stop=(k == CT - 1),
                                skip_group_check=True,
                            )
                            mm_cost(QCH)
                        yield
                    # square must be ACT: DVE cannot read two PSUM operands
                    sq = work.tile([128, QCH], BF16, tag="sq")
                    nc.scalar.activation(sq[:], pss,
                                         mybir.ActivationFunctionType.Square)
                    act_cost(QCH)
                    if pend is not None:
                        qkv_tail(m, *pend)
                        yield
                    pend = (pss, sq, n)
                qkv_tail(m, *pend)
                yield
                rope_mtile(m)

            def qkv_pair_gen(pn):
                # alternate q- and k-mtile steps (q primed 2 chunks ahead so
                # the two tails never bunch at the same boundary)
                a, b = qkv_unit(pn), qkv_unit(PAIRS + pn)
                for _ in range(8):
                    try:
                        next(a)
                    except StopIteration:
                        break
                while True:
                    na = nb = False
                    try:
                        next(a)
                    except StopIteration:
                        na = True
                    if ropeq:
                        ropeq.popleft()()
                    try:
                        next(b)
                    except StopIteration:
                        nb = True
                    if ropeq:
                        ropeq.popleft()()
                    if na and nb:
                        return
                    yield

            def proj_stage_gen(kfs, first_stage):
                for tt in range(NTT):
                    pss = ps_mm.tile([128, FL], F32, tag="mm",
                                     name=f"pj{kfs[0]}_{tt}")
                    for i, kf in enumerate(kfs):
                        nc.tensor.matmul(
                            pss,
                            lhsT=qk_sb[:, kf, tt * 128:(tt + 1) * 128],
                            rhs=wp_sb[:, kf, :],
                            start=(i == 0), stop=(i == len(kfs) - 1),
                        )
                        mm_cost(FL)
                        yield
                    pp = ppart[:, tt, :]
                    if first_stage:
                        nc.vector.tensor_copy(pp, pss)
                    else:
                        nc.vector.tensor_add(pp, pss, pp)
                    yield

            # ---------------- filler machinery ----------------
            # list of [label, generator, done]; fill_step picks the first
            # not-done generator whose gate is open (order = priority).
            fillers = []
            gate_from = {}   # label -> cc whose writeback must be issued
            cc_issued = set()
            cur_pair = [0]

            def fill_step():
                for ent in fillers:
                    if ent[2] or gate_from.get(ent[0], -1) not in cc_issued | {-1}:
                        continue
                    try:
                        next(ent[1])
                        return True
                    except StopIteration:
                        ent[2] = True
                        continue
                return False

            FILL_BIAS = 2000.0   # keep PE a bit ahead of ACT (rows-equiv)

            def fill():
                # pair 0: steady front-loaded drip of deferred v-proj tiles
                # (AV(j=tt) consumes v2 tile tt -- the drip beats deadlines)
                if cur_pair[0] == 0 and not fillers[0][2]:
                    for _ in range(2):
                        try:
                            next(fillers[0][1])
                        except StopIteration:
                            fillers[0][2] = True
                            break
                while led["pe"] < led["act"] + FILL_BIAS:
                    if not fill_step():
                        break

            def flush(label):
                for ent in fillers:
                    if ent[0] == label and not ent[2]:
                        for _ in ent[1]:
                            pass
                        ent[2] = True

            # ---------------- attention (flipped AV) ----------------
            # cross-pair queue of deferred work: ("tr", p, ao, qt) transposes
            # and ("cc", p) exchange launches drain one per j-iteration, so
            # cqi/pair epilogues never idle the scalar engine
            postq = deque()

            def do_transpose(p, ao, qt):
                # scratch from the mm ring: a transpose in the s2 ring would
                # halve the QK pipeline's lookahead (the scores ring is the
                # attention loop's binding resource)
                tr_t = ps_mm.tile([128, FL], F32, tag="mm",
                                  name=f"tr{p}_{qt}")
                area = tr_t[:, 0:64].bitcast(BF16)
                nc.tensor.transpose(area, ao[:].opt(), ident_sb[:])
                mm_cost(128)
                nc.vector.tensor_copy(
                    aoT_sb[:, p, qt * 128:(qt + 1) * 128], area)

            def do_cc(p):
                nc.sync.dma_start(cc_ins[p][:], aoT_sb[:, p, :])
                if not no_cc:
                    nc.gpsimd.collective_compute(
                        "AllGather",
                        mybir.AluOpType.bypass,
                        replica_groups=[[0, 1], [2, 3], [4, 5], [6, 7]],
                        ins=[cc_ins[p][:].opt()],
                        outs=[cc_outs[p][:].opt()],
                    )
                # qk slots p and 4+p are dead after attention p: receive the
                # gathered pair there (slot index == global f-tile index)
                if no_cc:
                    nc.sync.dma_start(qk_sb[:, p, :], cc_ins[p][:])
                    nc.sync.dma_start(qk_sb[:, PAIRS + p, :], cc_ins[p][:])
                else:
                    nc.sync.dma_start(qk_sb[:, p, :], cc_outs[p][0])
                    nc.sync.dma_start(qk_sb[:, PAIRS + p, :], cc_outs[p][1])
                cc_issued.add(p)

            def pop_post(curp=None):
                # keep >=3 of the CURRENT pair's transposes pending (their
                # divs need DVE time); older pairs' items drain immediately
                if not postq:
                    return False
                if len(postq) <= 3 and postq[0][1] == curp:
                    return False
                it = postq.popleft()
                if it[0] == "tr":
                    do_transpose(*it[1:])
                else:
                    do_cc(it[1])
                return True

            # AV issuance lags QK/exp by 2 k-tiles and carries across
            # q-chunk and pair boundaries: the scalar engine's exp stream
            # never waits for an epilogue
            pend_av = deque()

            def attention_pair(p):
                qT = qk_sb[:, p, :]
                kT = qk_sb[:, PAIRS + p, :]
                vbase = p * VW

                def do_qk(cqi, j):
                    qg0 = cqi * 4
                    jr = j - qg0
                    q0 = max(jr, 0) * 128
                    sq_sl = slice(cqi * QCH + q0, (cqi + 1) * QCH)
                    s2 = ps_s2.tile([128, 2, QCH], F32, tag="s2",
                                    name=f"s2_{p}_{cqi}_{j}")
                    nc.tensor.matmul(s2[:, 0, q0:QCH],
                                     lhsT=kT[0:64, j * 128:(j + 1) * 128],
                                     rhs=qT[0:64, sq_sl], start=True,
                                     stop=(jr < 0),
                                     skip_group_check=True)
                    mm_cost(QCH - q0)
                    nc.tensor.matmul(s2[:, 1, q0:QCH],
                                     lhsT=kT[64:128, j * 128:(j + 1) * 128],
                                     rhs=qT[64:128, sq_sl], start=True,
                                     stop=(jr < 0),
                                     skip_group_check=True)
                    mm_cost(QCH - q0)
                    if jr >= 0:
                        # causal bias on the diagonal block (per head --
                        # a matmul output must stay inside one psum bank)
                        for h in range(2):
                            nc.tensor.matmul(
                                s2[:, h, q0:q0 + 128], lhsT=mneg_sb[:],
                                rhs=ident2_sb[:, 0:128],
                                start=False, stop=(h == 1),
                                skip_group_check=True)
                            mm_cost(128)
                    pt = ptp.tile([128, 2, QCH], BF16, tag="pt")
                    nc.scalar.activation(pt[:, :, q0:QCH], s2[:, :, q0:QCH],
                                         mybir.ActivationFunctionType.Exp,
                                         scale=0.125)
                    act_cost(2 * (QCH - q0))
                    return pt

                for cqi in range(NQC):
                    qg0 = cqi * 4          # first global qtile of this chunk
                    kmax = qg0 + 4
                    y_t = [ps_y.tile([128, 2 * VW], F32, tag="y",
                                     name=f"yp{p}_{cqi}_{h}")
                           for h in range(2)]

                    def finalize(qt, yt, base):
                        # dens at cols base+64 and base+129
                        recip2 = work.tile([128, 2], F32, tag="rc",
                                           name=f"rc{p}_{qt}")
                        den_ap = _reap(yt[:, base + 64], [(HEAD_DIM + 1, 2)])
                        nc.vector.reciprocal_approx_fast(recip2[:], den_ap)
                        ao = aop.tile([128, 2, 64], BF16, tag="ao",
                                      name=f"ao{p}_{qt}")
                        for h in range(2):
                            nc.vector.tensor_scalar_mul(
                                ao[:, h, :],
                                _reap(yt[:, base + h * (HEAD_DIM + 1)], [(1, 64)]),
                                recip2[:, h:h + 1])
                        postq.append(("tr", p, ao, qt))

                    def issue_av(pt, j, qg0=qg0, y_t=y_t):
                        # default args bind THIS q-chunk's state: entries are
                        # popped after the loop variables have moved on
                        jr = j - qg0
                        for qtl in range(max(jr, 0), 4):
                            qt = qg0 + qtl
                            yt = y_t[qtl // 2]
                            base = (qtl % 2) * VW
                            for h in range(2):
                                o = h * (HEAD_DIM + 1)
                                # psum 'start' poisons the whole zero-region
                                # (bank): only the FIRST matmul touching each
                                # y tile may set it; the lazy zero-on-write
                                # initializes the other head/qt sub-groups
                                nc.tensor.matmul(
                                    yt[:, base + o: base + o + HEAD_DIM + 1],
                                    lhsT=pt[:, h, qtl * 128:(qtl + 1) * 128],
                                    rhs=v2_sb[:, j, vbase + o: vbase + o + HEAD_DIM + 1],
                                    start=(j == 0 and h == 0 and qtl % 2 == 0),
                                    stop=(j == qt),
                                    skip_group_check=True,
                                )
                                mm_cost(HEAD_DIM + 1)
                            if j == qt:
                                finalize(qt, yt, base)

                    for j in range(kmax):
                        pt = do_qk(cqi, j)
                        pend_av.append((issue_av, pt, j))
                        # one deferred transpose/cc per iteration
                        pop_post(p)
                        # fillers go BEFORE the lagged AV so the in-order PE
                        # chews them while exp(j-2) finishes
                        fill()
                        if len(pend_av) > 2:
                            fn, pt_, j_ = pend_av.popleft()
                            fn(pt_, j_)
                    # q-chunk epilogue: the accumulator ring recycles at the
                    # next y_t alloc, so all its AVs must be issued NOW (the
                    # transposes/cc stay deferred -- they use fresh scratch)
                    while pend_av:
                        fn, pt_, j_ = pend_av.popleft()
                        fn(pt_, j_)

            # ================= program =================
            # pre-phase: ALL qkv+norm+rope (their ACT/DVE tails hide under
            # the PE-dense mains), then v-proj tiles 0-3.  v tiles 4-15 and
            # the staged projection are the attention fillers: pure PE work
            # with no cross-engine chains to tangle with the attention loop.
            for pn in range(PAIRS):
                for _ in qkv_pair_gen(pn):
                    pass
            while ropeq:    # last pair's rope ops
                ropeq.popleft()()
            for tt in range(6):
                for _ in v_unit(tt):
                    pass

            def vdef_gen():
                for tt in range(6, NTT):
                    yield from v_unit(tt)

            fillers.append(["vdef", vdef_gen(), False])
            fillers.append(["projA", proj_stage_gen([0, PAIRS], True), False])
            fillers.append(["projB", proj_stage_gen([1, PAIRS + 1], False), False])
            fillers.append(["projC", proj_stage_gen([2, PAIRS + 2], False), False])
            gate_from.update({"projA": 0, "projB": 1, "projC": 2})

            for p in range(PAIRS):
                cur_pair[0] = p
                led["pe"] = led["act"] = 0.0
                attention_pair(p)
                # pair's transposes are already queued to postq (its last
                # q-chunk flushed pend_av), so FIFO keeps cc after them
                postq.append(("cc", p))
                if p == PAIRS - 2:
                    flush("vdef")  # safety: wv/xT die with the early pool
                    early_ctx.close()

            cur_pair[0] = PAIRS
            while postq:
                pop_post(None)
            for ent in fillers:
                flush(ent[0])

            # ---- projection tail: last pair's f-tiles + combine ----
            # psum cycles all three pools (6 slots): the adds/DMAs pipeline
            for tt in range(NTT):
                sel = tt % 3
                if sel == 0:
                    pss = ps_mm.tile([128, FL], F32, tag="mm", name=f"pf{tt}")
                elif sel == 1:
                    pss = ps_s2.tile([128, 2, QCH], F32, tag="s2",
                                     name=f"pf{tt}")[:, 0, :]
                else:
                    pss = ps_y.tile([128, FL], F32, tag="y", name=f"pf{tt}")
                for i, kf in enumerate([PAIRS - 1, 2 * PAIRS - 1]):
                    nc.tensor.matmul(
                        pss,
                        lhsT=qk_sb[:, kf, tt * 128:(tt + 1) * 128],
                        rhs=wp_sb[:, kf, :],
                        start=(i == 0), stop=(i == 1),
                    )
                    mm_cost(FL)
                ysb = evw.tile([128, FL], F32, tag="ev")
                nc.vector.tensor_add(ysb[:], pss, ppart[:, tt, :])
                nc.sync.dma_start(y_d[tt * 128:(tt + 1) * 128, :], ysb[:])

    nc.compile()
    return nc


def _prep_core_inputs(x, Wqkv, Wproj, q_norm_w, k_norm_w, core):
    b, g = core // 2, core % 2
    bf = ml_dtypes.bfloat16
    xT = np.ascontiguousarray(x[b].T).astype(bf)
    cols = slice(g * FL, (g + 1) * FL)
    wq = Wqkv[:, 0:C][:, cols] * np.tile(q_norm_w, H_LOCAL)[None, :]
    wk = Wqkv[:, C:2 * C][:, cols] * np.tile(k_norm_w, H_LOCAL)[None, :]
    wv = Wqkv[:, 2 * C:3 * C][:, cols]
    wp = Wproj[:, cols]
    return {
        "xT": xT,
        "Wq": np.ascontiguousarray(wq).astype(bf),
        "Wk": np.ascontiguousarray(wk).astype(bf),
        "Wv": np.ascontiguousarray(wv).astype(bf),
        "Wp": np.ascontiguousarray(wp).astype(bf),
    }


def kernel(x, Wqkv, Wproj, q_norm_w, k_norm_w):
    if "nc" not in _cached:
        _cached["nc"] = build_program()
    nc = _cached["nc"]

    x = np.asarray(x, dtype=np.float32)
    Wqkv = np.asarray(Wqkv, dtype=np.float32)
    Wproj = np.asarray(Wproj, dtype=np.float32)
    q_norm_w = np.asarray(q_norm_w, dtype=np.float32)
    k_norm_w = np.asarray(k_norm_w, dtype=np.float32)

    in_maps = [
        _prep_core_inputs(x, Wqkv, Wproj, q_norm_w, k_norm_w, c) for c in range(8)
    ]
    res = run_bass_kernel_spmd(nc, in_maps, list(range(8)))
    outs = res.results

    y = np.empty((B, T, C), dtype=np.float32)
    for b in range(B):
        y[b, :, 0:FL] = outs[2 * b]["y"]
        y[b, :, FL:C] = outs[2 * b + 1]["y"]
    return y


# revision 85
# speedup vs baseline: 1.0700x; 1.0051x over previous
"""Causal self-attention (QK-RMSNorm + RoPE) Trainium2 kernel.

Sharding: 8 cores = 4 batches x 2 head-groups (Megatron-style over heads).
Core c handles batch b=c//2, heads [g*8, g*8+8) with g=c%2.
Each core computes y[b, :, g*512:(g+1)*512] (output-column sharding of the
projection after a pairwise AllGather of attention outputs), so the host
only concatenates slices - no host-side arithmetic.

Perf notes (cost model charges out-free-size rows per matmul, independent of
contraction depth and output-partition count):
- AV is computed in the [q-tokens(part), head-dims(free)] orientation with a
  ones column appended to V per head: the 65-wide moving tensor makes AV cost
  65 rows/tile instead of 128-512, and the softmax denominator accumulates
  for free in column 64.  The division is then a per-partition scalar
  multiply (DVE), and the output is transposed back to [dims, tok] with
  cheap PE transposes (128 rows each) for the AllGather + projection.
- The per-head sum-of-squares for QK-RMSNorm uses one block-diagonal-ones
  matmul covering both packed heads; squares are computed on DVE in bf16
  from a Pool-engine drain of the qkv psum (keeps the scalar engine free
  for the attention exp()s, which are its binding load).
- The in-order PE is kept saturated (and in max p-state) by interleaving
  filler matmuls into the attention loop, driven by a PE-vs-ACT issued-work
  ledger: qkv of the next pair, deferred v-proj tiles, and the partial
  projection of already-gathered f-tiles (staged per AllGather arrival,
  accumulated into a bf16 partial on the Pool engine).  Only the last
  pair's two f-tiles + a DVE add remain after the final AllGather.
- Transpose scratch lives in the s2 (scores) PSUM ring, so the AV
  accumulator ring is released by the division and never serializes
  consecutive q-chunks.
"""


import numpy as np
import ml_dtypes
from collections import deque
from contextlib import ExitStack

import concourse.bass as bass
import concourse.bacc as bacc

# Force all activations into the one table set that covers Exp+Ln+Square+
# Copy+Identity, so the whole kernel needs exactly one ACT_TABLE_LOAD.
import concourse.hw_specs as _hw_specs
_orig_gat = _hw_specs.get_activation_tables

def _gat_one_set(arch):
    t = _orig_gat(arch)
    return {k: (v if k == "natural_log_exp_and_others" else set())
            for k, v in t.items()}

bacc.get_activation_tables = _gat_one_set
import concourse.mybir as mybir
import concourse.tile as tile
from concourse.bass_utils import run_bass_kernel_spmd

BF16 = mybir.dt.bfloat16
F32 = mybir.dt.float32

N_HEAD = 16
HEAD_DIM = 64
EPS = 1e-5
ROPE_BASE = 10000.0

B, T, C = 4, 2048, 1024
H_LOCAL = N_HEAD // 2          # heads per core
PAIRS = H_LOCAL // 2           # head-pairs per core (processed 2-at-a-time)
CT = C // 128                  # contraction tiles over C
FL = H_LOCAL * HEAD_DIM        # local feature width (512)
QCH = 512                      # q-chunk width
NQC = T // QCH                 # q-chunks
NKT = T // 128                 # k tiles
NTT = T // 128                 # token tiles
VW = 2 * (HEAD_DIM + 1)        # per-pair v2 width: [A dims|onesA|B dims|onesB]

_cached = {}


def _reap(ap, dims):
    """Rebuild an AP keeping tensor/offset/partition dim, with free dims
    `dims` given as (stride, size) pairs."""
    return bass.AP(tensor=ap.tensor, offset=ap.offset,
                   ap=[ap.ap[0]] + [list(d) for d in dims])


def _fbcast2(ap):
    """[128, N] AP -> [128, 2, N] with the middle (free) dim broadcast."""
    return bass.AP(
        tensor=ap.tensor, offset=ap.offset, ap=[ap.ap[0], [0, 2], ap.ap[1]]
    )


def _rope_tables():
    inv_freq = 1.0 / (ROPE_BASE ** (np.arange(0, HEAD_DIM, 2, dtype=np.float64) / HEAD_DIM))
    t = np.arange(T, dtype=np.float64)
    freqs = np.outer(t, inv_freq)                       # [T, 32]
    emb = np.concatenate([freqs, freqs], -1)            # [T, 64]
    cos = np.cos(emb).astype(np.float32).T              # [64, T]
    sin = np.sin(emb).astype(np.float32).T              # [64, T]
    cos2 = np.concatenate([cos, cos], 0)                # [128, T] two heads
    sin_s = sin.copy()
    sin_s[0:32] = -sin_s[0:32]                          # rotate-half sign
    sin2 = np.concatenate([sin_s, sin_s], 0)            # [128, T]
    return cos2.astype(ml_dtypes.bfloat16), sin2.astype(ml_dtypes.bfloat16)


def _diag_masks():
    # corner mask: keep where k_partition <= q_col (lower-triangular 128x128)
    p = np.arange(128)[:, None]
    qf = np.arange(128)[None, :]
    m = (p <= qf).astype(np.float32)
    return m.astype(ml_dtypes.bfloat16)                 # [128, 128]


def build_program(no_cc=False):
    nc = bacc.Bacc("TRN2", target_bir_lowering=False, debug=False,
                   num_devices=1 if no_cc else 8)

    xT_d = nc.dram_tensor("xT", [C, T], BF16, kind="ExternalInput")
    wq_d = nc.dram_tensor("Wq", [C, FL], BF16, kind="ExternalInput")
    wk_d = nc.dram_tensor("Wk", [C, FL], BF16, kind="ExternalInput")
    wv_d = nc.dram_tensor("Wv", [C, FL], BF16, kind="ExternalInput")
    wp_d = nc.dram_tensor("Wp", [C, FL], BF16, kind="ExternalInput")
    y_d = nc.dram_tensor("y", [T, FL], F32, kind="ExternalOutput")

    cos2_np, sin2_np = _rope_tables()
    cos_d = nc.inline_tensor(np.ascontiguousarray(cos2_np), "cos2")
    sin_d = nc.inline_tensor(np.ascontiguousarray(sin2_np), "sin2")
    # causal mask as a score bias: out[p,g,f] += mneg[f,p] = -30000*(p>f),
    # added to the diagonal 128x128 block by one PE matmul (keeps the
    # exp->AV chain off the vector engine)
    mneg_np = -30000.0 * (np.arange(128)[None, :] > np.arange(128)[:, None])
    mneg_d = nc.inline_tensor(
        np.ascontiguousarray(mneg_np.astype(ml_dtypes.bfloat16)), "mneg")
    id2_np = np.tile(np.eye(128, dtype=ml_dtypes.bfloat16), (1, 2))
    ident2_d = nc.inline_tensor(np.ascontiguousarray(id2_np), "ident2")
    bd_np = np.zeros((128, 128), dtype=ml_dtypes.bfloat16)
    bd_np[0:64, 0:64] = 1.0
    bd_np[64:128, 64:128] = 1.0
    onesbd_d = nc.inline_tensor(np.ascontiguousarray(bd_np), "onesbd")
    ident_d = nc.inline_tensor(
        np.ascontiguousarray(np.eye(128, dtype=ml_dtypes.bfloat16)), "ident")

    # per-pair exchange buffers
    cc_ins = [nc.dram_tensor(f"cc_in{p}", [128, T], BF16) for p in range(PAIRS)]
    cc_outs = [nc.dram_tensor(f"cc_out{p}", [2, 128, T], BF16) for p in range(PAIRS)]

    # --- PE-vs-ACT issued-work ledger (units: bf16 matmul rows = 0.4167ns) ---
    led = {"pe": 0.0, "act": 0.0}

    def mm_cost(rows):
        led["pe"] += rows

    def act_cost(free):
        led["act"] += 2.0 * free + 444.0

    with tile.TileContext(nc) as tc:
        with (
            tc.tile_pool(name="const", bufs=1) as const,
            tc.tile_pool(name="work", bufs=4) as work,
            tc.tile_pool(name="evw", bufs=6) as evw,
            tc.tile_pool(name="rope", bufs=2) as ropep,
            tc.tile_pool(name="pt", bufs=10) as ptp,
            tc.tile_pool(name="ao", bufs=8) as aop,
            tc.tile_pool(name="ps_s2", bufs=2, space="PSUM") as ps_s2,
            tc.tile_pool(name="ps_y", bufs=2, space="PSUM") as ps_y,
            tc.tile_pool(name="ps_mm", bufs=2, space="PSUM") as ps_mm,
        ):
            early_ctx = ExitStack()
            early = early_ctx.enter_context(tc.tile_pool(name="early", bufs=1))

            # ---- inputs: wq/wk/xT first (qkv(0) consumes them first) ----
            wq_sb = early.tile([128, CT, FL], BF16)
            wk_sb = early.tile([128, CT, FL], BF16)
            xT_sb = early.tile([128, CT, T], BF16)
            # few, large DMAs: the HWDGE serializes ~625ns per DMA instruction.
            # xT halved with first halves first: qkv chunk 0/1 start earlier.
            nc.sync.dma_start(wq_sb[:], wq_d[:].rearrange("(k p) f -> p k f", p=128))
            for k in range(CT):
                nc.sync.dma_start(xT_sb[:, k, 0:T // 2],
                                  xT_d[k * 128:(k + 1) * 128, 0:T // 2])
            # wk after xT half-0: the primed q-mtile only needs wq + xT
            nc.sync.dma_start(wk_sb[:], wk_d[:].rearrange("(k p) f -> p k f", p=128))
            for k in range(CT):
                nc.sync.dma_start(xT_sb[:, k, T // 2:T],
                                  xT_d[k * 128:(k + 1) * 128, T // 2:T])
            wv_sb = early.tile([128, CT, FL], BF16)
            nc.sync.dma_start(wv_sb[:], wv_d[:].rearrange("(k p) f -> p k f", p=128))
            cos_sb = early.tile([128, T], BF16)
            nc.sync.dma_start(cos_sb[:], cos_d[:])
            sin_sb = early.tile([128, T], BF16)
            nc.sync.dma_start(sin_sb[:], sin_d[:])
            mneg_sb = const.tile([128, 128], BF16)
            nc.sync.dma_start(mneg_sb[:], mneg_d[:])
            ident2_sb = const.tile([128, 256], BF16)
            nc.sync.dma_start(ident2_sb[:], ident2_d[:])
            ident_sb = const.tile([128, 128], BF16)
            nc.sync.dma_start(ident_sb[:], ident_d[:])
            wp_sb = const.tile([128, CT, FL], BF16)
            nc.sync.dma_start(wp_sb[:], wp_d[:].rearrange("(k p) f -> p k f", p=128))

            # block-diagonal ones: one matmul sums squares of both packed heads
            ones_bd = const.tile([128, 128], BF16)
            nc.sync.dma_start(ones_bd[:], onesbd_d[:])

            qk_sb = const.tile([128, 2 * PAIRS, T], BF16)
            # v2: per tt, per pair: [A dims(64) | onesA | B dims(64) | onesB]
            v2_sb = const.tile([128, NTT, PAIRS * VW], BF16)
            aoT_sb = const.tile([128, PAIRS, T], BF16)
            # bf16 partial of the projection over already-gathered f-tiles
            ppart = const.tile([128, NTT, FL], BF16)

            # ones columns of v2 (once, before any v-proj writes)
            pstr = v2_sb.ap[0][0]
            ones_ap = bass.AP(
                tensor=v2_sb.tensor, offset=v2_sb.offset + HEAD_DIM,
                ap=[[pstr, 128], [PAIRS * VW, NTT], [VW, PAIRS], [HEAD_DIM + 1, 2]])
            nc.vector.memset(ones_ap, 1.0)

            # ---------------- unit generators (PE fillers) ----------------
            def v_unit(tt):
                pss = ps_mm.tile([128, FL], F32, tag="mm", name=f"vp{tt}")
                for k0 in range(0, CT, 2):
                    for k in (k0, k0 + 1):
                        nc.tensor.matmul(
                            pss,
                            lhsT=xT_sb[:, k, tt * 128:(tt + 1) * 128],
                            rhs=wv_sb[:, k, :],
                            start=(k == 0), stop=(k == CT - 1),
                        )
                        mm_cost(FL)
                    yield
                # drain into v2 layout: [128,4,2,64] both sides (DVE --
                # GPSIMD cannot touch PSUM on real hardware)
                src = _reap(pss, [(128, PAIRS), (64, 2), (1, 64)])
                dst = bass.AP(tensor=v2_sb.tensor,
                              offset=v2_sb.offset + tt * PAIRS * VW,
                              ap=[[pstr, 128], [VW, PAIRS], [HEAD_DIM + 1, 2], [1, 64]])
                nc.vector.tensor_copy(dst, src)
                yield

            def qkv_tail(m, pss, sq, n):
                ss = ps_y.tile([128, QCH], F32, tag="y", name=f"ss{m}_{n}")
                nc.tensor.matmul(ss, lhsT=ones_bd[:], rhs=sq[:],
                                 start=True, stop=True, skip_group_check=True)
                mm_cost(QCH)
                # rstd = (ss/64)^(-1/2) = exp(-0.5*ln(ss/64)); eps negligible.
                rr = work.tile([128, QCH], BF16, tag="rr")
                nc.scalar.activation(rr[:], ss,
                                     mybir.ActivationFunctionType.Ln,
                                     scale=1.0 / HEAD_DIM)
                act_cost(QCH)
                rstd = work.tile([128, QCH], BF16, tag="rs")
                nc.scalar.activation(rstd[:], rr[:],
                                     mybir.ActivationFunctionType.Exp,
                                     scale=-0.5)
                act_cost(QCH)
                nc.vector.tensor_mul(qk_sb[:, m, n * QCH:(n + 1) * QCH],
                                     pss, rstd[:])

            # lazy rope: one closure per DVE op, drained one per qkv yield so
            # the qkv norm-muls (psum release) never queue behind a burst
            ropeq = deque()

            def rope_mtile(m):
                src = qk_sb[:, m, :]
                sw = ropep.tile([128, T], BF16, tag="rp")
                for off in (0, 64):
                    ropeq.append(lambda o=off: nc.vector.tensor_copy(
                        sw[o:o + 32, :], src[o + 32:o + 64, :]))
                    ropeq.append(lambda o=off: nc.vector.tensor_copy(
                        sw[o + 32:o + 64, :], src[o:o + 32, :]))
                ropeq.append(lambda: nc.vector.tensor_mul(sw[:], sw[:], sin_sb[:]))
                ropeq.append(lambda: nc.vector.tensor_mul(src, src, cos_sb[:]))
                ropeq.append(lambda: nc.vector.tensor_add(src, src, sw[:]))

            def qkv_unit(m):
                # pss borrows the (pre-phase-idle) s2 slots, ss the y slots:
                # the qkv pipeline gets baseline's 4-bank depth while the
                # attention pools keep their static reservation.
                # private 2-deep pss ring per generator (q-mtiles borrow the
                # pre-phase-idle s2 slots, k-mtiles the mm slots): a slot is
                # reused only 2 chunks later, far past its tail chain
                w_sb = wq_sb if m < PAIRS else wk_sb
                mloc = (m % PAIRS) * 128
                pend = None
                ps2 = None
                for n in range(NQC):
                    if m < PAIRS:
                        # 4-deep ring: two chunks per 2-bank s2 slot
                        if n % 2 == 0:
                            ps2 = ps_s2.tile([128, 2, QCH], F32, tag="s2",
                                             name=f"qk{m}_{n}")
                        pss = ps2[:, n % 2, :]
                    else:
                        pss = ps_mm.tile([128, QCH], F32, tag="mm",
                                         name=f"qk{m}_{n}")
                    for k0 in range(0, CT, 2):
                        for k in (k0, k0 + 1):
                            nc.tensor.matmul(
                                pss,
                                lhsT=w_sb[:, k, mloc:mloc + 128],
                                rhs=xT_sb[:, k, n * QCH:(n + 1) * QCH],
                                start=(k == 0), stop=(k == CT - 1),
                                skip_group_check=True,
                            )
                            mm_cost(QCH)
                        yield
                    # square must be ACT: DVE cannot read two PSUM operands
                    sq = work.tile([128, QCH], BF16, tag="sq")
                    nc.scalar.activation(sq[:], pss,
                                         mybir.ActivationFunctionType.Square)
                    act_cost(QCH)
                    if pend is not None:
                        qkv_tail(m, *pend)
                        yield
                    pend = (pss, sq, n)
                qkv_tail(m, *pend)
                yield
                rope_mtile(m)

            def qkv_pair_gen(pn):
                # alternate q- and k-mtile steps (q primed 2 chunks ahead so
                # the two tails never bunch at the same boundary)
                a, b = qkv_unit(pn), qkv_unit(PAIRS + pn)
                for _ in range(8):
                    try:
                        next(a)
                    except StopIteration:
                        break
                while True:
                    na = nb = False
                    try:
                        next(a)
                    except StopIteration:
                        na = True
                    if ropeq:
                        ropeq.popleft()()
                    try:
                        next(b)
                    except StopIteration:
                        nb = True
                    if ropeq:
                        ropeq.popleft()()
                    if na and nb:
                        return
                    yield

            def proj_stage_gen(kfs, first_stage):
                for tt in range(NTT):
                    pss = ps_mm.tile([128, FL], F32, tag="mm",
                                     name=f"pj{kfs[0]}_{tt}")
                    for i, kf in enumerate(kfs):
                        nc.tensor.matmul(
                            pss,
                            lhsT=qk_sb[:, kf, tt * 128:(tt + 1) * 128],
                            rhs=wp_sb[:, kf, :],
                            start=(i == 0), stop=(i == len(kfs) - 1),
                        )
                        mm_cost(FL)
                        yield
                    pp = ppart[:, tt, :]
                    if first_stage:
                        nc.vector.tensor_copy(pp, pss)
                    else:
                        nc.vector.tensor_add(pp, pss, pp)
                    yield

            # ---------------- filler machinery ----------------
            # list of [label, generator, done]; fill_step picks the first
            # not-done generator whose gate is open (order = priority).
            fillers = []
            gate_from = {}   # label -> cc whose writeback must be issued
            cc_issued = set()
            cur_pair = [0]

            def fill_step():
                for ent in fillers:
                    if ent[2] or gate_from.get(ent[0], -1) not in cc_issued | {-1}:
                        continue
                    try:
                        next(ent[1])
                        return True
                    except StopIteration:
                        ent[2] = True
                        continue
                return False

            FILL_BIAS = 2000.0   # keep PE a bit ahead of ACT (rows-equiv)

            def fill():
                # pair 0: steady front-loaded drip of deferred v-proj tiles
                # (AV(j=tt) consumes v2 tile tt -- the drip beats deadlines)
                if cur_pair[0] == 0 and not fillers[0][2]:
                    for _ in range(2):
                        try:
                            next(fillers[0][1])
                        except StopIteration:
                            fillers[0][2] = True
                            break
                while led["pe"] < led["act"] + FILL_BIAS:
                    if not fill_step():
                        break

            def flush(label):
                for ent in fillers:
                    if ent[0] == label and not ent[2]:
                        for _ in ent[1]:
                            pass
                        ent[2] = True

            # ---------------- attention (flipped AV) ----------------
            # cross-pair queue of deferred work: ("tr", p, ao, qt) transposes
            # and ("cc", p) exchange launches drain one per j-iteration, so
            # cqi/pair epilogues never idle the scalar engine
            postq = deque()

            def do_transpose(p, ao, qt):
                # scratch from the mm ring: a transpose in the s2 ring would
                # halve the QK pipeline's lookahead (the scores ring is the
                # attention loop's binding resource)
                tr_t = ps_mm.tile([128, FL], F32, tag="mm",
                                  name=f"tr{p}_{qt}")
                area = tr_t[:, 0:64].bitcast(BF16)
                nc.tensor.transpose(area, ao[:].opt(), ident_sb[:])
                mm_cost(128)
                nc.vector.tensor_copy(
                    aoT_sb[:, p, qt * 128:(qt + 1) * 128], area)

            def do_cc(p):
                nc.sync.dma_start(cc_ins[p][:], aoT_sb[:, p, :])
                if not no_cc:
                    nc.gpsimd.collective_compute(
                        "AllGather",
                        mybir.AluOpType.bypass,
                        replica_groups=[[0, 1], [2, 3], [4, 5], [6, 7]],
                        ins=[cc_ins[p][:].opt()],
                        outs=[cc_outs[p][:].opt()],
                    )
                # qk slots p and 4+p are dead after attention p: receive the
                # gathered pair there (slot index == global f-tile index)
                if no_cc:
                    nc.sync.dma_start(qk_sb[:, p, :], cc_ins[p][:])
                    nc.sync.dma_start(qk_sb[:, PAIRS + p, :], cc_ins[p][:])
                else:
                    nc.sync.dma_start(qk_sb[:, p, :], cc_outs[p][0])
                    nc.sync.dma_start(qk_sb[:, PAIRS + p, :], cc_outs[p][1])
                cc_issued.add(p)

            def pop_post(curp=None):
                # keep >=3 of the CURRENT pair's transposes pending (their
                # divs need DVE time); older pairs' items drain immediately
                if not postq:
                    return False
                if len(postq) <= 3 and postq[0][1] == curp:
                    return False
                it = postq.popleft()
                if it[0] == "tr":
                    do_transpose(*it[1:])
                else:
                    do_cc(it[1])
                return True

            # AV issuance lags QK/exp by 2 k-tiles and carries across
            # q-chunk and pair boundaries: the scalar engine's exp stream
            # never waits for an epilogue
            pend_av = deque()

            def attention_pair(p):
                qT = qk_sb[:, p, :]
                kT = qk_sb[:, PAIRS + p, :]
                vbase = p * VW

                def do_qk(cqi, j):
                    qg0 = cqi * 4
                    jr = j - qg0
                    q0 = max(jr, 0) * 128
                    sq_sl = slice(cqi * QCH + q0, (cqi + 1) * QCH)
                    s2 = ps_s2.tile([128, 2, QCH], F32, tag="s2",
                                    name=f"s2_{p}_{cqi}_{j}")
                    nc.tensor.matmul(s2[:, 0, q0:QCH],
                                     lhsT=kT[0:64, j * 128:(j + 1) * 128],
                                     rhs=qT[0:64, sq_sl], start=True,
                                     stop=(jr < 0),
                                     skip_group_check=True)
                    mm_cost(QCH - q0)
                    nc.tensor.matmul(s2[:, 1, q0:QCH],
                                     lhsT=kT[64:128, j * 128:(j + 1) * 128],
                                     rhs=qT[64:128, sq_sl], start=True,
                                     stop=(jr < 0),
                                     skip_group_check=True)
                    mm_cost(QCH - q0)
                    if jr >= 0:
                        # causal bias on the diagonal block (per head --
                        # a matmul output must stay inside one psum bank)
                        for h in range(2):
                            nc.tensor.matmul(
                                s2[:, h, q0:q0 + 128], lhsT=mneg_sb[:],
                                rhs=ident2_sb[:, 0:128],
                                start=False, stop=(h == 1),
                                skip_group_check=True)
                            mm_cost(128)
                    pt = ptp.tile([128, 2, QCH], BF16, tag="pt")
                    nc.scalar.activation(pt[:, :, q0:QCH], s2[:, :, q0:QCH],
                                         mybir.ActivationFunctionType.Exp,
                                         scale=0.125)
                    act_cost(2 * (QCH - q0))
                    return pt

                for cqi in range(NQC):
                    qg0 = cqi * 4          # first global qtile of this chunk
                    kmax = qg0 + 4
                    y_t = [ps_y.tile([128, 2 * VW], F32, tag="y",
                                     name=f"yp{p}_{cqi}_{h}")
                           for h in range(2)]

                    def finalize(qt, yt, base):
                        # dens at cols base+64 and base+129
                        recip2 = work.tile([128, 2], F32, tag="rc",
                                           name=f"rc{p}_{qt}")
                        den_ap = _reap(yt[:, base + 64], [(HEAD_DIM + 1, 2)])
                        nc.vector.reciprocal_approx_fast(recip2[:], den_ap)
                        ao = aop.tile([128, 2, 64], BF16, tag="ao",
                                      name=f"ao{p}_{qt}")
                        for h in range(2):
                            nc.vector.tensor_scalar_mul(
                                ao[:, h, :],
                                _reap(yt[:, base + h * (HEAD_DIM + 1)], [(1, 64)]),
                                recip2[:, h:h + 1])
                        postq.append(("tr", p, ao, qt))

                    def issue_av(pt, j, qg0=qg0, y_t=y_t):
                        # default args bind THIS q-chunk's state: entries are
                        # popped after the loop variables have moved on
                        jr = j - qg0
                        for qtl in range(max(jr, 0), 4):
                            qt = qg0 + qtl
                            yt = y_t[qtl // 2]
                            base = (qtl % 2) * VW
                            for h in range(2):
                                o = h * (HEAD_DIM + 1)
                                # psum 'start' poisons the whole zero-region
                                # (bank): only the FIRST matmul touching each
                                # y tile may set it; the lazy zero-on-write
                                # initializes the other head/qt sub-groups
                                nc.tensor.matmul(
                                    yt[:, base + o: base + o + HEAD_DIM + 1],
                                    lhsT=pt[:, h, qtl * 128:(qtl + 1) * 128],
                                    rhs=v2_sb[:, j, vbase + o: vbase + o + HEAD_DIM + 1],
                                    start=(j == 0 and h == 0 and qtl % 2 == 0),
                                    stop=(j == qt),
                                    skip_group_check=True,
                                )
                                mm_cost(HEAD_DIM + 1)
                            if j == qt:
                                finalize(qt, yt, base)

                    for j in range(kmax):
                        pt = do_qk(cqi, j)
                        pend_av.append((issue_av, pt, j))
                        # one deferred transpose/cc per iteration
                        pop_post(p)
                        # fillers go BEFORE the lagged AV so the in-order PE
                        # chews them while exp(j-2) finishes
                        fill()
                        if len(pend_av) > 2:
                            fn, pt_, j_ = pend_av.popleft()
                            fn(pt_, j_)
                    # q-chunk epilogue: the accumulator ring recycles at the
                    # next y_t alloc, so all its AVs must be issued NOW (the
                    # transposes/cc stay deferred -- they use fresh scratch)
                    while pend_av:
                        fn, pt_, j_ = pend_av.popleft()
                        fn(pt_, j_)

            # ================= program =================
            # pre-phase: ALL qkv+norm+rope (their ACT/DVE tails hide under
            # the PE-dense mains), then v-proj tiles 0-3.  v tiles 4-15 and
            # the staged projection are the attention fillers: pure PE work
            # with no cross-engine chains to tangle with the attention loop.
            for pn in range(PAIRS):
                for _ in qkv_pair_gen(pn):
                    pass
            while ropeq:    # last pair's rope ops
                ropeq.popleft()()
            for tt in range(6):
                for _ in v_unit(tt):
                    pass

            def vdef_gen():
                for tt in range(6, NTT):
                    yield from v_unit(tt)

            fillers.append(["vdef", vdef_gen(), False])
            fillers.append(["projA", proj_stage_gen([0, PAIRS], True), False])
            fillers.append(["projB", proj_stage_gen([1, PAIRS + 1], False), False])
            fillers.append(["projC", proj_stage_gen([2, PAIRS + 2], False), False])
            gate_from.update({"projA": 0, "projB": 1, "projC": 2})

            for p in range(PAIRS):
                cur_pair[0] = p
                led["pe"] = led["act"] = 0.0
                attention_pair(p)
                # pair's transposes are already queued to postq (its last
                # q-chunk flushed pend_av), so FIFO keeps cc after them
                postq.append(("cc", p))
                if p == PAIRS - 2:
                    flush("vdef")  # safety: wv/xT die with the early pool
                    early_ctx.close()

            cur_pair[0] = PAIRS
            while postq:
                pop_post(None)
            for ent in fillers:
                flush(ent[0])

            # ---- projection tail: last pair's f-tiles + combine ----
            # psum cycles all three pools (6 slots): the adds/DMAs pipeline
            for tt in range(NTT):
                sel = tt % 3
                if sel == 0:
                    pss = ps_mm.tile([128, FL], F32, tag="mm", name=f"pf{tt}")
                elif sel == 1:
                    pss = ps_s2.tile([128, 2, QCH], F32, tag="s2",
                                     name=f"pf{tt}")[:, 0, :]
                else:
                    pss = ps_y.tile([128, FL], F32, tag="y", name=f"pf{tt}")
                for i, kf in enumerate([PAIRS - 1, 2 * PAIRS - 1]):
                    nc.tensor.matmul(
                        pss,
                        lhsT=qk_sb[:, kf, tt * 128:(tt + 1) * 128],
                        rhs=wp_sb[:, kf, :],
                        start=(i == 0), stop=(i == 1),
                    )
                    mm_cost(FL)
                ysb = evw.tile([128, FL], F32, tag="ev")
                nc.vector.tensor_add(ysb[:], pss, ppart[:, tt, :])
                nc.sync.dma_start(y_d[tt * 128:(tt + 1) * 128, :], ysb[:])

    nc.compile()
    return nc


def _prep_core_inputs(x, Wqkv, Wproj, q_norm_w, k_norm_w, core):
    b, g = core // 2, core % 2
    bf = ml_dtypes.bfloat16
    xT = np.ascontiguousarray(x[b].T).astype(bf)
    cols = slice(g * FL, (g + 1) * FL)
    wq = Wqkv[:, 0:C][:, cols] * np.tile(q_norm_w, H_LOCAL)[None, :]
    wk = Wqkv[:, C:2 * C][:, cols] * np.tile(k_norm_w, H_LOCAL)[None, :]
    wv = Wqkv[:, 2 * C:3 * C][:, cols]
    wp = Wproj[:, cols]
    return {
        "xT": xT,
        "Wq": np.ascontiguousarray(wq).astype(bf),
        "Wk": np.ascontiguousarray(wk).astype(bf),
        "Wv": np.ascontiguousarray(wv).astype(bf),
        "Wp": np.ascontiguousarray(wp).astype(bf),
    }


def kernel(x, Wqkv, Wproj, q_norm_w, k_norm_w):
    if "nc" not in _cached:
        _cached["nc"] = build_program()
    nc = _cached["nc"]

    x = np.asarray(x, dtype=np.float32)
    Wqkv = np.asarray(Wqkv, dtype=np.float32)
    Wproj = np.asarray(Wproj, dtype=np.float32)
    q_norm_w = np.asarray(q_norm_w, dtype=np.float32)
    k_norm_w = np.asarray(k_norm_w, dtype=np.float32)

    in_maps = [
        _prep_core_inputs(x, Wqkv, Wproj, q_norm_w, k_norm_w, c) for c in range(8)
    ]
    res = run_bass_kernel_spmd(nc, in_maps, list(range(8)))
    outs = res.results

    y = np.empty((B, T, C), dtype=np.float32)
    for b in range(B):
        y[b, :, 0:FL] = outs[2 * b]["y"]
        y[b, :, FL:C] = outs[2 * b + 1]["y"]
    return y


# revision 86
# speedup vs baseline: 1.0732x; 1.0031x over previous
"""Causal self-attention (QK-RMSNorm + RoPE) Trainium2 kernel.

Sharding: 8 cores = 4 batches x 2 head-groups (Megatron-style over heads).
Core c handles batch b=c//2, heads [g*8, g*8+8) with g=c%2.
Each core computes y[b, :, g*512:(g+1)*512] (output-column sharding of the
projection after a pairwise AllGather of attention outputs), so the host
only concatenates slices - no host-side arithmetic.

Perf notes (cost model charges out-free-size rows per matmul, independent of
contraction depth and output-partition count):
- AV is computed in the [q-tokens(part), head-dims(free)] orientation with a
  ones column appended to V per head: the 65-wide moving tensor makes AV cost
  65 rows/tile instead of 128-512, and the softmax denominator accumulates
  for free in column 64.  The division is then a per-partition scalar
  multiply (DVE), and the output is transposed back to [dims, tok] with
  cheap PE transposes (128 rows each) for the AllGather + projection.
- The per-head sum-of-squares for QK-RMSNorm uses one block-diagonal-ones
  matmul covering both packed heads; squares are computed on DVE in bf16
  from a Pool-engine drain of the qkv psum (keeps the scalar engine free
  for the attention exp()s, which are its binding load).
- The in-order PE is kept saturated (and in max p-state) by interleaving
  filler matmuls into the attention loop, driven by a PE-vs-ACT issued-work
  ledger: qkv of the next pair, deferred v-proj tiles, and the partial
  projection of already-gathered f-tiles (staged per AllGather arrival,
  accumulated into a bf16 partial on the Pool engine).  Only the last
  pair's two f-tiles + a DVE add remain after the final AllGather.
- Transpose scratch lives in the s2 (scores) PSUM ring, so the AV
  accumulator ring is released by the division and never serializes
  consecutive q-chunks.
"""


import numpy as np
import ml_dtypes
from collections import deque
from contextlib import ExitStack

import concourse.bass as bass
import concourse.bacc as bacc

# Force all activations into the one table set that covers Exp+Ln+Square+
# Copy+Identity, so the whole kernel needs exactly one ACT_TABLE_LOAD.
import concourse.hw_specs as _hw_specs
_orig_gat = _hw_specs.get_activation_tables

def _gat_one_set(arch):
    t = _orig_gat(arch)
    return {k: (v if k == "natural_log_exp_and_others" else set())
            for k, v in t.items()}

bacc.get_activation_tables = _gat_one_set
import concourse.mybir as mybir
import concourse.tile as tile
from concourse.bass_utils import run_bass_kernel_spmd

BF16 = mybir.dt.bfloat16
F32 = mybir.dt.float32

N_HEAD = 16
HEAD_DIM = 64
EPS = 1e-5
ROPE_BASE = 10000.0

B, T, C = 4, 2048, 1024
H_LOCAL = N_HEAD // 2          # heads per core
PAIRS = H_LOCAL // 2           # head-pairs per core (processed 2-at-a-time)
CT = C // 128                  # contraction tiles over C
FL = H_LOCAL * HEAD_DIM        # local feature width (512)
QCH = 512                      # q-chunk width
NQC = T // QCH                 # q-chunks
NKT = T // 128                 # k tiles
NTT = T // 128                 # token tiles
VW = 2 * (HEAD_DIM + 1)        # per-pair v2 width: [A dims|onesA|B dims|onesB]

_cached = {}


def _reap(ap, dims):
    """Rebuild an AP keeping tensor/offset/partition dim, with free dims
    `dims` given as (stride, size) pairs."""
    return bass.AP(tensor=ap.tensor, offset=ap.offset,
                   ap=[ap.ap[0]] + [list(d) for d in dims])


def _fbcast2(ap):
    """[128, N] AP -> [128, 2, N] with the middle (free) dim broadcast."""
    return bass.AP(
        tensor=ap.tensor, offset=ap.offset, ap=[ap.ap[0], [0, 2], ap.ap[1]]
    )


def _rope_tables():
    inv_freq = 1.0 / (ROPE_BASE ** (np.arange(0, HEAD_DIM, 2, dtype=np.float64) / HEAD_DIM))
    t = np.arange(T, dtype=np.float64)
    freqs = np.outer(t, inv_freq)                       # [T, 32]
    emb = np.concatenate([freqs, freqs], -1)            # [T, 64]
    cos = np.cos(emb).astype(np.float32).T              # [64, T]
    sin = np.sin(emb).astype(np.float32).T              # [64, T]
    cos2 = np.concatenate([cos, cos], 0)                # [128, T] two heads
    sin_s = sin.copy()
    sin_s[0:32] = -sin_s[0:32]                          # rotate-half sign
    sin2 = np.concatenate([sin_s, sin_s], 0)            # [128, T]
    return cos2.astype(ml_dtypes.bfloat16), sin2.astype(ml_dtypes.bfloat16)


def _diag_masks():
    # corner mask: keep where k_partition <= q_col (lower-triangular 128x128)
    p = np.arange(128)[:, None]
    qf = np.arange(128)[None, :]
    m = (p <= qf).astype(np.float32)
    return m.astype(ml_dtypes.bfloat16)                 # [128, 128]


def build_program(no_cc=False):
    nc = bacc.Bacc("TRN2", target_bir_lowering=False, debug=False,
                   num_devices=1 if no_cc else 8)

    xT_d = nc.dram_tensor("xT", [C, T], BF16, kind="ExternalInput")
    wq_d = nc.dram_tensor("Wq", [C, FL], BF16, kind="ExternalInput")
    wk_d = nc.dram_tensor("Wk", [C, FL], BF16, kind="ExternalInput")
    wv_d = nc.dram_tensor("Wv", [C, FL], BF16, kind="ExternalInput")
    wp_d = nc.dram_tensor("Wp", [C, FL], BF16, kind="ExternalInput")
    y_d = nc.dram_tensor("y", [T, FL], F32, kind="ExternalOutput")

    cos2_np, sin2_np = _rope_tables()
    cos_d = nc.inline_tensor(np.ascontiguousarray(cos2_np), "cos2")
    sin_d = nc.inline_tensor(np.ascontiguousarray(sin2_np), "sin2")
    # causal mask as a score bias: out[p,g,f] += mneg[f,p] = -30000*(p>f),
    # added to the diagonal 128x128 block by one PE matmul (keeps the
    # exp->AV chain off the vector engine)
    mneg_np = -30000.0 * (np.arange(128)[None, :] > np.arange(128)[:, None])
    mneg_d = nc.inline_tensor(
        np.ascontiguousarray(mneg_np.astype(ml_dtypes.bfloat16)), "mneg")
    id2_np = np.tile(np.eye(128, dtype=ml_dtypes.bfloat16), (1, 2))
    ident2_d = nc.inline_tensor(np.ascontiguousarray(id2_np), "ident2")
    bd_np = np.zeros((128, 128), dtype=ml_dtypes.bfloat16)
    bd_np[0:64, 0:64] = 1.0
    bd_np[64:128, 64:128] = 1.0
    onesbd_d = nc.inline_tensor(np.ascontiguousarray(bd_np), "onesbd")
    ident_d = nc.inline_tensor(
        np.ascontiguousarray(np.eye(128, dtype=ml_dtypes.bfloat16)), "ident")

    # per-pair exchange buffers
    cc_ins = [nc.dram_tensor(f"cc_in{p}", [128, T], BF16) for p in range(PAIRS)]
    cc_outs = [nc.dram_tensor(f"cc_out{p}", [2, 128, T], BF16) for p in range(PAIRS)]

    # --- PE-vs-ACT issued-work ledger (units: bf16 matmul rows = 0.4167ns) ---
    led = {"pe": 0.0, "act": 0.0}

    def mm_cost(rows):
        led["pe"] += rows

    def act_cost(free):
        led["act"] += 2.0 * free + 444.0

    with tile.TileContext(nc) as tc:
        with (
            tc.tile_pool(name="const", bufs=1) as const,
            tc.tile_pool(name="work", bufs=4) as work,
            tc.tile_pool(name="evw", bufs=6) as evw,
            tc.tile_pool(name="rope", bufs=2) as ropep,
            tc.tile_pool(name="pt", bufs=10) as ptp,
            tc.tile_pool(name="ao", bufs=8) as aop,
            tc.tile_pool(name="ps_s2", bufs=2, space="PSUM") as ps_s2,
            tc.tile_pool(name="ps_y", bufs=2, space="PSUM") as ps_y,
            tc.tile_pool(name="ps_mm", bufs=2, space="PSUM") as ps_mm,
        ):
            early_ctx = ExitStack()
            early = early_ctx.enter_context(tc.tile_pool(name="early", bufs=1))

            # ---- inputs: wq/wk/xT first (qkv(0) consumes them first) ----
            wq_sb = early.tile([128, CT, FL], BF16)
            wk_sb = early.tile([128, CT, FL], BF16)
            xT_sb = early.tile([128, CT, T], BF16)
            # few, large DMAs: the HWDGE serializes ~625ns per DMA instruction.
            # xT halved with first halves first: qkv chunk 0/1 start earlier.
            nc.sync.dma_start(wq_sb[:], wq_d[:].rearrange("(k p) f -> p k f", p=128))
            for k in range(CT):
                nc.sync.dma_start(xT_sb[:, k, 0:T // 2],
                                  xT_d[k * 128:(k + 1) * 128, 0:T // 2])
            # wk after xT half-0: the primed q-mtile only needs wq + xT
            nc.sync.dma_start(wk_sb[:], wk_d[:].rearrange("(k p) f -> p k f", p=128))
            for k in range(CT):
                nc.sync.dma_start(xT_sb[:, k, T // 2:T],
                                  xT_d[k * 128:(k + 1) * 128, T // 2:T])
            wv_sb = early.tile([128, CT, FL], BF16)
            nc.sync.dma_start(wv_sb[:], wv_d[:].rearrange("(k p) f -> p k f", p=128))
            cos_sb = early.tile([128, T], BF16)
            nc.sync.dma_start(cos_sb[:], cos_d[:])
            sin_sb = early.tile([128, T], BF16)
            nc.sync.dma_start(sin_sb[:], sin_d[:])
            mneg_sb = const.tile([128, 128], BF16)
            nc.sync.dma_start(mneg_sb[:], mneg_d[:])
            ident2_sb = const.tile([128, 256], BF16)
            nc.sync.dma_start(ident2_sb[:], ident2_d[:])
            ident_sb = const.tile([128, 128], BF16)
            nc.sync.dma_start(ident_sb[:], ident_d[:])
            wp_sb = const.tile([128, CT, FL], BF16)
            nc.sync.dma_start(wp_sb[:], wp_d[:].rearrange("(k p) f -> p k f", p=128))

            # block-diagonal ones: one matmul sums squares of both packed heads
            ones_bd = const.tile([128, 128], BF16)
            nc.sync.dma_start(ones_bd[:], onesbd_d[:])

            qk_sb = const.tile([128, 2 * PAIRS, T], BF16)
            # v2: per tt, per pair: [A dims(64) | onesA | B dims(64) | onesB]
            v2_sb = const.tile([128, NTT, PAIRS * VW], BF16)
            aoT_sb = const.tile([128, PAIRS, T], BF16)
            # bf16 partial of the projection over already-gathered f-tiles
            ppart = const.tile([128, NTT, FL], BF16)

            # ones columns of v2 (once, before any v-proj writes)
            pstr = v2_sb.ap[0][0]
            ones_ap = bass.AP(
                tensor=v2_sb.tensor, offset=v2_sb.offset + HEAD_DIM,
                ap=[[pstr, 128], [PAIRS * VW, NTT], [VW, PAIRS], [HEAD_DIM + 1, 2]])
            nc.vector.memset(ones_ap, 1.0)

            # ---------------- unit generators (PE fillers) ----------------
            def v_unit(tt):
                pss = ps_mm.tile([128, FL], F32, tag="mm", name=f"vp{tt}")
                for k0 in range(0, CT, 2):
                    for k in (k0, k0 + 1):
                        nc.tensor.matmul(
                            pss,
                            lhsT=xT_sb[:, k, tt * 128:(tt + 1) * 128],
                            rhs=wv_sb[:, k, :],
                            start=(k == 0), stop=(k == CT - 1),
                        )
                        mm_cost(FL)
                    yield
                # drain into v2 layout: [128,4,2,64] both sides (DVE --
                # GPSIMD cannot touch PSUM on real hardware)
                src = _reap(pss, [(128, PAIRS), (64, 2), (1, 64)])
                dst = bass.AP(tensor=v2_sb.tensor,
                              offset=v2_sb.offset + tt * PAIRS * VW,
                              ap=[[pstr, 128], [VW, PAIRS], [HEAD_DIM + 1, 2], [1, 64]])
                nc.vector.tensor_copy(dst, src)
                yield

            def qkv_tail(m, pss, sq, n):
                ss = ps_y.tile([128, QCH], F32, tag="y", name=f"ss{m}_{n}")
                nc.tensor.matmul(ss, lhsT=ones_bd[:], rhs=sq[:],
                                 start=True, stop=True, skip_group_check=True)
                mm_cost(QCH)
                # rstd = (ss/64)^(-1/2) = exp(-0.5*ln(ss/64)); eps negligible.
                rr = work.tile([128, QCH], BF16, tag="rr")
                nc.scalar.activation(rr[:], ss,
                                     mybir.ActivationFunctionType.Ln,
                                     scale=1.0 / HEAD_DIM)
                act_cost(QCH)
                rstd = work.tile([128, QCH], BF16, tag="rs")
                nc.scalar.activation(rstd[:], rr[:],
                                     mybir.ActivationFunctionType.Exp,
                                     scale=-0.5)
                act_cost(QCH)
                nc.vector.tensor_mul(qk_sb[:, m, n * QCH:(n + 1) * QCH],
                                     pss, rstd[:])

            # lazy rope: one closure per DVE op, drained one per qkv yield so
            # the qkv norm-muls (psum release) never queue behind a burst
            ropeq = deque()

            def rope_mtile(m):
                src = qk_sb[:, m, :]
                sw = ropep.tile([128, T], BF16, tag="rp")
                for off in (0, 64):
                    ropeq.append(lambda o=off: nc.vector.tensor_copy(
                        sw[o:o + 32, :], src[o + 32:o + 64, :]))
                    ropeq.append(lambda o=off: nc.vector.tensor_copy(
                        sw[o + 32:o + 64, :], src[o:o + 32, :]))
                ropeq.append(lambda: nc.vector.tensor_mul(sw[:], sw[:], sin_sb[:]))
                ropeq.append(lambda: nc.vector.tensor_mul(src, src, cos_sb[:]))
                ropeq.append(lambda: nc.vector.tensor_add(src, src, sw[:]))

            def qkv_unit(m):
                # pss borrows the (pre-phase-idle) s2 slots, ss the y slots:
                # the qkv pipeline gets baseline's 4-bank depth while the
                # attention pools keep their static reservation.
                # private 2-deep pss ring per generator (q-mtiles borrow the
                # pre-phase-idle s2 slots, k-mtiles the mm slots): a slot is
                # reused only 2 chunks later, far past its tail chain
                w_sb = wq_sb if m < PAIRS else wk_sb
                mloc = (m % PAIRS) * 128
                pend = None
                ps2 = None
                for n in range(NQC):
                    if m < PAIRS:
                        # 4-deep ring: two chunks per 2-bank s2 slot
                        if n % 2 == 0:
                            ps2 = ps_s2.tile([128, 2, QCH], F32, tag="s2",
                                             name=f"qk{m}_{n}")
                        pss = ps2[:, n % 2, :]
                    else:
                        pss = ps_mm.tile([128, QCH], F32, tag="mm",
                                         name=f"qk{m}_{n}")
                    for k0 in range(0, CT, 2):
                        for k in (k0, k0 + 1):
                            nc.tensor.matmul(
                                pss,
                                lhsT=w_sb[:, k, mloc:mloc + 128],
                                rhs=xT_sb[:, k, n * QCH:(n + 1) * QCH],
                                start=(k == 0), stop=(k == CT - 1),
                                skip_group_check=True,
                            )
                            mm_cost(QCH)
                        yield
                    # square must be ACT: DVE cannot read two PSUM operands
                    sq = work.tile([128, QCH], BF16, tag="sq")
                    nc.scalar.activation(sq[:], pss,
                                         mybir.ActivationFunctionType.Square)
                    act_cost(QCH)
                    if pend is not None:
                        qkv_tail(m, *pend)
                        yield
                    pend = (pss, sq, n)
                qkv_tail(m, *pend)
                yield
                rope_mtile(m)

            def qkv_pair_gen(pn):
                # alternate q- and k-mtile steps (q primed 2 chunks ahead so
                # the two tails never bunch at the same boundary)
                a, b = qkv_unit(pn), qkv_unit(PAIRS + pn)
                for _ in range(8):
                    try:
                        next(a)
                    except StopIteration:
                        break
                while True:
                    na = nb = False
                    try:
                        next(a)
                    except StopIteration:
                        na = True
                    if ropeq:
                        ropeq.popleft()()
                    try:
                        next(b)
                    except StopIteration:
                        nb = True
                    if ropeq:
                        ropeq.popleft()()
                    if na and nb:
                        return
                    yield

            def proj_stage_gen(kfs, first_stage):
                for tt in range(NTT):
                    pss = ps_mm.tile([128, FL], F32, tag="mm",
                                     name=f"pj{kfs[0]}_{tt}")
                    for i, kf in enumerate(kfs):
                        nc.tensor.matmul(
                            pss,
                            lhsT=qk_sb[:, kf, tt * 128:(tt + 1) * 128],
                            rhs=wp_sb[:, kf, :],
                            start=(i == 0), stop=(i == len(kfs) - 1),
                        )
                        mm_cost(FL)
                        yield
                    pp = ppart[:, tt, :]
                    if first_stage:
                        nc.vector.tensor_copy(pp, pss)
                    else:
                        nc.vector.tensor_add(pp, pss, pp)
                    yield

            # ---------------- filler machinery ----------------
            # list of [label, generator, done]; fill_step picks the first
            # not-done generator whose gate is open (order = priority).
            fillers = []
            gate_from = {}   # label -> cc whose writeback must be issued
            cc_issued = set()
            cur_pair = [0]

            def fill_step():
                for ent in fillers:
                    if ent[2] or gate_from.get(ent[0], -1) not in cc_issued | {-1}:
                        continue
                    try:
                        next(ent[1])
                        return True
                    except StopIteration:
                        ent[2] = True
                        continue
                return False

            FILL_BIAS = 2000.0   # keep PE a bit ahead of ACT (rows-equiv)

            def fill():
                # pair 0: steady front-loaded drip of deferred v-proj tiles
                # (AV(j=tt) consumes v2 tile tt -- the drip beats deadlines)
                if cur_pair[0] == 0 and not fillers[0][2]:
                    for _ in range(2):
                        try:
                            next(fillers[0][1])
                        except StopIteration:
                            fillers[0][2] = True
                            break
                while led["pe"] < led["act"] + FILL_BIAS:
                    if not fill_step():
                        break

            def flush(label):
                for ent in fillers:
                    if ent[0] == label and not ent[2]:
                        for _ in ent[1]:
                            pass
                        ent[2] = True

            # ---------------- attention (flipped AV) ----------------
            # cross-pair queue of deferred work: ("tr", p, ao, qt) transposes
            # and ("cc", p) exchange launches drain one per j-iteration, so
            # cqi/pair epilogues never idle the scalar engine
            postq = deque()

            def do_transpose(p, ao, qt):
                # scratch from the mm ring: a transpose in the s2 ring would
                # halve the QK pipeline's lookahead (the scores ring is the
                # attention loop's binding resource)
                tr_t = ps_mm.tile([128, FL], F32, tag="mm",
                                  name=f"tr{p}_{qt}")
                area = tr_t[:, 0:64].bitcast(BF16)
                nc.tensor.transpose(area, ao[:].opt(), ident_sb[:])
                mm_cost(128)
                nc.vector.tensor_copy(
                    aoT_sb[:, p, qt * 128:(qt + 1) * 128], area)

            def do_cc(p):
                nc.sync.dma_start(cc_ins[p][:], aoT_sb[:, p, :])
                if not no_cc:
                    nc.gpsimd.collective_compute(
                        "AllGather",
                        mybir.AluOpType.bypass,
                        replica_groups=[[0, 1], [2, 3], [4, 5], [6, 7]],
                        ins=[cc_ins[p][:].opt()],
                        outs=[cc_outs[p][:].opt()],
                    )
                # qk slots p and 4+p are dead after attention p: receive the
                # gathered pair there (slot index == global f-tile index)
                if no_cc:
                    nc.sync.dma_start(qk_sb[:, p, :], cc_ins[p][:])
                    nc.sync.dma_start(qk_sb[:, PAIRS + p, :], cc_ins[p][:])
                else:
                    nc.sync.dma_start(qk_sb[:, p, :], cc_outs[p][0])
                    nc.sync.dma_start(qk_sb[:, PAIRS + p, :], cc_outs[p][1])
                cc_issued.add(p)

            def pop_post(curp=None):
                # keep >=3 of the CURRENT pair's transposes pending (their
                # divs need DVE time); older pairs' items drain immediately
                if not postq:
                    return False
                if len(postq) <= 3 and postq[0][1] == curp:
                    return False
                it = postq.popleft()
                if it[0] == "tr":
                    do_transpose(*it[1:])
                else:
                    do_cc(it[1])
                return True

            # AV issuance lags QK/exp by 2 k-tiles and carries across
            # q-chunk and pair boundaries: the scalar engine's exp stream
            # never waits for an epilogue
            pend_av = deque()

            def attention_pair(p):
                qT = qk_sb[:, p, :]
                kT = qk_sb[:, PAIRS + p, :]
                vbase = p * VW

                def do_qk(cqi, j):
                    qg0 = cqi * 4
                    jr = j - qg0
                    q0 = max(jr, 0) * 128
                    sq_sl = slice(cqi * QCH + q0, (cqi + 1) * QCH)
                    s2 = ps_s2.tile([128, 2, QCH], F32, tag="s2",
                                    name=f"s2_{p}_{cqi}_{j}")
                    nc.tensor.matmul(s2[:, 0, q0:QCH],
                                     lhsT=kT[0:64, j * 128:(j + 1) * 128],
                                     rhs=qT[0:64, sq_sl], start=True,
                                     stop=(jr < 0),
                                     skip_group_check=True)
                    mm_cost(QCH - q0)
                    nc.tensor.matmul(s2[:, 1, q0:QCH],
                                     lhsT=kT[64:128, j * 128:(j + 1) * 128],
                                     rhs=qT[64:128, sq_sl], start=True,
                                     stop=(jr < 0),
                                     skip_group_check=True)
                    mm_cost(QCH - q0)
                    if jr >= 0:
                        # causal bias on the diagonal block (per head --
                        # a matmul output must stay inside one psum bank)
                        for h in range(2):
                            nc.tensor.matmul(
                                s2[:, h, q0:q0 + 128], lhsT=mneg_sb[:],
                                rhs=ident2_sb[:, 0:128],
                                start=False, stop=(h == 1),
                                skip_group_check=True)
                            mm_cost(128)
                    pt = ptp.tile([128, 2, QCH], BF16, tag="pt")
                    nc.scalar.activation(pt[:, :, q0:QCH], s2[:, :, q0:QCH],
                                         mybir.ActivationFunctionType.Exp,
                                         scale=0.125)
                    act_cost(2 * (QCH - q0))
                    return pt

                for cqi in range(NQC):
                    qg0 = cqi * 4          # first global qtile of this chunk
                    kmax = qg0 + 4
                    y_t = [ps_y.tile([128, 2 * VW], F32, tag="y",
                                     name=f"yp{p}_{cqi}_{h}")
                           for h in range(2)]

                    def finalize(qt, yt, base):
                        # dens at cols base+64 and base+129
                        recip2 = work.tile([128, 2], F32, tag="rc",
                                           name=f"rc{p}_{qt}")
                        den_ap = _reap(yt[:, base + 64], [(HEAD_DIM + 1, 2)])
                        nc.vector.reciprocal_approx_fast(recip2[:], den_ap)
                        ao = aop.tile([128, 2, 64], BF16, tag="ao",
                                      name=f"ao{p}_{qt}")
                        for h in range(2):
                            nc.vector.tensor_scalar_mul(
                                ao[:, h, :],
                                _reap(yt[:, base + h * (HEAD_DIM + 1)], [(1, 64)]),
                                recip2[:, h:h + 1])
                        postq.append(("tr", p, ao, qt))

                    def issue_av(pt, j, qg0=qg0, y_t=y_t):
                        # default args bind THIS q-chunk's state: entries are
                        # popped after the loop variables have moved on
                        jr = j - qg0
                        for qtl in range(max(jr, 0), 4):
                            qt = qg0 + qtl
                            yt = y_t[qtl // 2]
                            base = (qtl % 2) * VW
                            for h in range(2):
                                o = h * (HEAD_DIM + 1)
                                # psum 'start' poisons the whole zero-region
                                # (bank): only the FIRST matmul touching each
                                # y tile may set it; the lazy zero-on-write
                                # initializes the other head/qt sub-groups
                                nc.tensor.matmul(
                                    yt[:, base + o: base + o + HEAD_DIM + 1],
                                    lhsT=pt[:, h, qtl * 128:(qtl + 1) * 128],
                                    rhs=v2_sb[:, j, vbase + o: vbase + o + HEAD_DIM + 1],
                                    start=(j == 0 and h == 0 and qtl % 2 == 0),
                                    stop=(j == qt),
                                    skip_group_check=True,
                                )
                                mm_cost(HEAD_DIM + 1)
                            if j == qt:
                                finalize(qt, yt, base)

                    for j in range(kmax):
                        pt = do_qk(cqi, j)
                        pend_av.append((issue_av, pt, j))
                        # one deferred transpose/cc per iteration
                        pop_post(p)
                        # fillers go BEFORE the lagged AV so the in-order PE
                        # chews them while exp(j-2) finishes
                        fill()
                        if len(pend_av) > 3:
                            fn, pt_, j_ = pend_av.popleft()
                            fn(pt_, j_)
                    # q-chunk epilogue: the accumulator ring recycles at the
                    # next y_t alloc, so all its AVs must be issued NOW (the
                    # transposes/cc stay deferred -- they use fresh scratch)
                    while pend_av:
                        fn, pt_, j_ = pend_av.popleft()
                        fn(pt_, j_)

            # ================= program =================
            # pre-phase: ALL qkv+norm+rope (their ACT/DVE tails hide under
            # the PE-dense mains), then v-proj tiles 0-3.  v tiles 4-15 and
            # the staged projection are the attention fillers: pure PE work
            # with no cross-engine chains to tangle with the attention loop.
            for pn in range(PAIRS):
                for _ in qkv_pair_gen(pn):
                    pass
            while ropeq:    # last pair's rope ops
                ropeq.popleft()()
            for tt in range(6):
                for _ in v_unit(tt):
                    pass

            def vdef_gen():
                for tt in range(6, NTT):
                    yield from v_unit(tt)

            fillers.append(["vdef", vdef_gen(), False])
            fillers.append(["projA", proj_stage_gen([0, PAIRS], True), False])
            fillers.append(["projB", proj_stage_gen([1, PAIRS + 1], False), False])
            fillers.append(["projC", proj_stage_gen([2, PAIRS + 2], False), False])
            gate_from.update({"projA": 0, "projB": 1, "projC": 2})

            for p in range(PAIRS):
                cur_pair[0] = p
                led["pe"] = led["act"] = 0.0
                attention_pair(p)
                # pair's transposes are already queued to postq (its last
                # q-chunk flushed pend_av), so FIFO keeps cc after them
                postq.append(("cc", p))
                if p == PAIRS - 2:
                    flush("vdef")  # safety: wv/xT die with the early pool
                    early_ctx.close()

            cur_pair[0] = PAIRS
            while postq:
                pop_post(None)
            for ent in fillers:
                flush(ent[0])

            # ---- projection tail: last pair's f-tiles + combine ----
            # psum cycles all three pools (6 slots): the adds/DMAs pipeline
            for tt in range(NTT):
                sel = tt % 3
                if sel == 0:
                    pss = ps_mm.tile([128, FL], F32, tag="mm", name=f"pf{tt}")
                elif sel == 1:
                    pss = ps_s2.tile([128, 2, QCH], F32, tag="s2",
                                     name=f"pf{tt}")[:, 0, :]
                else:
                    pss = ps_y.tile([128, FL], F32, tag="y", name=f"pf{tt}")
                for i, kf in enumerate([PAIRS - 1, 2 * PAIRS - 1]):
                    nc.tensor.matmul(
                        pss,
                        lhsT=qk_sb[:, kf, tt * 128:(tt + 1) * 128],
                        rhs=wp_sb[:, kf, :],
                        start=(i == 0), stop=(i == 1),
                    )
                    mm_cost(FL)
                ysb = evw.tile([128, FL], F32, tag="ev")
                nc.vector.tensor_add(ysb[:], pss, ppart[:, tt, :])
                nc.sync.dma_start(y_d[tt * 128:(tt + 1) * 128, :], ysb[:])

    nc.compile()
    return nc


def _prep_core_inputs(x, Wqkv, Wproj, q_norm_w, k_norm_w, core):
    b, g = core // 2, core % 2
    bf = ml_dtypes.bfloat16
    xT = np.ascontiguousarray(x[b].T).astype(bf)
    cols = slice(g * FL, (g + 1) * FL)
    wq = Wqkv[:, 0:C][:, cols] * np.tile(q_norm_w, H_LOCAL)[None, :]
    wk = Wqkv[:, C:2 * C][:, cols] * np.tile(k_norm_w, H_LOCAL)[None, :]
    wv = Wqkv[:, 2 * C:3 * C][:, cols]
    wp = Wproj[:, cols]
    return {
        "xT": xT,
        "Wq": np.ascontiguousarray(wq).astype(bf),
        "Wk": np.ascontiguousarray(wk).astype(bf),
        "Wv": np.ascontiguousarray(wv).astype(bf),
        "Wp": np.ascontiguousarray(wp).astype(bf),
    }


def kernel(x, Wqkv, Wproj, q_norm_w, k_norm_w):
    if "nc" not in _cached:
        _cached["nc"] = build_program()
    nc = _cached["nc"]

    x = np.asarray(x, dtype=np.float32)
    Wqkv = np.asarray(Wqkv, dtype=np.float32)
    Wproj = np.asarray(Wproj, dtype=np.float32)
    q_norm_w = np.asarray(q_norm_w, dtype=np.float32)
    k_norm_w = np.asarray(k_norm_w, dtype=np.float32)

    in_maps = [
        _prep_core_inputs(x, Wqkv, Wproj, q_norm_w, k_norm_w, c) for c in range(8)
    ]
    res = run_bass_kernel_spmd(nc, in_maps, list(range(8)))
    outs = res.results

    y = np.empty((B, T, C), dtype=np.float32)
    for b in range(B):
        y[b, :, 0:FL] = outs[2 * b]["y"]
        y[b, :, FL:C] = outs[2 * b + 1]["y"]
    return y
